# revision 68
# baseline (speedup 1.0000x reference)
"""Trainium2 Bass kernel for nn_BiVision_VQA2 (B=64,T=32,D=768,N=901).

Data-parallel over batch: 8 batch elems per core x 8 cores.
Key math simplifications (validated vs reference in numpy, rel err ~1e-6):
  - ga/go attention use a single key token -> softmax==1 -> those paths are
    linear in cls; question_embeds is mathematically unused.
  - GRU input `a` is constant over time; wx computed once.
  - local attention: scores = (qemb @ W0_h) @ W1_h^T / sqrt(dk) @ X^T ;
    row-constant score terms (K bias, Q.b1) drop out of softmax; query
    pooling applied to the attention matrix before the @X contraction;
    constant bias terms folded into one vector.
"""

import numpy as np
from contextlib import ExitStack

import concourse.bass as bass
import concourse.tile as tile
from concourse import bacc, mybir
from concourse.bass_utils import run_bass_kernel_spmd
from concourse.masks import make_identity

FP = mybir.dt.float32
FPR = mybir.dt.float32r
OP = mybir.AluOpType
AF = mybir.ActivationFunctionType
BF = mybir.dt.bfloat16
F8 = mybir.dt.float8e4
DR = mybir.MatmulPerfMode.DoubleRow

NCORES = 8
BL = 8
D = 768
T = 32
G = 3 * D
NK = 900
NH = 2
DK = 384
ET = D // 128
RQ = BL * T
USE_FPR = True


def chunks(total):
    out, o = [], 0
    while o < total:
        w = min(512, total - o)
        out.append((o, w))
        o += w
    return out


CH_G = chunks(G)
CH_NK = [(0, 512), (512, 388)]
CH_D = [(0, 512), (512, 256)]


def _r(ap):
    return ap.bitcast(FPR) if USE_FPR else ap


def kchunks(n):
    out, o = [], 0
    while o < n:
        out.append((o, min(128, n - o)))
        o += 128
    return out


import os
PHASES = int(os.environ.get("KPHASES", "4"))


def build():
    nc = bacc.Bacc("TRN2", target_bir_lowering=False, debug=False,
                   enable_asserts=False, num_swdge_queues=4)

    def gd(q, out, in_, **kw):
        inst = nc.gpsimd.dma_start(out, in_, **kw)
        if q:
            inst.ins.queue = f"qPoolDynamic{q}"
        return inst

    img = nc.dram_tensor("img", [BL, 901, D], FP, kind="ExternalInput").ap()
    h0 = nc.dram_tensor("h0", [BL, D], FP, kind="ExternalInput").ap()
    w_ih = nc.dram_tensor("gru_w_ih", [G, D], FP, kind="ExternalInput").ap()
    w_hh = nc.dram_tensor("gru_w_hh", [G, D], FP, kind="ExternalInput").ap()
    b_ih = nc.dram_tensor("gru_b_ih", [G], FP, kind="ExternalInput").ap()
    b_hh = nc.dram_tensor("gru_b_hh", [G], FP, kind="ExternalInput").ap()
    ga_w = nc.dram_tensor("ga_w", [4, D, D], FP, kind="ExternalInput").ap()
    ga_b = nc.dram_tensor("ga_b", [4, D], FP, kind="ExternalInput").ap()
    ga_pool = nc.dram_tensor("ga_pool", [1], FP, kind="ExternalInput").ap()
    la_w = nc.dram_tensor("la_w", [4, D, D], FP, kind="ExternalInput").ap()
    la_b = nc.dram_tensor("la_b", [4, D], FP, kind="ExternalInput").ap()
    la_pool = nc.dram_tensor("la_pool", [T], FP, kind="ExternalInput").ap()
    go_w = nc.dram_tensor("go_w", [4, D, D], FP, kind="ExternalInput").ap()
    go_b = nc.dram_tensor("go_b", [4, D], FP, kind="ExternalInput").ap()
    go_pool = nc.dram_tensor("go_pool", [T], FP, kind="ExternalInput").ap()
    f1_w = nc.dram_tensor("f1_w", [2 * D, 1024], FP, kind="ExternalInput").ap()
    f1_b = nc.dram_tensor("f1_b", [1024], FP, kind="ExternalInput").ap()
    f2_w = nc.dram_tensor("f2_w", [1024, 512], FP, kind="ExternalInput").ap()
    f2_b = nc.dram_tensor("f2_b", [512], FP, kind="ExternalInput").ap()
    f3_w = nc.dram_tensor("f3_w", [512, 1024], FP, kind="ExternalInput").ap()
    f3_b = nc.dram_tensor("f3_b", [1024], FP, kind="ExternalInput").ap()
    out_d = nc.dram_tensor("out", [BL, 1024], FP, kind="ExternalOutput").ap()

    with tile.TileContext(nc) as tc, ExitStack() as ctx:
        cpool = ctx.enter_context(tc.tile_pool(name="const", bufs=1))
        gstate = ctx.enter_context(tc.tile_pool(name="gstate", bufs=2))
        xall = ctx.enter_context(tc.tile_pool(name="xall", bufs=1))
        tailw = ctx.enter_context(tc.tile_pool(name="tailw", bufs=1))
        psB = ctx.enter_context(tc.tile_pool(name="psB", bufs=2, space="PSUM"))
        psC = ctx.enter_context(tc.tile_pool(name="psC", bufs=1, space="PSUM"))

        ident = cpool.tile([128, 128], FP, tag="ident")
        make_identity(nc, ident[:])
        ones1 = cpool.tile([1, 128], FP, tag="ones1")
        nc.vector.memset(ones1[:], 1.0)
        onesT = cpool.tile([T, 128], FP, tag="onesT")
        nc.vector.memset(onesT[:], 1.0)
        identr = cpool.tile([128, 128], FP, tag="identr")
        nc.vector.tensor_copy(_r(identr[:]), ident[:])
        identb = cpool.tile([128, 128], BF, tag="identb")
        nc.vector.tensor_copy(identb[:], ident[:])
        ones1r = cpool.tile([1, 128], FP, tag="ones1r")
        nc.vector.tensor_copy(_r(ones1r[:]), ones1[:])

        def colvec(dram_1d, n, tag):
            nt = n // 128
            t_ = cpool.tile([128, nt], FP, tag=tag)
            for j in range(nt):
                nc.sync.dma_start(t_[:, j:j + 1], dram_1d[j * 128:(j + 1) * 128][:, None])
            return t_

        b2gaT = colvec(ga_b[2], D, "b2gaT")
        b3gaT = colvec(ga_b[3], D, "b3gaT")
        b2goT = colvec(go_b[2], D, "b2goT")
        b3goT = colvec(go_b[3], D, "b3goT")
        b0laT = colvec(la_b[0], D, "b0laT")
        b2laT = colvec(la_b[2], D, "b2laT")
        b3laT = colvec(la_b[3], D, "b3laT")
        b1fT = colvec(f1_b, 1024, "b1fT")
        b2fT = colvec(f2_b, 512, "b2fT")
        b3fT = colvec(f3_b, 1024, "b3fT")

        lapool_c = cpool.tile([T, 1], FP, tag="lapool_c")
        nc.sync.dma_start(lapool_c[:], la_pool[:][:, None])
        gopool_c = cpool.tile([T, 1], FP, tag="gopool_c")
        nc.sync.dma_start(gopool_c[:], go_pool[:][:, None])
        gapool_c = cpool.tile([1, 1], FP, tag="gapool_c")
        nc.sync.dma_start(gapool_c[:], ga_pool[:][:, None])

        def sum_bcast(vcol, k, tag):
            p = psC.tile([128, 1], FP, tag="pd")
            lhs = onesT if k == T else ones1
            nc.tensor.matmul(p[:], lhs[:k, :], vcol[:k, :], start=True, stop=True)
            s = cpool.tile([128, 1], FP, tag=tag)
            nc.vector.tensor_copy(s[:], p[:])
            return s

        Sla = sum_bcast(lapool_c, T, "Sla")
        Sgo = sum_bcast(gopool_c, T, "Sgo")
        Sga = sum_bcast(gapool_c, 1, "Sga")

        pmask = cpool.tile([64, 2], FP, tag="pmask")
        nc.vector.memset(pmask[:], 0.0)
        nc.sync.dma_start(pmask[0:T, 0:1], la_pool[:][:, None])
        nc.sync.dma_start(pmask[T:2 * T, 1:2], la_pool[:][:, None])

        qemb8 = cpool.tile([128, ET, BL, T], F8, tag="qemb8")
        goutT = cpool.tile([128, ET, BL], BF, tag="goutT")
        aT = cpool.tile([128, ET, BL], FP, tag="aT")
        bhhN_r = cpool.tile([1, D], FP, tag="bhhN_r")

        # img patch tokens, all 8 batch elems, prefetched early (bf16)
        XnA = xall.tile([128, BL, 8, D], BF, tag="XnA")
        # early-persisted tail weights (DMAs issued pre-loop, overlap GRU)
        W2b = tailw.tile([128, ET, D], BF, tag="W2b")
        W3b = tailw.tile([128, ET, D], BF, tag="W3b")
        f3b = tailw.tile([128, 4, 1024], BF, tag="f3b")

        # ================= phase A: cls -> a (ga path only) ===============
        clsTb = cpool.tile([128, ET, BL], BF, tag="clsTb")

        def dense_T(pool, wdt, w_nat_dram, rhsT, biasT, scaleT, otile, wtag, dmaq):
            wsb = pool.tile([128, ET, D], wdt, tag=wtag)
            for c in range(ET):
                dmaq.dma_start(wsb[:, c, :], w_nat_dram[128 * c:128 * (c + 1), :])
            for mt in range(ET):
                p = psC.tile([128, BL], FP, tag="pd")
                for kt in range(ET):
                    nc.tensor.matmul(p[:], wsb[:, kt, 128 * mt:128 * (mt + 1)],
                                     rhsT[:, kt, :], start=(kt == 0), stop=(kt == ET - 1))
                if scaleT is None:
                    nc.vector.tensor_scalar(otile[:, mt, :], p[:], biasT[:, mt:mt + 1],
                                            None, OP.add)
                else:
                    nc.vector.tensor_scalar(otile[:, mt, :], p[:], biasT[:, mt:mt + 1],
                                            scaleT[:, 0:1], OP.add, OP.mult)

        with tc.tile_pool(name="ph0", bufs=1) as ph0:
            clsn = ph0.tile([BL, D], FP, tag="clsn")
            nc.sync.dma_start(clsn[:], img[0:BL, 0, :])
            ptr = psC.tile([128, 512], FP, tag="pd")
            for kt in range(ET):
                nc.tensor.matmul(ptr[:, 8 * kt:8 * kt + 8], clsn[:, 128 * kt:128 * (kt + 1)],
                                 ident[:BL, :BL], is_transpose=True, skip_group_check=True)
            clsT = ph0.tile([128, ET, BL], FP, tag="clsT")
            nc.vector.tensor_copy(clsT[:].rearrange("p a b -> p (a b)"), ptr[:, :8 * ET])
            nc.scalar.copy(clsTb[:], clsT[:])
            A2T = ph0.tile([128, ET, BL], BF, tag="A2T")
            gaw2 = ph0.tile([128, ET, D], BF, tag="wA")
            for c in range(ET):
                gd(1 + c % 3, gaw2[:, c, :], ga_w[2][128 * c:128 * (c + 1), :])
            gaw3 = ph0.tile([128, ET, D], BF, tag="wB")
            for c in range(ET):
                gd(1 + c % 3, gaw3[:, c, :], ga_w[3][128 * c:128 * (c + 1), :])
            for mt in range(ET):
                p = psC.tile([128, BL], FP, tag="pd")
                for kt in range(ET):
                    nc.tensor.matmul(p[:], gaw2[:, kt, 128 * mt:128 * (mt + 1)],
                                     clsTb[:, kt, :], start=(kt == 0), stop=(kt == ET - 1))
                nc.vector.tensor_scalar(A2T[:, mt, :], p[:], b2gaT[:, mt:mt + 1],
                                        None, OP.add)
            for mt in range(ET):
                p = psC.tile([128, BL], FP, tag="pd")
                for kt in range(ET):
                    nc.tensor.matmul(p[:], gaw3[:, kt, 128 * mt:128 * (mt + 1)],
                                     A2T[:, kt, :], start=(kt == 0), stop=(kt == ET - 1))
                nc.vector.tensor_scalar(aT[:, mt, :], p[:], b3gaT[:, mt:mt + 1],
                                        Sga[:, 0:1], OP.add, OP.mult)

        # ================= phase B: GRU (fp8 DoubleRow) ===================
        cde = ctx.enter_context(tc.tile_pool(name="cde", bufs=1))
        QtT = cde.tile([128, ET, NH * RQ], BF, tag="QtT")
        prep_cm = tc.tile_pool(name="prep", bufs=1)
        prep = prep_cm.__enter__()
        wb8_cm = tc.tile_pool(name="wb8", bufs=1)
        wb8 = wb8_cm.__enter__()
        WhhT8 = wb8.tile([128, ET, G], F8, tag="WhhT8")

        with tc.tile_pool(name="wpro", bufs=1) as wpro, \
             tc.tile_pool(name="wnat", bufs=4) as wnat:
            combr = wpro.tile([1, 2 * D], FP, tag="combr")
            nc.sync.dma_start(combr[:], b_ih[0:2 * D][None, :])
            nc.gpsimd.dma_start(combr[:], b_hh[0:2 * D][None, :], accum_op=OP.add)
            bhhN_t = wpro.tile([1, D], FP, tag="bhhN_t")
            nc.sync.dma_start(bhhN_t[:], b_hh[2 * D:3 * D][None, :])
            nc.vector.tensor_copy(_r(bhhN_r[:]), bhhN_t[:])
            bihN = wpro.tile([1, D], FP, tag="bhhN_t", name="bihN")
            nc.sync.dma_start(bihN[:], b_ih[2 * D:3 * D][None, :])

            aT8 = cpool.tile([128, ET, 128], F8, tag="aT8")
            nc.vector.memset(aT8[:].rearrange("p a b -> p (a b)"), 0.0)
            nc.vector.tensor_copy(aT8[:, :, 0:BL], aT[:])

            WihT8 = prep.tile([128, ET, G], F8, tag="gow3", name="WihT8")

            def build_W8(w_dram, dst):
                jts = kchunks(G)
                for g0 in range(0, len(jts), 4):
                    grp = jts[g0:g0 + 4]
                    nats = []
                    for qi, (j0, jw) in enumerate(grp):
                        wn = wnat.tile([128, D], BF, tag="wn")
                        gd(qi % 4, wn[:jw, :], w_dram[j0:j0 + jw, :])
                        nats.append((wn, j0, jw))
                    for et in range(ET):
                        pt = psB.tile([128, 512], BF, tag="ptw")
                        for i, (wn, j0, jw) in enumerate(nats):
                            nc.tensor.matmul(pt[:, 128 * i:128 * i + jw],
                                             wn[:jw, 128 * et:128 * (et + 1)],
                                             identb[:jw, :jw], is_transpose=True,
                                             skip_group_check=True)
                        w0 = grp[0][0]
                        wlen = sum(jw for (_, _, jw) in nats)
                        if et % 2 == 0:
                            nc.vector.tensor_copy(dst[:, et, w0:w0 + wlen], pt[:, :wlen])
                        else:
                            nc.scalar.copy(dst[:, et, w0:w0 + wlen], pt[:, :wlen])

            build_W8(w_ih, WihT8)

            # wx (+ all biases folded): rz sections get bih+bhh, n gets bih
            wxbRZ = prep.tile([BL, 2 * D], FP, tag="wxbRZ")
            wxbN = prep.tile([BL, D], FP, tag="wxbN")
            for (j0, jw) in CH_G:
                p = psC.tile([128, 512], FP, tag="pd")
                for kp in range(3):
                    nc.tensor.matmul(p[:, :jw], aT8[:, 2 * kp:2 * kp + 2, :],
                                     WihT8[:, 2 * kp:2 * kp + 2, j0:j0 + jw],
                                     start=(kp == 0), stop=False, perf_mode=DR)
                src = combr[:, j0:j0 + jw] if j0 < 2 * D else bihN[:, j0 - 2 * D:j0 - 2 * D + jw]
                nc.tensor.matmul(p[:, :jw], ones1[:1, :], src,
                                 start=False, stop=True)
                if j0 < 2 * D:
                    nc.vector.tensor_copy(_r(wxbRZ[:, j0:j0 + jw]), p[:BL, :jw])
                else:
                    nc.vector.tensor_copy(wxbN[:, j0 - 2 * D:j0 - 2 * D + jw], p[:BL, :jw])

            build_W8(w_hh, WhhT8)

            hnat = prep.tile([BL, D], FP, tag="W08", name="hnat")
            nc.sync.dma_start(hnat[:], h0[:, :])
            ptr0 = psC.tile([128, 512], FP, tag="pd")
            for kt in range(ET):
                nc.tensor.matmul(ptr0[:, 8 * kt:8 * kt + 8], hnat[:, 128 * kt:128 * (kt + 1)],
                                 ident[:BL, :BL], is_transpose=True, skip_group_check=True)
            hT32 = gstate.tile([128, ET, BL], FP, tag="hT32")
            nc.vector.tensor_copy(hT32[:].rearrange("p a b -> p (a b)"), ptr0[:, :8 * ET])
            h8_a = cpool.tile([128, ET, 128], F8, tag="h8_a")
            h8_b = cpool.tile([128, ET, 128], F8, tag="h8_b")
            nc.vector.memset(h8_a[:].rearrange("p a b -> p (a b)"), 0.0)
            nc.vector.memset(h8_b[:].rearrange("p a b -> p (a b)"), 0.0)
            nc.scalar.copy(h8_a[:, :, 0:BL], hT32[:])
            h8 = h8_a

        # ---- pre-loop early DMA emission (overlaps the GRU steps) --------
        for b in range(BL):
            nc.vector.memset(XnA[:, b, 7, :], 0.0)
            for c, (k0, kw) in enumerate(kchunks(NK)):
                gd(1 + (b * 8 + c) % 3, XnA[:kw, b, c, :], img[b, 1 + k0:1 + k0 + kw, :])
        W08 = prep.tile([128, ET, D], F8, tag="W08")
        for c in range(ET):
            gd(1 + c % 3, W08[:, c, :], la_w[0][128 * c:128 * (c + 1), :])
        gow2 = prep.tile([128, ET, D], BF, tag="gow2")
        for c in range(ET):
            gd(1 + c % 3, gow2[:, c, :], go_w[2][128 * c:128 * (c + 1), :])
        for c in range(ET):
            gd(1 + c % 3, W2b[:, c, :], la_w[2][128 * c:128 * (c + 1), :])
        for c in range(ET):
            gd(1 + c % 3, W3b[:, c, :], la_w[3][128 * c:128 * (c + 1), :])
        for c in range(4):
            gd(1 + c % 3, f3b[:, c, :], f3_w[128 * c:128 * (c + 1), :])
        # stall-prone loads (wait on in-loop readers) go last on queue 0
        gow3 = prep.tile([128, ET, D], BF, tag="gow3")
        for c in range(ET):
            gd(0, gow3[:, c, :], go_w[3][128 * c:128 * (c + 1), :])
        # W1 lands in gow2's buffer once the go stage-1 matmuls are done
        W1n = prep.tile([128, ET, D], BF, tag="gow2", name="W1n")
        for c in range(ET):
            gd(0, W1n[:, c, :], la_w[1][128 * c:128 * (c + 1), :])

        with tc.tile_pool(name="g1", bufs=1) as g1, \
             tc.tile_pool(name="psG", bufs=1, space="PSUM") as psG:

            # section psums: A = j[0:1024] (r + z1), Z = j[1024:1536] (z2),
            # N = j[1536:2304] (n); emission order A0 A1 N0 N1 Z
            STEP_CHUNKS = [("a", 0, 0, 512), ("a", 512, 512, 512),
                           ("n", 0, 1536, 512), ("n", 512, 2048, 256),
                           ("z", 0, 1024, 512)]
            KSTEPS = int(os.environ.get("KSTEPS", str(T)))
            KFILL = int(os.environ.get("KFILL", "4"))
            fill_i = 0
            for t in range(KSTEPS):
                hp8, hp32 = h8, hT32
                pA = psG.tile([128, 1024], FP, tag="a")
                pN = psG.tile([128, D], FP, tag="n")
                pZ = psG.tile([128, 512], FP, tag="z")
                tiles = {"a": pA, "n": pN, "z": pZ}
                for (sec, c0, j0, jw) in STEP_CHUNKS:
                    p = tiles[sec]
                    for kp in range(3):
                        nc.tensor.matmul(p[:, c0:c0 + jw], hp8[:, 2 * kp:2 * kp + 2, :],
                                         WhhT8[:, 2 * kp:2 * kp + 2, j0:j0 + jw],
                                         start=(kp == 0), stop=False, perf_mode=DR)
                    if j0 >= 2 * D:
                        nc.tensor.matmul(p[:, c0:c0 + jw], _r(ones1r[:1, :]),
                                         _r(bhhN_r[:, j0 - 2 * D:j0 - 2 * D + jw]),
                                         start=False, stop=True)
                    else:
                        nc.tensor.matmul(p[:, c0:c0 + jw], _r(identr[:BL, :]),
                                         _r(wxbRZ[:, j0:j0 + jw]), start=False, stop=True)
                r_sig = g1.tile([BL, D], FP, tag="rsig")
                nc.scalar.activation(r_sig[:], pA[:BL, 0:768], AF.Sigmoid)
                z_nat = g1.tile([BL, D], FP, tag="znat")
                nc.scalar.activation(z_nat[:, 0:256], pA[:BL, 768:1024], AF.Sigmoid)
                rwn = g1.tile([BL, D], FP, tag="rwn")
                nc.vector.tensor_mul(rwn[:], r_sig[:], pN[:BL, :])
                npre = rwn
                nc.vector.tensor_add(npre[:], rwn[:], wxbN[:])
                nc.scalar.activation(z_nat[:, 256:768], pZ[:BL, :], AF.Sigmoid)
                nt_ = g1.tile([BL, D], FP, tag="nt")
                nc.scalar.activation(nt_[:, 0:512], npre[:, 0:512], AF.Tanh)
                nc.scalar.activation(nt_[:, 512:768], npre[:, 512:768], AF.Tanh)
                # fills between mm block and transposes keep PE p-state hot
                for _ in range(KFILL // 2):
                    pf = psB.tile([128, 512], FP, tag="ptw")
                    for kp in range(3):
                        nc.tensor.matmul(pf[:, 0:512], hp8[:, 2 * kp:2 * kp + 2, :],
                                         WhhT8[:, 2 * kp:2 * kp + 2, 512 * (fill_i % 4):512 * (fill_i % 4) + 512],
                                         start=(kp == 0), stop=(kp == 2), perf_mode=DR)
                    fill_i += 1
                ptt = psC.tile([128, 512], FP, tag="pd")
                for kt in range(ET):
                    nc.tensor.matmul(ptt[:, 8 * kt:8 * kt + 8], z_nat[:, 128 * kt:128 * (kt + 1)],
                                     ident[:BL, :BL], is_transpose=True, skip_group_check=True)
                for kt in range(ET):
                    nc.tensor.matmul(ptt[:, 64 + 8 * kt:64 + 8 * kt + 8],
                                     nt_[:, 128 * kt:128 * (kt + 1)],
                                     ident[:BL, :BL], is_transpose=True, skip_group_check=True)
                zT = ptt[:, 0:48]
                ntT = ptt[:, 64:112]
                if t < KSTEPS - 1:
                    for _ in range(KFILL - KFILL // 2):
                        pf = psB.tile([128, 512], FP, tag="ptw")
                        for kp in range(3):
                            nc.tensor.matmul(pf[:, 0:512], hp8[:, 2 * kp:2 * kp + 2, :],
                                             WhhT8[:, 2 * kp:2 * kp + 2, 512 * (fill_i % 4):512 * (fill_i % 4) + 512],
                                             start=(kp == 0), stop=(kp == 2), perf_mode=DR)
                        fill_i += 1
                dT = g1.tile([128, 8 * ET], FP, tag="dT")
                nc.vector.tensor_sub(dT[:], hp32[:].rearrange("p a b -> p (a b)"), ntT)
                zdT = g1.tile([128, 8 * ET], FP, tag="zdT")
                nc.vector.tensor_mul(zdT[:], zT, dT[:])
                hT32 = gstate.tile([128, ET, BL], FP, tag="hT32")
                nc.vector.tensor_add(hT32[:].rearrange("p a b -> p (a b)"), ntT, zdT[:])
                h8 = h8_b if t % 2 == 0 else h8_a
                nc.scalar.copy(h8[:, :, 0:BL], hT32[:])
                nc.scalar.copy(qemb8[:, :, :, t], hT32[:])
                if t == 24:
                    G2Tb = prep.tile([128, ET, BL], BF, tag="G2Tb")
                    for mt in range(ET):
                        p2 = psB.tile([128, 512], FP, tag="ptw")
                        for kt in range(ET):
                            nc.tensor.matmul(p2[:, 0:BL], gow2[:, kt, 128 * mt:128 * (mt + 1)],
                                             clsTb[:, kt, :], start=(kt == 0),
                                             stop=(kt == ET - 1))
                        nc.vector.tensor_scalar(G2Tb[:, mt, :], p2[:, 0:BL],
                                                b2goT[:, mt:mt + 1], None, OP.add)
                if t == 26:
                    for mt in range(ET):
                        p2 = psB.tile([128, 512], FP, tag="ptw")
                        for kt in range(ET):
                            nc.tensor.matmul(p2[:, 0:BL], gow3[:, kt, 128 * mt:128 * (mt + 1)],
                                             G2Tb[:, kt, :], start=(kt == 0),
                                             stop=(kt == ET - 1))
                        nc.vector.tensor_scalar(goutT[:, mt, :], p2[:, 0:BL],
                                                b3goT[:, mt:mt + 1], Sgo[:, 0:1],
                                                OP.add, OP.mult)

        wb8_cm.__exit__(None, None, None)

        # ================= phase C: Q^T, W1^T, Qt^T =======================
        if PHASES >= 2:
          if True:
              QT8 = prep.tile([128, ET, RQ], F8, tag="QT8")
              qflat = qemb8[:].rearrange("p a b t -> p a (b t)")
              for mt in range(ET):
                  p = psC.tile([128, RQ], FP, tag="pd")
                  for kp in range(3):
                      nc.tensor.matmul(p[:], W08[:, 2 * kp:2 * kp + 2, 128 * mt:128 * (mt + 1)],
                                       qflat[:, 2 * kp:2 * kp + 2, :],
                                       start=(kp == 0), stop=(kp == 2), perf_mode=DR)
                  nc.vector.tensor_scalar(QT8[:, mt, :], p[:], b0laT[:, mt:mt + 1], None, OP.add)
              W1T8 = prep.tile([128, ET, D], F8, tag="W08", name="W1T8")
              for hd in range(ET):
                  for grp in range(2):
                      pt2 = psB.tile([128, 512], BF, tag="ptw")
                      for i in range(3):
                          e2 = grp * 3 + i
                          nc.tensor.matmul(pt2[:, 128 * i:128 * (i + 1)],
                                           W1n[:, e2, 128 * hd:128 * (hd + 1)],
                                           identb[:], is_transpose=True, skip_group_check=True)
                      if grp == 0:
                          nc.vector.tensor_copy(W1T8[:, hd, 0:384], pt2[:, 0:384])
                      else:
                          nc.scalar.copy(W1T8[:, hd, 384:768], pt2[:, 0:384])
              scl = 1.0 / float(np.sqrt(DK))
              for h in range(NH):
                  for mt in range(ET):
                      p = psC.tile([128, RQ], FP, tag="pd")
                      nc.tensor.matmul(p[:], W1T8[:, 3 * h:3 * h + 2, 128 * mt:128 * (mt + 1)],
                                       QT8[:, 3 * h:3 * h + 2, :],
                                       start=True, stop=False, perf_mode=DR)
                      nc.tensor.matmul(p[:], W1T8[:, 3 * h + 2, 128 * mt:128 * (mt + 1)],
                                       QT8[:, 3 * h + 2, :], start=False, stop=True)
                      dst = QtT[:, mt, :].rearrange("p (b h2 t) -> p b h2 t",
                                                    h2=NH, t=T)[:, :, h, :]
                      nc.scalar.activation(dst, p[:], AF.Copy, scale=scl)
        prep_cm.__exit__(None, None, None)

        # ================= phase D: per-b attention =======================
        de = ctx.enter_context(tc.tile_pool(name="de", bufs=1))
        pcxall = de.tile([2, BL * D], BF, tag="pcxall")
        f1 = de.tile([128, 12, 1024], BF, tag="f1")
        for c in range(12):
            gd(1 + c % 3, f1[:, c, :], f1_w[128 * c:128 * (c + 1), :])

        if PHASES >= 3:
            with tc.tile_pool(name="xb", bufs=2) as xb, \
                 tc.tile_pool(name="ab", bufs=1) as ab, \
                 tc.tile_pool(name="psD", bufs=1, space="PSUM") as psA:
              KC = kchunks(NK)
              for b in range(BL):
                  Xn = XnA[:, b, :, :]
                  XT = xb.tile([128, ET, NK], BF, tag="XT")
                  cpeng = [nc.vector.tensor_copy, nc.scalar.copy, nc.gpsimd.tensor_copy]
                  for et in range(ET):
                      for g in range(2):
                          pt = psB.tile([128, 512], BF, tag="ptw")
                          for i in range(4):
                              c = g * 4 + i
                              nc.tensor.matmul(pt[:, 128 * i:128 * (i + 1)],
                                               Xn[:, c, 128 * et:128 * (et + 1)],
                                               identb[:], is_transpose=True,
                                               skip_group_check=True)
                          w = 512 if g == 0 else NK - 512
                          cpeng[(et * 2 + g) % 2](XT[:, et, 512 * g:512 * g + w], pt[:, :w])
                  att = ab.tile([64, NK], BF, tag="att")
                  zacc = ab.tile([64, 2], FP, tag="zacc")
                  for ci, (n0, nw) in enumerate(CH_NK):
                      p = psA.tile([64, 512], FP, tag=f"wh{ci}")
                      for kt in range(ET):
                          nc.tensor.matmul(p[:, :nw],
                                           QtT[:, kt, b * 2 * T:(b + 1) * 2 * T],
                                           XT[:, kt, n0:n0 + nw],
                                           start=(kt == 0), stop=(kt == ET - 1))
                      nc.scalar.activation(att[:, n0:n0 + nw], p[:, :nw], AF.Exp,
                                           accum_out=zacc[:, ci:ci + 1])
                  zs = ab.tile([64, 1], FP, tag="zs")
                  nc.vector.tensor_add(zs[:], zacc[:, 0:1], zacc[:, 1:2])
                  rz = ab.tile([64, 1], FP, tag="rz1")
                  nc.vector.reciprocal(rz[:], zs[:])
                  wm = ab.tile([64, 2], BF, tag="wm")
                  nc.vector.tensor_scalar(wm[:], pmask[:], rz[:, 0:1], None, OP.mult)
                  pa_sb = ab.tile([2, NK], BF, tag="pa_sb")
                  for ci, (n0, nw) in enumerate(CH_NK):
                      p = psA.tile([2, 512], FP, tag=f"wh{2 + ci}")
                      nc.tensor.matmul(p[:, :nw], wm[:], att[:, n0:n0 + nw],
                                       start=True, stop=True)
                      nc.vector.tensor_copy(pa_sb[:, n0:n0 + nw], p[:, :nw])
                  paT = ab.tile([128, len(KC), 2], BF, tag="paT")
                  nc.vector.memset(paT[:].rearrange("p a b -> p (a b)"), 0.0)
                  ptp = psC.tile([128, 512], BF, tag="pd")
                  for c, (k0, kw) in enumerate(KC):
                      nc.tensor.matmul(ptp[:kw, 2 * c:2 * c + 2], pa_sb[:, k0:k0 + kw],
                                       identb[:2, :2], is_transpose=True, skip_group_check=True)
                      nc.vector.tensor_copy(paT[:kw, c, :], ptp[:kw, 2 * c:2 * c + 2])
                  for ci, (n0, nw) in enumerate(CH_D):
                      p = psA.tile([2, 512], FP, tag=f"wh{4 - ci}")
                      for c in range(len(KC)):
                          nc.tensor.matmul(p[:, :nw], paT[:, c, :],
                                           Xn[:, c, n0:n0 + nw],
                                           start=(c == 0), stop=(c == len(KC) - 1))
                      nc.vector.tensor_copy(pcxall[:, b * D + n0:b * D + n0 + nw], p[:, :nw])

        # ================= phase E: projections + MLP =====================
        if PHASES >= 4:
            with tc.tile_pool(name="tail", bufs=1) as tail:
              f2 = tail.tile([128, 8, 512], BF, tag="f2")
              for c in range(8):
                  gd(1 + c % 3, f2[:, c, :], f2_w[128 * c:128 * (c + 1), :])
              f3 = f3b
              W3 = W3b
              b2laTb = tail.tile([128, ET], BF, tag="b2laTb")
              nc.vector.tensor_copy(b2laTb[:], b2laT[:])
              vconT = tail.tile([128, ET], FP, tag="vconT")
              for mt in range(ET):
                  p = psC.tile([128, 1], FP, tag="pd")
                  for kt in range(ET):
                      nc.tensor.matmul(p[:], W3[:, kt, 128 * mt:128 * (mt + 1)],
                                       b2laTb[:, kt:kt + 1], start=(kt == 0), stop=(kt == ET - 1))
                  nc.vector.tensor_scalar(vconT[:, mt:mt + 1], p[:], b3laT[:, mt:mt + 1],
                                          Sla[:, 0:1], OP.add, OP.mult)
              pcxT = tail.tile([128, ET, 2 * BL], BF, tag="pcxT")
              ptc = psC.tile([128, 512], BF, tag="pd")
              for b2 in range(BL):
                  for kt in range(ET):
                      nc.tensor.matmul(ptc[:, 2 * (b2 * ET + kt):2 * (b2 * ET + kt) + 2],
                                       pcxall[:, b2 * D + 128 * kt:b2 * D + 128 * (kt + 1)],
                                       identb[:2, :2], is_transpose=True, skip_group_check=True)
              src_v = ptc[:, :96].rearrange("p (b a h) -> p a b h", b=BL, a=ET)
              dst_v = pcxT[:].rearrange("p a (b h) -> p a b h", h=NH)
              nc.vector.tensor_copy(dst_v, src_v)
              W2 = W2b
              pctxT = tail.tile([128, ET, BL], BF, tag="pctxT")
              pcv = pcxT[:].rearrange("p a (b h) -> p a b h", h=NH)
              for h in range(NH):
                  for mi in range(3):
                      mt = h * 3 + mi
                      p = psC.tile([128, BL], FP, tag="pd")
                      for kt in range(ET):
                          nc.tensor.matmul(p[:], W2[:, kt, 128 * mt:128 * (mt + 1)],
                                           pcv[:, kt, :, h], start=(kt == 0), stop=(kt == ET - 1))
                      nc.vector.tensor_copy(pctxT[:, mt, :], p[:])
              loT = tail.tile([128, ET, BL], BF, tag="loT")
              for mt in range(ET):
                  p = psC.tile([128, BL], FP, tag="pd")
                  for kt in range(ET):
                      nc.tensor.matmul(p[:], W3[:, kt, 128 * mt:128 * (mt + 1)],
                                       pctxT[:, kt, :], start=(kt == 0), stop=(kt == ET - 1))
                  nc.vector.tensor_scalar(loT[:, mt, :], p[:], vconT[:, mt:mt + 1], None, OP.add)

              y1T = tail.tile([128, 8, BL], BF, tag="y1T")
              for mt in range(8):
                  p = psC.tile([128, BL], FP, tag="pd")
                  for kt in range(12):
                      r_ = loT[:, kt, :] if kt < ET else goutT[:, kt - ET, :]
                      nc.tensor.matmul(p[:], f1[:, kt, 128 * mt:128 * (mt + 1)], r_,
                                       start=(kt == 0), stop=(kt == 11))
                  nc.vector.tensor_scalar(y1T[:, mt, :], p[:], b1fT[:, mt:mt + 1], None, OP.add)
              y2T = tail.tile([128, 4, BL], BF, tag="y2T")
              for mt in range(4):
                  p = psC.tile([128, BL], FP, tag="pd")
                  for kt in range(8):
                      nc.tensor.matmul(p[:], f2[:, kt, 128 * mt:128 * (mt + 1)],
                                       y1T[:, kt, :], start=(kt == 0), stop=(kt == 7))
                  nc.scalar.activation(y2T[:, mt, :], p[:], AF.Relu, bias=b2fT[:, mt:mt + 1])
              yT = tail.tile([128, 8, BL], FP, tag="yT")
              for mt in range(8):
                  p = psC.tile([128, BL], FP, tag="pd")
                  for kt in range(4):
                      nc.tensor.matmul(p[:], f3[:, kt, 128 * mt:128 * (mt + 1)],
                                       y2T[:, kt, :], start=(kt == 0), stop=(kt == 3))
                  nc.vector.tensor_scalar(yT[:, mt, :], p[:], b3fT[:, mt:mt + 1], None, OP.add)
              ynat = tail.tile([BL, 1024], FP, tag="ynat")
              for g in range(2):
                  po = psB.tile([128, 512], FP, tag="ptw")
                  for i in range(4):
                      mt = g * 4 + i
                      nc.tensor.matmul(po[:BL, 128 * i:128 * (i + 1)], yT[:, mt, :],
                                       ident[:128, :128], is_transpose=True,
                                       skip_group_check=True)
                  nc.vector.tensor_copy(ynat[:, 512 * g:512 * (g + 1)], po[:BL, :])
              nc.sync.dma_start(out_d[:, :], ynat[:])

    nc.compile()
    return nc


_NC = None


def kernel(**inputs):
    global _NC
    if _NC is None:
        _NC = build()
    B = inputs["image_local_embeds"].shape[0]
    per = B // NCORES
    in_maps = []
    for c in range(NCORES):
        sl = slice(c * per, (c + 1) * per)
        m = {
            "img": np.ascontiguousarray(np.asarray(inputs["image_local_embeds"])[sl], dtype=np.float32),
            "h0": np.ascontiguousarray(np.asarray(inputs["h0"])[sl], dtype=np.float32),
        }
        for k in ["gru_w_ih", "gru_w_hh", "gru_b_ih", "gru_b_hh", "ga_w", "ga_b",
                  "ga_pool", "la_w", "la_b", "la_pool", "go_w", "go_b", "go_pool",
                  "f1_w", "f1_b", "f2_w", "f2_b", "f3_w", "f3_b"]:
            m[k] = np.ascontiguousarray(np.asarray(inputs[k], dtype=np.float32))
        in_maps.append(m)
    res = run_bass_kernel_spmd(_NC, in_maps, core_ids=list(range(NCORES)))
    return np.concatenate([res.results[c]["out"] for c in range(NCORES)], axis=0)



# revision 74
# speedup vs baseline: 1.0858x; 1.0858x over previous
"""Trainium2 Bass kernel for nn_BiVision_VQA2 (B=64,T=32,D=768,N=901).

Data-parallel over batch: 8 batch elems per core x 8 cores.
Key math simplifications (validated vs reference in numpy, rel err ~1e-6):
  - ga/go attention use a single key token -> softmax==1 -> those paths are
    linear in cls; question_embeds is mathematically unused.
  - GRU input `a` is constant over time; wx computed once.
  - local attention: scores = (qemb @ W0_h) @ W1_h^T / sqrt(dk) @ X^T ;
    row-constant score terms (K bias, Q.b1) drop out of softmax; query
    pooling applied to the attention matrix before the @X contraction;
    constant bias terms folded into one vector.
"""

import numpy as np
from contextlib import ExitStack

import concourse.bass as bass
import concourse.tile as tile
from concourse import bacc, mybir
from concourse.bass_utils import run_bass_kernel_spmd
from concourse.masks import make_identity

FP = mybir.dt.float32
FPR = mybir.dt.float32r
OP = mybir.AluOpType
AF = mybir.ActivationFunctionType
BF = mybir.dt.bfloat16
F8 = mybir.dt.float8e4
DR = mybir.MatmulPerfMode.DoubleRow

NCORES = 8
BL = 8
D = 768
T = 32
G = 3 * D
NK = 900
NH = 2
DK = 384
ET = D // 128
RQ = BL * T
USE_FPR = True


def chunks(total):
    out, o = [], 0
    while o < total:
        w = min(512, total - o)
        out.append((o, w))
        o += w
    return out


CH_G = chunks(G)
CH_NK = [(0, 512), (512, 388)]
CH_D = [(0, 512), (512, 256)]


def _r(ap):
    return ap.bitcast(FPR) if USE_FPR else ap


def kchunks(n):
    out, o = [], 0
    while o < n:
        out.append((o, min(128, n - o)))
        o += 128
    return out


import os
PHASES = int(os.environ.get("KPHASES", "4"))


def build():
    nc = bacc.Bacc("TRN2", target_bir_lowering=False, debug=False,
                   enable_asserts=False, num_swdge_queues=4)

    def gd(q, out, in_, **kw):
        inst = nc.gpsimd.dma_start(out, in_, **kw)
        if q:
            inst.ins.queue = f"qPoolDynamic{q}"
        return inst

    img = nc.dram_tensor("img", [BL, 901, D], FP, kind="ExternalInput").ap()
    h0 = nc.dram_tensor("h0", [BL, D], FP, kind="ExternalInput").ap()
    w_ih = nc.dram_tensor("gru_w_ih", [G, D], FP, kind="ExternalInput").ap()
    w_hh = nc.dram_tensor("gru_w_hh", [G, D], FP, kind="ExternalInput").ap()
    b_ih = nc.dram_tensor("gru_b_ih", [G], FP, kind="ExternalInput").ap()
    b_hh = nc.dram_tensor("gru_b_hh", [G], FP, kind="ExternalInput").ap()
    ga_w = nc.dram_tensor("ga_w", [4, D, D], FP, kind="ExternalInput").ap()
    ga_b = nc.dram_tensor("ga_b", [4, D], FP, kind="ExternalInput").ap()
    ga_pool = nc.dram_tensor("ga_pool", [1], FP, kind="ExternalInput").ap()
    la_w = nc.dram_tensor("la_w", [4, D, D], FP, kind="ExternalInput").ap()
    la_b = nc.dram_tensor("la_b", [4, D], FP, kind="ExternalInput").ap()
    la_pool = nc.dram_tensor("la_pool", [T], FP, kind="ExternalInput").ap()
    go_w = nc.dram_tensor("go_w", [4, D, D], FP, kind="ExternalInput").ap()
    go_b = nc.dram_tensor("go_b", [4, D], FP, kind="ExternalInput").ap()
    go_pool = nc.dram_tensor("go_pool", [T], FP, kind="ExternalInput").ap()
    f1_w = nc.dram_tensor("f1_w", [2 * D, 1024], FP, kind="ExternalInput").ap()
    f1_b = nc.dram_tensor("f1_b", [1024], FP, kind="ExternalInput").ap()
    f2_w = nc.dram_tensor("f2_w", [1024, 512], FP, kind="ExternalInput").ap()
    f2_b = nc.dram_tensor("f2_b", [512], FP, kind="ExternalInput").ap()
    f3_w = nc.dram_tensor("f3_w", [512, 1024], FP, kind="ExternalInput").ap()
    f3_b = nc.dram_tensor("f3_b", [1024], FP, kind="ExternalInput").ap()
    out_d = nc.dram_tensor("out", [BL, 1024], FP, kind="ExternalOutput").ap()

    with tile.TileContext(nc) as tc, ExitStack() as ctx:
        cpool = ctx.enter_context(tc.tile_pool(name="const", bufs=1))
        gstate = ctx.enter_context(tc.tile_pool(name="gstate", bufs=2))
        xall = ctx.enter_context(tc.tile_pool(name="xall", bufs=1))
        tailw = ctx.enter_context(tc.tile_pool(name="tailw", bufs=1))
        psB = ctx.enter_context(tc.tile_pool(name="psB", bufs=2, space="PSUM"))
        psC = ctx.enter_context(tc.tile_pool(name="psC", bufs=1, space="PSUM"))

        ident = cpool.tile([128, 128], FP, tag="ident")
        make_identity(nc, ident[:])
        ones1 = cpool.tile([1, 128], FP, tag="ones1")
        nc.vector.memset(ones1[:], 1.0)
        onesT = cpool.tile([T, 128], FP, tag="onesT")
        nc.vector.memset(onesT[:], 1.0)
        identr = cpool.tile([128, 128], FP, tag="identr")
        nc.vector.tensor_copy(_r(identr[:]), ident[:])
        identb = cpool.tile([128, 128], BF, tag="identb")
        nc.vector.tensor_copy(identb[:], ident[:])
        ones1r = cpool.tile([1, 128], FP, tag="ones1r")
        nc.vector.tensor_copy(_r(ones1r[:]), ones1[:])

        def colvec(dram_1d, n, tag):
            nt = n // 128
            t_ = cpool.tile([128, nt], FP, tag=tag)
            for j in range(nt):
                nc.sync.dma_start(t_[:, j:j + 1], dram_1d[j * 128:(j + 1) * 128][:, None])
            return t_

        b2gaT = colvec(ga_b[2], D, "b2gaT")
        b3gaT = colvec(ga_b[3], D, "b3gaT")

        gapool_c = cpool.tile([1, 1], FP, tag="gapool_c")
        nc.sync.dma_start(gapool_c[:], ga_pool[:][:, None])

        def sum_bcast(vcol, k, tag):
            p = psC.tile([128, 1], FP, tag="pd")
            lhs = onesT if k == T else ones1
            nc.tensor.matmul(p[:], lhs[:k, :], vcol[:k, :], start=True, stop=True)
            s = cpool.tile([128, 1], FP, tag=tag)
            nc.vector.tensor_copy(s[:], p[:])
            return s

        Sga = sum_bcast(gapool_c, 1, "Sga")

        qemb8 = cpool.tile([128, ET, BL, T], F8, tag="qemb8")
        goutT = cpool.tile([128, ET, BL], BF, tag="goutT")
        aT = cpool.tile([128, ET, BL], FP, tag="aT")
        bhhN_r = cpool.tile([1, D], FP, tag="bhhN_r")

        # img patch tokens, all 8 batch elems, prefetched early (bf16)
        XnA = xall.tile([128, BL, 8, D], BF, tag="XnA")
        # early-persisted tail weights (DMAs issued pre-loop, overlap GRU)
        W2b = tailw.tile([128, ET, D], BF, tag="W2b")
        W3b = tailw.tile([128, ET, D], BF, tag="W3b")
        f3b = tailw.tile([128, 4, 1024], BF, tag="f3b")

        # ================= phase A: cls -> a (ga path only) ===============
        clsTb = cpool.tile([128, ET, BL], BF, tag="clsTb")
        clsT8 = cpool.tile([128, ET, BL], F8, tag="clsT8")

        def dense_T(pool, wdt, w_nat_dram, rhsT, biasT, scaleT, otile, wtag, dmaq):
            wsb = pool.tile([128, ET, D], wdt, tag=wtag)
            for c in range(ET):
                dmaq.dma_start(wsb[:, c, :], w_nat_dram[128 * c:128 * (c + 1), :])
            for mt in range(ET):
                p = psC.tile([128, BL], FP, tag="pd")
                for kt in range(ET):
                    nc.tensor.matmul(p[:], wsb[:, kt, 128 * mt:128 * (mt + 1)],
                                     rhsT[:, kt, :], start=(kt == 0), stop=(kt == ET - 1))
                if scaleT is None:
                    nc.vector.tensor_scalar(otile[:, mt, :], p[:], biasT[:, mt:mt + 1],
                                            None, OP.add)
                else:
                    nc.vector.tensor_scalar(otile[:, mt, :], p[:], biasT[:, mt:mt + 1],
                                            scaleT[:, 0:1], OP.add, OP.mult)

        with tc.tile_pool(name="ph0", bufs=1) as ph0:
            clsn = ph0.tile([BL, D], FP, tag="clsn")
            nc.sync.dma_start(clsn[:], img[0:BL, 0, :])
            ptr = psC.tile([128, 512], FP, tag="pd")
            for kt in range(ET):
                nc.tensor.matmul(ptr[:, 8 * kt:8 * kt + 8], clsn[:, 128 * kt:128 * (kt + 1)],
                                 ident[:BL, :BL], is_transpose=True, skip_group_check=True)
            clsT = ph0.tile([128, ET, BL], FP, tag="clsT")
            nc.vector.tensor_copy(clsT[:].rearrange("p a b -> p (a b)"), ptr[:, :8 * ET])
            nc.scalar.copy(clsTb[:], clsT[:])
            nc.scalar.copy(clsT8[:], clsT[:])

        # ================= phase B: GRU (fp8 DoubleRow) ===================
        cde = ctx.enter_context(tc.tile_pool(name="cde", bufs=1))
        QtT = cde.tile([128, ET, NH * RQ], BF, tag="QtT")
        prep_cm = tc.tile_pool(name="prep", bufs=1)
        prep = prep_cm.__enter__()
        wb8_cm = tc.tile_pool(name="wb8", bufs=1)
        wb8 = wb8_cm.__enter__()
        WhhT8 = wb8.tile([128, ET, G], F8, tag="WhhT8")

        with tc.tile_pool(name="wpro", bufs=1) as wpro, \
             tc.tile_pool(name="wnat", bufs=4) as wnat:
            combr = wpro.tile([1, 2 * D], FP, tag="combr")
            nc.sync.dma_start(combr[:], b_ih[0:2 * D][None, :])
            nc.gpsimd.dma_start(combr[:], b_hh[0:2 * D][None, :], accum_op=OP.add)
            bhhN_t = wpro.tile([1, D], FP, tag="bhhN_t")
            nc.sync.dma_start(bhhN_t[:], b_hh[2 * D:3 * D][None, :])
            nc.vector.tensor_copy(_r(bhhN_r[:]), bhhN_t[:])
            bihN = wpro.tile([1, D], FP, tag="bhhN_t", name="bihN")
            nc.sync.dma_start(bihN[:], b_ih[2 * D:3 * D][None, :])


            WihT8 = prep.tile([128, ET, G], F8, tag="gow3", name="WihT8")

            def build_W8(w_dram, dst):
                jts = kchunks(G)
                for g0 in range(0, len(jts), 4):
                    grp = jts[g0:g0 + 4]
                    nats = []
                    for qi, (j0, jw) in enumerate(grp):
                        wn = wnat.tile([128, D], BF, tag="wn")
                        gd(qi % 4, wn[:jw, :], w_dram[j0:j0 + jw, :])
                        nats.append((wn, j0, jw))
                    for et in range(ET):
                        pt = psB.tile([128, 512], BF, tag="ptw")
                        for i, (wn, j0, jw) in enumerate(nats):
                            nc.tensor.matmul(pt[:, 128 * i:128 * i + jw],
                                             wn[:jw, 128 * et:128 * (et + 1)],
                                             identb[:jw, :jw], is_transpose=True,
                                             skip_group_check=True)
                        w0 = grp[0][0]
                        wlen = sum(jw for (_, _, jw) in nats)
                        if et % 2 == 0:
                            nc.vector.tensor_copy(dst[:, et, w0:w0 + wlen], pt[:, :wlen])
                        else:
                            nc.scalar.copy(dst[:, et, w0:w0 + wlen], pt[:, :wlen])

            gaw2 = prep.tile([128, ET, D], F8, tag="W08", name="gaw2")
            for c in range(ET):
                gd(1 + c % 3, gaw2[:, c, :], ga_w[2][128 * c:128 * (c + 1), :])
            gaw3 = prep.tile([128, ET, D], F8, tag="gow2", name="gaw3")
            for c in range(ET):
                gd(1 + c % 3, gaw3[:, c, :], ga_w[3][128 * c:128 * (c + 1), :])

            build_W8(w_ih, WihT8)

            A2T = wpro.tile([128, ET, BL], F8, tag="A2T")
            for mt in range(ET):
                p = psC.tile([128, BL], FP, tag="pd")
                for kt in range(ET):
                    nc.tensor.matmul(p[:], gaw2[:, kt, 128 * mt:128 * (mt + 1)],
                                     clsT8[:, kt, :], start=(kt == 0), stop=(kt == ET - 1))
                nc.vector.tensor_scalar(A2T[:, mt, :], p[:], b2gaT[:, mt:mt + 1],
                                        None, OP.add)
            for mt in range(ET):
                p = psC.tile([128, BL], FP, tag="pd")
                for kt in range(ET):
                    nc.tensor.matmul(p[:], gaw3[:, kt, 128 * mt:128 * (mt + 1)],
                                     A2T[:, kt, :], start=(kt == 0), stop=(kt == ET - 1))
                nc.vector.tensor_scalar(aT[:, mt, :], p[:], b3gaT[:, mt:mt + 1],
                                        Sga[:, 0:1], OP.add, OP.mult)
            aT8 = cpool.tile([128, ET, 128], F8, tag="aT8")
            nc.vector.memset(aT8[:].rearrange("p a b -> p (a b)"), 0.0)
            nc.vector.tensor_copy(aT8[:, :, 0:BL], aT[:])

            hnat = prep.tile([BL, D], FP, tag="W08", name="hnat")
            nc.sync.dma_start(hnat[:], h0[:, :])
            ptr0 = psC.tile([128, 512], FP, tag="pd")
            for kt in range(ET):
                nc.tensor.matmul(ptr0[:, 8 * kt:8 * kt + 8], hnat[:, 128 * kt:128 * (kt + 1)],
                                 ident[:BL, :BL], is_transpose=True, skip_group_check=True)
            hT32 = gstate.tile([128, ET, BL], FP, tag="hT32")
            nc.vector.tensor_copy(hT32[:].rearrange("p a b -> p (a b)"), ptr0[:, :8 * ET])
            h8_a = cpool.tile([128, ET, 128], F8, tag="h8_a")
            h8_b = cpool.tile([128, ET, 128], F8, tag="h8_b")
            nc.vector.memset(h8_a[:].rearrange("p a b -> p (a b)"), 0.0)
            nc.vector.memset(h8_b[:].rearrange("p a b -> p (a b)"), 0.0)
            nc.scalar.copy(h8_a[:, :, 0:BL], hT32[:])
            h8 = h8_a

            # wx (+ all biases folded): rz sections get bih+bhh, n gets bih
            wxbRZ = prep.tile([BL, 2 * D], FP, tag="wxbRZ")
            wxbN = prep.tile([BL, D], FP, tag="wxbN")
            for (j0, jw) in CH_G:
                p = psC.tile([128, 512], FP, tag="pd")
                for kp in range(3):
                    nc.tensor.matmul(p[:, :jw], aT8[:, 2 * kp:2 * kp + 2, :],
                                     WihT8[:, 2 * kp:2 * kp + 2, j0:j0 + jw],
                                     start=(kp == 0), stop=False, perf_mode=DR)
                src = combr[:, j0:j0 + jw] if j0 < 2 * D else bihN[:, j0 - 2 * D:j0 - 2 * D + jw]
                nc.tensor.matmul(p[:, :jw], ones1[:1, :], src,
                                 start=False, stop=True)
                if j0 < 2 * D:
                    nc.vector.tensor_copy(_r(wxbRZ[:, j0:j0 + jw]), p[:BL, :jw])
                else:
                    nc.vector.tensor_copy(wxbN[:, j0 - 2 * D:j0 - 2 * D + jw], p[:BL, :jw])

            build_W8(w_hh, WhhT8)


        # ---- deferred small constants (off the build critical path) ------
        b2goT = colvec(go_b[2], D, "b2goT")
        b3goT = colvec(go_b[3], D, "b3goT")
        b0laT = colvec(la_b[0], D, "b0laT")
        b2laT = colvec(la_b[2], D, "b2laT")
        b3laT = colvec(la_b[3], D, "b3laT")
        b1fT = colvec(f1_b, 1024, "b1fT")
        b2fT = colvec(f2_b, 512, "b2fT")
        b3fT = colvec(f3_b, 1024, "b3fT")
        lapool_c = cpool.tile([T, 1], FP, tag="lapool_c")
        nc.sync.dma_start(lapool_c[:], la_pool[:][:, None])
        gopool_c = cpool.tile([T, 1], FP, tag="gopool_c")
        nc.sync.dma_start(gopool_c[:], go_pool[:][:, None])
        Sla = sum_bcast(lapool_c, T, "Sla")
        Sgo = sum_bcast(gopool_c, T, "Sgo")
        pmask = cpool.tile([64, 2], FP, tag="pmask")
        nc.vector.memset(pmask[:], 0.0)
        nc.sync.dma_start(pmask[0:T, 0:1], la_pool[:][:, None])
        nc.sync.dma_start(pmask[T:2 * T, 1:2], la_pool[:][:, None])

        # ---- pre-loop early DMA emission (overlaps the GRU steps) --------
        for b in range(BL):
            nc.vector.memset(XnA[:, b, 7, :], 0.0)
            for c, (k0, kw) in enumerate(kchunks(NK)):
                gd(1 + (b * 8 + c) % 3, XnA[:kw, b, c, :], img[b, 1 + k0:1 + k0 + kw, :])
        W08 = prep.tile([128, ET, D], F8, tag="W08")
        for c in range(ET):
            gd(1 + c % 3, W08[:, c, :], la_w[0][128 * c:128 * (c + 1), :])
        gow2 = prep.tile([128, ET, D], BF, tag="gow2")
        for c in range(ET):
            gd(1 + c % 3, gow2[:, c, :], go_w[2][128 * c:128 * (c + 1), :])
        for c in range(ET):
            gd(1 + c % 3, W2b[:, c, :], la_w[2][128 * c:128 * (c + 1), :])
        for c in range(ET):
            gd(1 + c % 3, W3b[:, c, :], la_w[3][128 * c:128 * (c + 1), :])
        for c in range(4):
            gd(1 + c % 3, f3b[:, c, :], f3_w[128 * c:128 * (c + 1), :])
        # stall-prone loads (wait on in-loop readers) go last on queue 0
        gow3 = prep.tile([128, ET, D], BF, tag="gow3")
        for c in range(ET):
            gd(0, gow3[:, c, :], go_w[3][128 * c:128 * (c + 1), :])
        # W1 lands in gow2's buffer once the go stage-1 matmuls are done
        W1n = prep.tile([128, ET, D], BF, tag="gow2", name="W1n")
        for c in range(ET):
            gd(0, W1n[:, c, :], la_w[1][128 * c:128 * (c + 1), :])

        with tc.tile_pool(name="g1", bufs=1) as g1, \
             tc.tile_pool(name="psG", bufs=1, space="PSUM") as psG:

            # section psums: A = j[0:1024] (r + z1), Z = j[1024:1536] (z2),
            # N = j[1536:2304] (n); emission order A0 A1 N0 N1 Z
            STEP_CHUNKS = [("a", 0, 0, 512), ("a", 512, 512, 512),
                           ("n", 0, 1536, 512), ("n", 512, 2048, 256),
                           ("z", 0, 1024, 512)]
            KSTEPS = int(os.environ.get("KSTEPS", str(T)))
            KFILL = int(os.environ.get("KFILL", "4"))
            fill_i = 0
            for t in range(KSTEPS):
                hp8, hp32 = h8, hT32
                pA = psG.tile([128, 1024], FP, tag="a")
                pN = psG.tile([128, D], FP, tag="n")
                pZ = psG.tile([128, 512], FP, tag="z")
                tiles = {"a": pA, "n": pN, "z": pZ}
                for (sec, c0, j0, jw) in STEP_CHUNKS:
                    p = tiles[sec]
                    for kp in range(3):
                        nc.tensor.matmul(p[:, c0:c0 + jw], hp8[:, 2 * kp:2 * kp + 2, :],
                                         WhhT8[:, 2 * kp:2 * kp + 2, j0:j0 + jw],
                                         start=(kp == 0), stop=False, perf_mode=DR)
                    if j0 >= 2 * D:
                        nc.tensor.matmul(p[:, c0:c0 + jw], _r(ones1r[:1, :]),
                                         _r(bhhN_r[:, j0 - 2 * D:j0 - 2 * D + jw]),
                                         start=False, stop=True)
                    else:
                        nc.tensor.matmul(p[:, c0:c0 + jw], _r(identr[:BL, :]),
                                         _r(wxbRZ[:, j0:j0 + jw]), start=False, stop=True)
                r_sig = g1.tile([BL, D], FP, tag="rsig")
                nc.scalar.activation(r_sig[:], pA[:BL, 0:768], AF.Sigmoid)
                z_nat = g1.tile([BL, D], FP, tag="znat")
                nc.scalar.activation(z_nat[:, 0:256], pA[:BL, 768:1024], AF.Sigmoid)
                rwn = g1.tile([BL, D], FP, tag="rwn")
                nc.vector.tensor_mul(rwn[:], r_sig[:], pN[:BL, :])
                npre = rwn
                nc.vector.tensor_add(npre[:], rwn[:], wxbN[:])
                nc.scalar.activation(z_nat[:, 256:768], pZ[:BL, :], AF.Sigmoid)
                nt_ = g1.tile([BL, D], FP, tag="nt")
                nc.scalar.activation(nt_[:, 0:512], npre[:, 0:512], AF.Tanh)
                nc.scalar.activation(nt_[:, 512:768], npre[:, 512:768], AF.Tanh)
                # fills between mm block and transposes keep PE p-state hot
                for _ in range(KFILL // 2):
                    pf = psB.tile([128, 512], FP, tag="ptw")
                    for kp in range(3):
                        nc.tensor.matmul(pf[:, 0:512], hp8[:, 2 * kp:2 * kp + 2, :],
                                         WhhT8[:, 2 * kp:2 * kp + 2, 512 * (fill_i % 4):512 * (fill_i % 4) + 512],
                                         start=(kp == 0), stop=(kp == 2), perf_mode=DR)
                    fill_i += 1
                ptt = psC.tile([128, 512], FP, tag="pd")
                for kt in range(ET):
                    nc.tensor.matmul(ptt[:, 8 * kt:8 * kt + 8], z_nat[:, 128 * kt:128 * (kt + 1)],
                                     ident[:BL, :BL], is_transpose=True, skip_group_check=True)
                for kt in range(ET):
                    nc.tensor.matmul(ptt[:, 64 + 8 * kt:64 + 8 * kt + 8],
                                     nt_[:, 128 * kt:128 * (kt + 1)],
                                     ident[:BL, :BL], is_transpose=True, skip_group_check=True)
                zT = ptt[:, 0:48]
                ntT = ptt[:, 64:112]
                if t < KSTEPS - 1:
                    for _ in range(KFILL - KFILL // 2):
                        pf = psB.tile([128, 512], FP, tag="ptw")
                        for kp in range(3):
                            nc.tensor.matmul(pf[:, 0:512], hp8[:, 2 * kp:2 * kp + 2, :],
                                             WhhT8[:, 2 * kp:2 * kp + 2, 512 * (fill_i % 4):512 * (fill_i % 4) + 512],
                                             start=(kp == 0), stop=(kp == 2), perf_mode=DR)
                        fill_i += 1
                dT = g1.tile([128, 8 * ET], FP, tag="dT")
                nc.vector.tensor_sub(dT[:], hp32[:].rearrange("p a b -> p (a b)"), ntT)
                zdT = g1.tile([128, 8 * ET], FP, tag="zdT")
                nc.vector.tensor_mul(zdT[:], zT, dT[:])
                hT32 = gstate.tile([128, ET, BL], FP, tag="hT32")
                nc.vector.tensor_add(hT32[:].rearrange("p a b -> p (a b)"), ntT, zdT[:])
                h8 = h8_b if t % 2 == 0 else h8_a
                nc.scalar.copy(h8[:, :, 0:BL], hT32[:])
                nc.scalar.copy(qemb8[:, :, :, t], hT32[:])
                if t == 24:
                    G2Tb = prep.tile([128, ET, BL], BF, tag="G2Tb")
                    for mt in range(ET):
                        p2 = psB.tile([128, 512], FP, tag="ptw")
                        for kt in range(ET):
                            nc.tensor.matmul(p2[:, 0:BL], gow2[:, kt, 128 * mt:128 * (mt + 1)],
                                             clsTb[:, kt, :], start=(kt == 0),
                                             stop=(kt == ET - 1))
                        nc.vector.tensor_scalar(G2Tb[:, mt, :], p2[:, 0:BL],
                                                b2goT[:, mt:mt + 1], None, OP.add)
                if t == 26:
                    for mt in range(ET):
                        p2 = psB.tile([128, 512], FP, tag="ptw")
                        for kt in range(ET):
                            nc.tensor.matmul(p2[:, 0:BL], gow3[:, kt, 128 * mt:128 * (mt + 1)],
                                             G2Tb[:, kt, :], start=(kt == 0),
                                             stop=(kt == ET - 1))
                        nc.vector.tensor_scalar(goutT[:, mt, :], p2[:, 0:BL],
                                                b3goT[:, mt:mt + 1], Sgo[:, 0:1],
                                                OP.add, OP.mult)

        wb8_cm.__exit__(None, None, None)

        # ================= phase C: Q^T, W1^T, Qt^T =======================
        if PHASES >= 2:
          if True:
              QT8 = prep.tile([128, ET, RQ], F8, tag="QT8")
              qflat = qemb8[:].rearrange("p a b t -> p a (b t)")
              for mt in range(ET):
                  p = psC.tile([128, RQ], FP, tag="pd")
                  for kp in range(3):
                      nc.tensor.matmul(p[:], W08[:, 2 * kp:2 * kp + 2, 128 * mt:128 * (mt + 1)],
                                       qflat[:, 2 * kp:2 * kp + 2, :],
                                       start=(kp == 0), stop=(kp == 2), perf_mode=DR)
                  nc.vector.tensor_scalar(QT8[:, mt, :], p[:], b0laT[:, mt:mt + 1], None, OP.add)
              W1T8 = prep.tile([128, ET, D], F8, tag="W08", name="W1T8")
              for hd in range(ET):
                  for grp in range(2):
                      pt2 = psB.tile([128, 512], BF, tag="ptw")
                      for i in range(3):
                          e2 = grp * 3 + i
                          nc.tensor.matmul(pt2[:, 128 * i:128 * (i + 1)],
                                           W1n[:, e2, 128 * hd:128 * (hd + 1)],
                                           identb[:], is_transpose=True, skip_group_check=True)
                      if grp == 0:
                          nc.vector.tensor_copy(W1T8[:, hd, 0:384], pt2[:, 0:384])
                      else:
                          nc.scalar.copy(W1T8[:, hd, 384:768], pt2[:, 0:384])
              scl = 1.0 / float(np.sqrt(DK))
              for h in range(NH):
                  for mt in range(ET):
                      p = psC.tile([128, RQ], FP, tag="pd")
                      nc.tensor.matmul(p[:], W1T8[:, 3 * h:3 * h + 2, 128 * mt:128 * (mt + 1)],
                                       QT8[:, 3 * h:3 * h + 2, :],
                                       start=True, stop=False, perf_mode=DR)
                      nc.tensor.matmul(p[:], W1T8[:, 3 * h + 2, 128 * mt:128 * (mt + 1)],
                                       QT8[:, 3 * h + 2, :], start=False, stop=True)
                      dst = QtT[:, mt, :].rearrange("p (b h2 t) -> p b h2 t",
                                                    h2=NH, t=T)[:, :, h, :]
                      nc.scalar.activation(dst, p[:], AF.Copy, scale=scl)
        prep_cm.__exit__(None, None, None)

        # ================= phase D: per-b attention =======================
        de = ctx.enter_context(tc.tile_pool(name="de", bufs=1))
        pcxall = de.tile([2, BL * D], BF, tag="pcxall")
        f1 = de.tile([128, 12, 1024], BF, tag="f1")
        for c in range(12):
            gd(1 + c % 3, f1[:, c, :], f1_w[128 * c:128 * (c + 1), :])

        if PHASES >= 3:
            with tc.tile_pool(name="xb", bufs=2) as xb, \
                 tc.tile_pool(name="ab", bufs=1) as ab, \
                 tc.tile_pool(name="psD", bufs=1, space="PSUM") as psA:
              KC = kchunks(NK)
              for b in range(BL):
                  Xn = XnA[:, b, :, :]
                  XT = xb.tile([128, ET, NK], BF, tag="XT")
                  cpeng = [nc.vector.tensor_copy, nc.scalar.copy, nc.gpsimd.tensor_copy]
                  for et in range(ET):
                      for g in range(2):
                          pt = psB.tile([128, 512], BF, tag="ptw")
                          for i in range(4):
                              c = g * 4 + i
                              nc.tensor.matmul(pt[:, 128 * i:128 * (i + 1)],
                                               Xn[:, c, 128 * et:128 * (et + 1)],
                                               identb[:], is_transpose=True,
                                               skip_group_check=True)
                          w = 512 if g == 0 else NK - 512
                          cpeng[(et * 2 + g) % 2](XT[:, et, 512 * g:512 * g + w], pt[:, :w])
                  att = ab.tile([64, NK], BF, tag="att")
                  zacc = ab.tile([64, 2], FP, tag="zacc")
                  for ci, (n0, nw) in enumerate(CH_NK):
                      p = psA.tile([64, 512], FP, tag=f"wh{ci}")
                      for kt in range(ET):
                          nc.tensor.matmul(p[:, :nw],
                                           QtT[:, kt, b * 2 * T:(b + 1) * 2 * T],
                                           XT[:, kt, n0:n0 + nw],
                                           start=(kt == 0), stop=(kt == ET - 1))
                      nc.scalar.activation(att[:, n0:n0 + nw], p[:, :nw], AF.Exp,
                                           accum_out=zacc[:, ci:ci + 1])
                  zs = ab.tile([64, 1], FP, tag="zs")
                  nc.vector.tensor_add(zs[:], zacc[:, 0:1], zacc[:, 1:2])
                  rz = ab.tile([64, 1], FP, tag="rz1")
                  nc.vector.reciprocal(rz[:], zs[:])
                  wm = ab.tile([64, 2], BF, tag="wm")
                  nc.vector.tensor_scalar(wm[:], pmask[:], rz[:, 0:1], None, OP.mult)
                  pa_sb = ab.tile([2, NK], BF, tag="pa_sb")
                  for ci, (n0, nw) in enumerate(CH_NK):
                      p = psA.tile([2, 512], FP, tag=f"wh{2 + ci}")
                      nc.tensor.matmul(p[:, :nw], wm[:], att[:, n0:n0 + nw],
                                       start=True, stop=True)
                      nc.vector.tensor_copy(pa_sb[:, n0:n0 + nw], p[:, :nw])
                  paT = ab.tile([128, len(KC), 2], BF, tag="paT")
                  nc.vector.memset(paT[:].rearrange("p a b -> p (a b)"), 0.0)
                  ptp = psC.tile([128, 512], BF, tag="pd")
                  for c, (k0, kw) in enumerate(KC):
                      nc.tensor.matmul(ptp[:kw, 2 * c:2 * c + 2], pa_sb[:, k0:k0 + kw],
                                       identb[:2, :2], is_transpose=True, skip_group_check=True)
                      nc.vector.tensor_copy(paT[:kw, c, :], ptp[:kw, 2 * c:2 * c + 2])
                  for ci, (n0, nw) in enumerate(CH_D):
                      p = psA.tile([2, 512], FP, tag=f"wh{4 - ci}")
                      for c in range(len(KC)):
                          nc.tensor.matmul(p[:, :nw], paT[:, c, :],
                                           Xn[:, c, n0:n0 + nw],
                                           start=(c == 0), stop=(c == len(KC) - 1))
                      nc.vector.tensor_copy(pcxall[:, b * D + n0:b * D + n0 + nw], p[:, :nw])

        # ================= phase E: projections + MLP =====================
        if PHASES >= 4:
            with tc.tile_pool(name="tail", bufs=1) as tail:
              f2 = tail.tile([128, 8, 512], BF, tag="f2")
              for c in range(8):
                  gd(1 + c % 3, f2[:, c, :], f2_w[128 * c:128 * (c + 1), :])
              f3 = f3b
              W3 = W3b
              b2laTb = tail.tile([128, ET], BF, tag="b2laTb")
              nc.vector.tensor_copy(b2laTb[:], b2laT[:])
              vconT = tail.tile([128, ET], FP, tag="vconT")
              for mt in range(ET):
                  p = psC.tile([128, 1], FP, tag="pd")
                  for kt in range(ET):
                      nc.tensor.matmul(p[:], W3[:, kt, 128 * mt:128 * (mt + 1)],
                                       b2laTb[:, kt:kt + 1], start=(kt == 0), stop=(kt == ET - 1))
                  nc.vector.tensor_scalar(vconT[:, mt:mt + 1], p[:], b3laT[:, mt:mt + 1],
                                          Sla[:, 0:1], OP.add, OP.mult)
              pcxT = tail.tile([128, ET, 2 * BL], BF, tag="pcxT")
              ptc = psC.tile([128, 512], BF, tag="pd")
              for b2 in range(BL):
                  for kt in range(ET):
                      nc.tensor.matmul(ptc[:, 2 * (b2 * ET + kt):2 * (b2 * ET + kt) + 2],
                                       pcxall[:, b2 * D + 128 * kt:b2 * D + 128 * (kt + 1)],
                                       identb[:2, :2], is_transpose=True, skip_group_check=True)
              src_v = ptc[:, :96].rearrange("p (b a h) -> p a b h", b=BL, a=ET)
              dst_v = pcxT[:].rearrange("p a (b h) -> p a b h", h=NH)
              nc.vector.tensor_copy(dst_v, src_v)
              W2 = W2b
              pctxT = tail.tile([128, ET, BL], BF, tag="pctxT")
              pcv = pcxT[:].rearrange("p a (b h) -> p a b h", h=NH)
              for h in range(NH):
                  for mi in range(3):
                      mt = h * 3 + mi
                      p = psC.tile([128, BL], FP, tag="pd")
                      for kt in range(ET):
                          nc.tensor.matmul(p[:], W2[:, kt, 128 * mt:128 * (mt + 1)],
                                           pcv[:, kt, :, h], start=(kt == 0), stop=(kt == ET - 1))
                      nc.vector.tensor_copy(pctxT[:, mt, :], p[:])
              loT = tail.tile([128, ET, BL], BF, tag="loT")
              for mt in range(ET):
                  p = psC.tile([128, BL], FP, tag="pd")
                  for kt in range(ET):
                      nc.tensor.matmul(p[:], W3[:, kt, 128 * mt:128 * (mt + 1)],
                                       pctxT[:, kt, :], start=(kt == 0), stop=(kt == ET - 1))
                  nc.vector.tensor_scalar(loT[:, mt, :], p[:], vconT[:, mt:mt + 1], None, OP.add)

              y1T = tail.tile([128, 8, BL], BF, tag="y1T")
              for mt in range(8):
                  p = psC.tile([128, BL], FP, tag="pd")
                  for kt in range(12):
                      r_ = loT[:, kt, :] if kt < ET else goutT[:, kt - ET, :]
                      nc.tensor.matmul(p[:], f1[:, kt, 128 * mt:128 * (mt + 1)], r_,
                                       start=(kt == 0), stop=(kt == 11))
                  nc.vector.tensor_scalar(y1T[:, mt, :], p[:], b1fT[:, mt:mt + 1], None, OP.add)
              y2T = tail.tile([128, 4, BL], BF, tag="y2T")
              for mt in range(4):
                  p = psC.tile([128, BL], FP, tag="pd")
                  for kt in range(8):
                      nc.tensor.matmul(p[:], f2[:, kt, 128 * mt:128 * (mt + 1)],
                                       y1T[:, kt, :], start=(kt == 0), stop=(kt == 7))
                  nc.scalar.activation(y2T[:, mt, :], p[:], AF.Relu, bias=b2fT[:, mt:mt + 1])
              yT = tail.tile([128, 8, BL], FP, tag="yT")
              for mt in range(8):
                  p = psC.tile([128, BL], FP, tag="pd")
                  for kt in range(4):
                      nc.tensor.matmul(p[:], f3[:, kt, 128 * mt:128 * (mt + 1)],
                                       y2T[:, kt, :], start=(kt == 0), stop=(kt == 3))
                  nc.vector.tensor_scalar(yT[:, mt, :], p[:], b3fT[:, mt:mt + 1], None, OP.add)
              ynat = tail.tile([BL, 1024], FP, tag="ynat")
              for g in range(2):
                  po = psB.tile([128, 512], FP, tag="ptw")
                  for i in range(4):
                      mt = g * 4 + i
                      nc.tensor.matmul(po[:BL, 128 * i:128 * (i + 1)], yT[:, mt, :],
                                       ident[:128, :128], is_transpose=True,
                                       skip_group_check=True)
                  nc.vector.tensor_copy(ynat[:, 512 * g:512 * (g + 1)], po[:BL, :])
              nc.sync.dma_start(out_d[:, :], ynat[:])

    nc.compile()
    return nc


_NC = None


def kernel(**inputs):
    global _NC
    if _NC is None:
        _NC = build()
    B = inputs["image_local_embeds"].shape[0]
    per = B // NCORES
    in_maps = []
    for c in range(NCORES):
        sl = slice(c * per, (c + 1) * per)
        m = {
            "img": np.ascontiguousarray(np.asarray(inputs["image_local_embeds"])[sl], dtype=np.float32),
            "h0": np.ascontiguousarray(np.asarray(inputs["h0"])[sl], dtype=np.float32),
        }
        for k in ["gru_w_ih", "gru_w_hh", "gru_b_ih", "gru_b_hh", "ga_w", "ga_b",
                  "ga_pool", "la_w", "la_b", "la_pool", "go_w", "go_b", "go_pool",
                  "f1_w", "f1_b", "f2_w", "f2_b", "f3_w", "f3_b"]:
            m[k] = np.ascontiguousarray(np.asarray(inputs[k], dtype=np.float32))
        in_maps.append(m)
    res = run_bass_kernel_spmd(_NC, in_maps, core_ids=list(range(NCORES)))
    return np.concatenate([res.results[c]["out"] for c in range(NCORES)], axis=0)



# revision 77
# speedup vs baseline: 1.0939x; 1.0075x over previous
"""Trainium2 Bass kernel for nn_BiVision_VQA2 (B=64,T=32,D=768,N=901).

Data-parallel over batch: 8 batch elems per core x 8 cores.
Key math simplifications (validated vs reference in numpy, rel err ~1e-6):
  - ga/go attention use a single key token -> softmax==1 -> those paths are
    linear in cls; question_embeds is mathematically unused.
  - GRU input `a` is constant over time; wx computed once.
  - local attention: scores = (qemb @ W0_h) @ W1_h^T / sqrt(dk) @ X^T ;
    row-constant score terms (K bias, Q.b1) drop out of softmax; query
    pooling applied to the attention matrix before the @X contraction;
    constant bias terms folded into one vector.
"""

import numpy as np
from contextlib import ExitStack

import concourse.bass as bass
import concourse.tile as tile
from concourse import bacc, mybir
from concourse.bass_utils import run_bass_kernel_spmd
from concourse.masks import make_identity

FP = mybir.dt.float32
FPR = mybir.dt.float32r
OP = mybir.AluOpType
AF = mybir.ActivationFunctionType
BF = mybir.dt.bfloat16
F8 = mybir.dt.float8e4
DR = mybir.MatmulPerfMode.DoubleRow

NCORES = 8
BL = 8
D = 768
T = 32
G = 3 * D
NK = 900
NH = 2
DK = 384
ET = D // 128
RQ = BL * T
USE_FPR = True


def chunks(total):
    out, o = [], 0
    while o < total:
        w = min(512, total - o)
        out.append((o, w))
        o += w
    return out


CH_G = chunks(G)
CH_NK = [(0, 512), (512, 388)]
CH_D = [(0, 512), (512, 256)]


def _r(ap):
    return ap.bitcast(FPR) if USE_FPR else ap


def kchunks(n):
    out, o = [], 0
    while o < n:
        out.append((o, min(128, n - o)))
        o += 128
    return out


import os
PHASES = int(os.environ.get("KPHASES", "4"))


def build():
    nc = bacc.Bacc("TRN2", target_bir_lowering=False, debug=False,
                   enable_asserts=False, num_swdge_queues=4)

    def gd(q, out, in_, **kw):
        inst = nc.gpsimd.dma_start(out, in_, **kw)
        if q:
            inst.ins.queue = f"qPoolDynamic{q}"
        return inst

    img = nc.dram_tensor("img", [BL, 901, D], FP, kind="ExternalInput").ap()
    h0 = nc.dram_tensor("h0", [BL, D], FP, kind="ExternalInput").ap()
    w_ih = nc.dram_tensor("gru_w_ih", [G, D], FP, kind="ExternalInput").ap()
    w_hh = nc.dram_tensor("gru_w_hh", [G, D], FP, kind="ExternalInput").ap()
    b_ih = nc.dram_tensor("gru_b_ih", [G], FP, kind="ExternalInput").ap()
    b_hh = nc.dram_tensor("gru_b_hh", [G], FP, kind="ExternalInput").ap()
    ga_w = nc.dram_tensor("ga_w", [4, D, D], FP, kind="ExternalInput").ap()
    ga_b = nc.dram_tensor("ga_b", [4, D], FP, kind="ExternalInput").ap()
    ga_pool = nc.dram_tensor("ga_pool", [1], FP, kind="ExternalInput").ap()
    la_w = nc.dram_tensor("la_w", [4, D, D], FP, kind="ExternalInput").ap()
    la_b = nc.dram_tensor("la_b", [4, D], FP, kind="ExternalInput").ap()
    la_pool = nc.dram_tensor("la_pool", [T], FP, kind="ExternalInput").ap()
    go_w = nc.dram_tensor("go_w", [4, D, D], FP, kind="ExternalInput").ap()
    go_b = nc.dram_tensor("go_b", [4, D], FP, kind="ExternalInput").ap()
    go_pool = nc.dram_tensor("go_pool", [T], FP, kind="ExternalInput").ap()
    f1_w = nc.dram_tensor("f1_w", [2 * D, 1024], FP, kind="ExternalInput").ap()
    f1_b = nc.dram_tensor("f1_b", [1024], FP, kind="ExternalInput").ap()
    f2_w = nc.dram_tensor("f2_w", [1024, 512], FP, kind="ExternalInput").ap()
    f2_b = nc.dram_tensor("f2_b", [512], FP, kind="ExternalInput").ap()
    f3_w = nc.dram_tensor("f3_w", [512, 1024], FP, kind="ExternalInput").ap()
    f3_b = nc.dram_tensor("f3_b", [1024], FP, kind="ExternalInput").ap()
    out_d = nc.dram_tensor("out", [BL, 1024], FP, kind="ExternalOutput").ap()

    with tile.TileContext(nc) as tc, ExitStack() as ctx:
        cpool = ctx.enter_context(tc.tile_pool(name="const", bufs=1))
        gstate = ctx.enter_context(tc.tile_pool(name="gstate", bufs=2))
        xall = ctx.enter_context(tc.tile_pool(name="xall", bufs=1))
        tailw = ctx.enter_context(tc.tile_pool(name="tailw", bufs=1))
        psB = ctx.enter_context(tc.tile_pool(name="psB", bufs=2, space="PSUM"))
        psC = ctx.enter_context(tc.tile_pool(name="psC", bufs=1, space="PSUM"))

        ident = cpool.tile([128, 128], FP, tag="ident")
        make_identity(nc, ident[:])
        ones1 = cpool.tile([1, 128], FP, tag="ones1")
        nc.vector.memset(ones1[:], 1.0)
        onesT = cpool.tile([T, 128], FP, tag="onesT")
        nc.vector.memset(onesT[:], 1.0)
        identr = cpool.tile([128, 128], FP, tag="identr")
        nc.vector.tensor_copy(_r(identr[:]), ident[:])
        identb = cpool.tile([128, 128], BF, tag="identb")
        nc.vector.tensor_copy(identb[:], ident[:])
        ones1r = cpool.tile([1, 128], FP, tag="ones1r")
        nc.vector.tensor_copy(_r(ones1r[:]), ones1[:])

        def colvec(dram_1d, n, tag):
            nt = n // 128
            t_ = cpool.tile([128, nt], FP, tag=tag)
            for j in range(nt):
                nc.sync.dma_start(t_[:, j:j + 1], dram_1d[j * 128:(j + 1) * 128][:, None])
            return t_

        b2gaT = colvec(ga_b[2], D, "b2gaT")
        b3gaT = colvec(ga_b[3], D, "b3gaT")

        gapool_c = cpool.tile([1, 1], FP, tag="gapool_c")
        nc.sync.dma_start(gapool_c[:], ga_pool[:][:, None])

        def sum_bcast(vcol, k, tag):
            p = psC.tile([128, 1], FP, tag="pd")
            lhs = onesT if k == T else ones1
            nc.tensor.matmul(p[:], lhs[:k, :], vcol[:k, :], start=True, stop=True)
            s = cpool.tile([128, 1], FP, tag=tag)
            nc.vector.tensor_copy(s[:], p[:])
            return s

        Sga = sum_bcast(gapool_c, 1, "Sga")

        qemb8 = cpool.tile([128, ET, BL, T], F8, tag="qemb8")
        goutT = cpool.tile([128, ET, BL], BF, tag="goutT")
        aT = cpool.tile([128, ET, BL], FP, tag="aT")
        bhhN_r = cpool.tile([1, D], FP, tag="bhhN_r")

        # img patch tokens, all 8 batch elems, prefetched early (bf16)
        XnA = xall.tile([128, BL, 8, D], BF, tag="XnA")
        # early-persisted tail weights (DMAs issued pre-loop, overlap GRU)
        W2b = tailw.tile([128, ET, D], BF, tag="W2b")
        W3b = tailw.tile([128, ET, D], BF, tag="W3b")
        f3b = tailw.tile([128, 4, 1024], BF, tag="f3b")

        # ================= phase A: cls -> a (ga path only) ===============
        clsTb = cpool.tile([128, ET, BL], BF, tag="clsTb")
        clsT8 = cpool.tile([128, ET, BL], F8, tag="clsT8")

        def dense_T(pool, wdt, w_nat_dram, rhsT, biasT, scaleT, otile, wtag, dmaq):
            wsb = pool.tile([128, ET, D], wdt, tag=wtag)
            for c in range(ET):
                dmaq.dma_start(wsb[:, c, :], w_nat_dram[128 * c:128 * (c + 1), :])
            for mt in range(ET):
                p = psC.tile([128, BL], FP, tag="pd")
                for kt in range(ET):
                    nc.tensor.matmul(p[:], wsb[:, kt, 128 * mt:128 * (mt + 1)],
                                     rhsT[:, kt, :], start=(kt == 0), stop=(kt == ET - 1))
                if scaleT is None:
                    nc.vector.tensor_scalar(otile[:, mt, :], p[:], biasT[:, mt:mt + 1],
                                            None, OP.add)
                else:
                    nc.vector.tensor_scalar(otile[:, mt, :], p[:], biasT[:, mt:mt + 1],
                                            scaleT[:, 0:1], OP.add, OP.mult)

        with tc.tile_pool(name="ph0", bufs=1) as ph0:
            clsn = ph0.tile([BL, D], FP, tag="clsn")
            nc.sync.dma_start(clsn[:], img[0:BL, 0, :])
            ptr = psC.tile([128, 512], FP, tag="pd")
            for kt in range(ET):
                nc.tensor.matmul(ptr[:, 8 * kt:8 * kt + 8], clsn[:, 128 * kt:128 * (kt + 1)],
                                 ident[:BL, :BL], is_transpose=True, skip_group_check=True)
            clsT = ph0.tile([128, ET, BL], FP, tag="clsT")
            nc.vector.tensor_copy(clsT[:].rearrange("p a b -> p (a b)"), ptr[:, :8 * ET])
            nc.scalar.copy(clsTb[:], clsT[:])
            nc.scalar.copy(clsT8[:], clsT[:])

        # ================= phase B: GRU (fp8 DoubleRow) ===================
        cde = ctx.enter_context(tc.tile_pool(name="cde", bufs=1))
        QtT = cde.tile([128, ET, NH * RQ], BF, tag="QtT")
        prep_cm = tc.tile_pool(name="prep", bufs=1)
        prep = prep_cm.__enter__()
        wb8_cm = tc.tile_pool(name="wb8", bufs=1)
        wb8 = wb8_cm.__enter__()
        WhhT8 = wb8.tile([128, ET, G], F8, tag="WhhT8")

        with tc.tile_pool(name="wpro", bufs=1) as wpro, \
             tc.tile_pool(name="wnat", bufs=4) as wnat:
            combr = wpro.tile([1, 2 * D], FP, tag="combr")
            nc.sync.dma_start(combr[:], b_ih[0:2 * D][None, :])
            nc.gpsimd.dma_start(combr[:], b_hh[0:2 * D][None, :], accum_op=OP.add)
            bhhN_t = wpro.tile([1, D], FP, tag="bhhN_t")
            nc.sync.dma_start(bhhN_t[:], b_hh[2 * D:3 * D][None, :])
            nc.vector.tensor_copy(_r(bhhN_r[:]), bhhN_t[:])
            bihN = wpro.tile([1, D], FP, tag="bhhN_t", name="bihN")
            nc.sync.dma_start(bihN[:], b_ih[2 * D:3 * D][None, :])


            WihT8 = prep.tile([128, ET, G], F8, tag="gow3", name="WihT8")

            def build_W8(w_dram, dst):
                jts = kchunks(G)
                for g0 in range(0, len(jts), 4):
                    grp = jts[g0:g0 + 4]
                    nats = []
                    for qi, (j0, jw) in enumerate(grp):
                        wn = wnat.tile([128, D], BF, tag="wn")
                        gd(qi % 4, wn[:jw, :], w_dram[j0:j0 + jw, :])
                        nats.append((wn, j0, jw))
                    for et in range(ET):
                        pt = psB.tile([128, 512], BF, tag="ptw")
                        for i, (wn, j0, jw) in enumerate(nats):
                            nc.tensor.matmul(pt[:, 128 * i:128 * i + jw],
                                             wn[:jw, 128 * et:128 * (et + 1)],
                                             identb[:jw, :jw], is_transpose=True,
                                             skip_group_check=True)
                        w0 = grp[0][0]
                        wlen = sum(jw for (_, _, jw) in nats)
                        if et % 2 == 0:
                            nc.vector.tensor_copy(dst[:, et, w0:w0 + wlen], pt[:, :wlen])
                        else:
                            nc.scalar.copy(dst[:, et, w0:w0 + wlen], pt[:, :wlen])

            gaw2 = prep.tile([128, ET, D], F8, tag="W08", name="gaw2")
            for c in range(ET):
                gd(1 + c % 3, gaw2[:, c, :], ga_w[2][128 * c:128 * (c + 1), :])
            gaw3 = prep.tile([128, ET, D], F8, tag="gow2", name="gaw3")
            for c in range(ET):
                gd(1 + c % 3, gaw3[:, c, :], ga_w[3][128 * c:128 * (c + 1), :])

            build_W8(w_ih, WihT8)

            A2T = wpro.tile([128, ET, BL], F8, tag="A2T")
            for mt in range(ET):
                p = psC.tile([128, BL], FP, tag="pd")
                for kt in range(ET):
                    nc.tensor.matmul(p[:], gaw2[:, kt, 128 * mt:128 * (mt + 1)],
                                     clsT8[:, kt, :], start=(kt == 0), stop=(kt == ET - 1))
                nc.vector.tensor_scalar(A2T[:, mt, :], p[:], b2gaT[:, mt:mt + 1],
                                        None, OP.add)
            for mt in range(ET):
                p = psC.tile([128, BL], FP, tag="pd")
                for kt in range(ET):
                    nc.tensor.matmul(p[:], gaw3[:, kt, 128 * mt:128 * (mt + 1)],
                                     A2T[:, kt, :], start=(kt == 0), stop=(kt == ET - 1))
                nc.vector.tensor_scalar(aT[:, mt, :], p[:], b3gaT[:, mt:mt + 1],
                                        Sga[:, 0:1], OP.add, OP.mult)
            aT8 = cpool.tile([128, ET, 128], F8, tag="aT8")
            nc.vector.memset(aT8[:].rearrange("p a b -> p (a b)"), 0.0)
            nc.vector.tensor_copy(aT8[:, :, 0:BL], aT[:])

            hnat = prep.tile([BL, D], FP, tag="W08", name="hnat")
            nc.sync.dma_start(hnat[:], h0[:, :])
            ptr0 = psC.tile([128, 512], FP, tag="pd")
            for kt in range(ET):
                nc.tensor.matmul(ptr0[:, 8 * kt:8 * kt + 8], hnat[:, 128 * kt:128 * (kt + 1)],
                                 ident[:BL, :BL], is_transpose=True, skip_group_check=True)
            hT32 = gstate.tile([128, ET, BL], FP, tag="hT32")
            nc.vector.tensor_copy(hT32[:].rearrange("p a b -> p (a b)"), ptr0[:, :8 * ET])
            h8_a = cpool.tile([128, ET, 128], F8, tag="h8_a")
            h8_b = cpool.tile([128, ET, 128], F8, tag="h8_b")
            nc.vector.memset(h8_a[:].rearrange("p a b -> p (a b)"), 0.0)
            nc.vector.memset(h8_b[:].rearrange("p a b -> p (a b)"), 0.0)
            nc.scalar.copy(h8_a[:, :, 0:BL], hT32[:])
            h8 = h8_a

            # wx (+ all biases folded): rz sections get bih+bhh, n gets bih
            wxbRZ = prep.tile([BL, 2 * D], FP, tag="wxbRZ")
            wxbN = prep.tile([BL, D], FP, tag="wxbN")
            for (j0, jw) in CH_G:
                p = psC.tile([128, 512], FP, tag="pd")
                for kp in range(3):
                    nc.tensor.matmul(p[:, :jw], aT8[:, 2 * kp:2 * kp + 2, :],
                                     WihT8[:, 2 * kp:2 * kp + 2, j0:j0 + jw],
                                     start=(kp == 0), stop=False, perf_mode=DR)
                src = combr[:, j0:j0 + jw] if j0 < 2 * D else bihN[:, j0 - 2 * D:j0 - 2 * D + jw]
                nc.tensor.matmul(p[:, :jw], ones1[:1, :], src,
                                 start=False, stop=True)
                if j0 < 2 * D:
                    nc.vector.tensor_copy(_r(wxbRZ[:, j0:j0 + jw]), p[:BL, :jw])
                else:
                    nc.vector.tensor_copy(wxbN[:, j0 - 2 * D:j0 - 2 * D + jw], p[:BL, :jw])

            build_W8(w_hh, WhhT8)


        # ---- deferred small constants (off the build critical path) ------
        b2goT = colvec(go_b[2], D, "b2goT")
        b3goT = colvec(go_b[3], D, "b3goT")
        b0laT = colvec(la_b[0], D, "b0laT")
        b2laT = colvec(la_b[2], D, "b2laT")
        b3laT = colvec(la_b[3], D, "b3laT")
        b1fT = colvec(f1_b, 1024, "b1fT")
        b2fT = colvec(f2_b, 512, "b2fT")
        b3fT = colvec(f3_b, 1024, "b3fT")
        lapool_c = cpool.tile([T, 1], FP, tag="lapool_c")
        nc.sync.dma_start(lapool_c[:], la_pool[:][:, None])
        gopool_c = cpool.tile([T, 1], FP, tag="gopool_c")
        nc.sync.dma_start(gopool_c[:], go_pool[:][:, None])
        Sla = sum_bcast(lapool_c, T, "Sla")
        Sgo = sum_bcast(gopool_c, T, "Sgo")
        pmask = cpool.tile([64, 2], FP, tag="pmask")
        nc.vector.memset(pmask[:], 0.0)
        nc.sync.dma_start(pmask[0:T, 0:1], la_pool[:][:, None])
        nc.sync.dma_start(pmask[T:2 * T, 1:2], la_pool[:][:, None])

        # ---- pre-loop early DMA emission (overlaps the GRU steps) --------
        for b in range(BL):
            nc.vector.memset(XnA[:, b, 7, :], 0.0)
            for c, (k0, kw) in enumerate(kchunks(NK)):
                gd(1 + (b * 8 + c) % 3, XnA[:kw, b, c, :], img[b, 1 + k0:1 + k0 + kw, :])
        W08 = prep.tile([128, ET, D], F8, tag="W08")
        for c in range(ET):
            gd(1 + c % 3, W08[:, c, :], la_w[0][128 * c:128 * (c + 1), :])
        gow2 = prep.tile([128, ET, D], BF, tag="gow2")
        for c in range(ET):
            gd(1 + c % 3, gow2[:, c, :], go_w[2][128 * c:128 * (c + 1), :])
        for c in range(ET):
            gd(1 + c % 3, W2b[:, c, :], la_w[2][128 * c:128 * (c + 1), :])
        for c in range(ET):
            gd(1 + c % 3, W3b[:, c, :], la_w[3][128 * c:128 * (c + 1), :])
        for c in range(4):
            gd(1 + c % 3, f3b[:, c, :], f3_w[128 * c:128 * (c + 1), :])
        # stall-prone loads (wait on in-loop readers) go last on queue 0
        gow3 = prep.tile([128, ET, D], BF, tag="gow3")
        for c in range(ET):
            gd(0, gow3[:, c, :], go_w[3][128 * c:128 * (c + 1), :])
        # W1 lands in gow2's buffer once the go stage-1 matmuls are done
        W1n = prep.tile([128, ET, D], BF, tag="gow2", name="W1n")
        for c in range(ET):
            gd(0, W1n[:, c, :], la_w[1][128 * c:128 * (c + 1), :])

        with tc.tile_pool(name="g1", bufs=1) as g1, \
             tc.tile_pool(name="psG", bufs=1, space="PSUM") as psG:

            # section psums: A = j[0:1024] (r + z1), Z = j[1024:1536] (z2),
            # N = j[1536:2304] (n); emission order A0 A1 N0 N1 Z
            STEP_CHUNKS = [("a", 0, 0, 512), ("a", 512, 512, 512),
                           ("n", 0, 1536, 512), ("n", 512, 2048, 256),
                           ("z", 0, 1024, 512)]
            KSTEPS = int(os.environ.get("KSTEPS", str(T)))
            KFILL = int(os.environ.get("KFILL", "2"))
            fill_i = 0
            for t in range(KSTEPS):
                hp8, hp32 = h8, hT32
                pA = psG.tile([128, 1024], FP, tag="a")
                pN = psG.tile([128, D], FP, tag="n")
                pZ = psG.tile([128, 512], FP, tag="z")
                tiles = {"a": pA, "n": pN, "z": pZ}
                for (sec, c0, j0, jw) in STEP_CHUNKS:
                    p = tiles[sec]
                    for kp in range(3):
                        nc.tensor.matmul(p[:, c0:c0 + jw], hp8[:, 2 * kp:2 * kp + 2, :],
                                         WhhT8[:, 2 * kp:2 * kp + 2, j0:j0 + jw],
                                         start=(kp == 0), stop=False, perf_mode=DR)
                    if j0 >= 2 * D:
                        nc.tensor.matmul(p[:, c0:c0 + jw], _r(ones1r[:1, :]),
                                         _r(bhhN_r[:, j0 - 2 * D:j0 - 2 * D + jw]),
                                         start=False, stop=True)
                    else:
                        nc.tensor.matmul(p[:, c0:c0 + jw], _r(identr[:BL, :]),
                                         _r(wxbRZ[:, j0:j0 + jw]), start=False, stop=True)
                r_sig = g1.tile([BL, D], FP, tag="rsig")
                nc.scalar.activation(r_sig[:], pA[:BL, 0:768], AF.Sigmoid)
                z_nat = g1.tile([BL, D], FP, tag="znat")
                nc.scalar.activation(z_nat[:, 0:256], pA[:BL, 768:1024], AF.Sigmoid)
                rwn = g1.tile([BL, D], FP, tag="rwn")
                nc.vector.tensor_mul(rwn[:], r_sig[:], pN[:BL, :])
                npre = rwn
                nc.vector.tensor_add(npre[:], rwn[:], wxbN[:])
                nc.scalar.activation(z_nat[:, 256:768], pZ[:BL, :], AF.Sigmoid)
                nt_ = g1.tile([BL, D], FP, tag="nt")
                nc.scalar.activation(nt_[:, 0:512], npre[:, 0:512], AF.Tanh)
                nc.scalar.activation(nt_[:, 512:768], npre[:, 512:768], AF.Tanh)
                # fills between mm block and transposes keep PE p-state hot
                for _ in range(KFILL // 2):
                    pf = psB.tile([128, 512], FP, tag="ptw")
                    for kp in range(3):
                        nc.tensor.matmul(pf[:, 0:512], hp8[:, 2 * kp:2 * kp + 2, :],
                                         WhhT8[:, 2 * kp:2 * kp + 2, 512 * (fill_i % 4):512 * (fill_i % 4) + 512],
                                         start=(kp == 0), stop=(kp == 2), perf_mode=DR)
                    fill_i += 1
                ptt = psC.tile([128, 512], FP, tag="pd")
                for kt in range(ET):
                    nc.tensor.matmul(ptt[:, 8 * kt:8 * kt + 8], z_nat[:, 128 * kt:128 * (kt + 1)],
                                     ident[:BL, :BL], is_transpose=True, skip_group_check=True)
                for kt in range(ET):
                    nc.tensor.matmul(ptt[:, 64 + 8 * kt:64 + 8 * kt + 8],
                                     nt_[:, 128 * kt:128 * (kt + 1)],
                                     ident[:BL, :BL], is_transpose=True, skip_group_check=True)
                zT = ptt[:, 0:48]
                ntT = ptt[:, 64:112]
                if t < KSTEPS - 1:
                    for _ in range(KFILL - KFILL // 2):
                        pf = psB.tile([128, 512], FP, tag="ptw")
                        for kp in range(3):
                            nc.tensor.matmul(pf[:, 0:512], hp8[:, 2 * kp:2 * kp + 2, :],
                                             WhhT8[:, 2 * kp:2 * kp + 2, 512 * (fill_i % 4):512 * (fill_i % 4) + 512],
                                             start=(kp == 0), stop=(kp == 2), perf_mode=DR)
                        fill_i += 1
                dT = g1.tile([128, 8 * ET], FP, tag="dT")
                nc.vector.tensor_sub(dT[:], hp32[:].rearrange("p a b -> p (a b)"), ntT)
                zdT = g1.tile([128, 8 * ET], FP, tag="zdT")
                nc.vector.tensor_mul(zdT[:], zT, dT[:])
                hT32 = gstate.tile([128, ET, BL], FP, tag="hT32")
                nc.vector.tensor_add(hT32[:].rearrange("p a b -> p (a b)"), ntT, zdT[:])
                h8 = h8_b if t % 2 == 0 else h8_a
                nc.vector.tensor_add(h8[:, :, 0:BL],
                                     ntT.rearrange("p (a b) -> p a b", a=ET),
                                     zdT[:].rearrange("p (a b) -> p a b", a=ET))
                nc.scalar.copy(qemb8[:, :, :, t], hT32[:])
                if t == 24:
                    G2Tb = prep.tile([128, ET, BL], BF, tag="G2Tb")
                    for mt in range(ET):
                        p2 = psB.tile([128, 512], FP, tag="ptw")
                        for kt in range(ET):
                            nc.tensor.matmul(p2[:, 0:BL], gow2[:, kt, 128 * mt:128 * (mt + 1)],
                                             clsTb[:, kt, :], start=(kt == 0),
                                             stop=(kt == ET - 1))
                        nc.vector.tensor_scalar(G2Tb[:, mt, :], p2[:, 0:BL],
                                                b2goT[:, mt:mt + 1], None, OP.add)
                if t == 26:
                    for mt in range(ET):
                        p2 = psB.tile([128, 512], FP, tag="ptw")
                        for kt in range(ET):
                            nc.tensor.matmul(p2[:, 0:BL], gow3[:, kt, 128 * mt:128 * (mt + 1)],
                                             G2Tb[:, kt, :], start=(kt == 0),
                                             stop=(kt == ET - 1))
                        nc.vector.tensor_scalar(goutT[:, mt, :], p2[:, 0:BL],
                                                b3goT[:, mt:mt + 1], Sgo[:, 0:1],
                                                OP.add, OP.mult)

        wb8_cm.__exit__(None, None, None)

        # ================= phase C: Q^T, W1^T, Qt^T =======================
        if PHASES >= 2:
          if True:
              QT8 = prep.tile([128, ET, RQ], F8, tag="QT8")
              qflat = qemb8[:].rearrange("p a b t -> p a (b t)")
              for mt in range(ET):
                  p = psC.tile([128, RQ], FP, tag="pd")
                  for kp in range(3):
                      nc.tensor.matmul(p[:], W08[:, 2 * kp:2 * kp + 2, 128 * mt:128 * (mt + 1)],
                                       qflat[:, 2 * kp:2 * kp + 2, :],
                                       start=(kp == 0), stop=(kp == 2), perf_mode=DR)
                  nc.vector.tensor_scalar(QT8[:, mt, :], p[:], b0laT[:, mt:mt + 1], None, OP.add)
              W1T8 = prep.tile([128, ET, D], F8, tag="W08", name="W1T8")
              for hd in range(ET):
                  for grp in range(2):
                      pt2 = psB.tile([128, 512], BF, tag="ptw")
                      for i in range(3):
                          e2 = grp * 3 + i
                          nc.tensor.matmul(pt2[:, 128 * i:128 * (i + 1)],
                                           W1n[:, e2, 128 * hd:128 * (hd + 1)],
                                           identb[:], is_transpose=True, skip_group_check=True)
                      if grp == 0:
                          nc.vector.tensor_copy(W1T8[:, hd, 0:384], pt2[:, 0:384])
                      else:
                          nc.scalar.copy(W1T8[:, hd, 384:768], pt2[:, 0:384])
              scl = 1.0 / float(np.sqrt(DK))
              for h in range(NH):
                  for mt in range(ET):
                      p = psC.tile([128, RQ], FP, tag="pd")
                      nc.tensor.matmul(p[:], W1T8[:, 3 * h:3 * h + 2, 128 * mt:128 * (mt + 1)],
                                       QT8[:, 3 * h:3 * h + 2, :],
                                       start=True, stop=False, perf_mode=DR)
                      nc.tensor.matmul(p[:], W1T8[:, 3 * h + 2, 128 * mt:128 * (mt + 1)],
                                       QT8[:, 3 * h + 2, :], start=False, stop=True)
                      dst = QtT[:, mt, :].rearrange("p (b h2 t) -> p b h2 t",
                                                    h2=NH, t=T)[:, :, h, :]
                      nc.scalar.activation(dst, p[:], AF.Copy, scale=scl)
        prep_cm.__exit__(None, None, None)

        # ================= phase D: per-b attention =======================
        de = ctx.enter_context(tc.tile_pool(name="de", bufs=1))
        pcxall = de.tile([2, BL * D], BF, tag="pcxall")
        f1 = de.tile([128, 12, 1024], BF, tag="f1")
        for c in range(12):
            gd(1 + c % 3, f1[:, c, :], f1_w[128 * c:128 * (c + 1), :])

        if PHASES >= 3:
            with tc.tile_pool(name="xb", bufs=2) as xb, \
                 tc.tile_pool(name="ab", bufs=1) as ab, \
                 tc.tile_pool(name="psD", bufs=1, space="PSUM") as psA:
              KC = kchunks(NK)
              for b in range(BL):
                  Xn = XnA[:, b, :, :]
                  XT = xb.tile([128, ET, NK], BF, tag="XT")
                  cpeng = [nc.vector.tensor_copy, nc.scalar.copy, nc.gpsimd.tensor_copy]
                  for et in range(ET):
                      for g in range(2):
                          pt = psB.tile([128, 512], BF, tag="ptw")
                          for i in range(4):
                              c = g * 4 + i
                              nc.tensor.matmul(pt[:, 128 * i:128 * (i + 1)],
                                               Xn[:, c, 128 * et:128 * (et + 1)],
                                               identb[:], is_transpose=True,
                                               skip_group_check=True)
                          w = 512 if g == 0 else NK - 512
                          cpeng[(et * 2 + g) % 2](XT[:, et, 512 * g:512 * g + w], pt[:, :w])
                  att = ab.tile([64, NK], BF, tag="att")
                  zacc = ab.tile([64, 2], FP, tag="zacc")
                  for ci, (n0, nw) in enumerate(CH_NK):
                      p = psA.tile([64, 512], FP, tag=f"wh{ci}")
                      for kt in range(ET):
                          nc.tensor.matmul(p[:, :nw],
                                           QtT[:, kt, b * 2 * T:(b + 1) * 2 * T],
                                           XT[:, kt, n0:n0 + nw],
                                           start=(kt == 0), stop=(kt == ET - 1))
                      nc.scalar.activation(att[:, n0:n0 + nw], p[:, :nw], AF.Exp,
                                           accum_out=zacc[:, ci:ci + 1])
                  zs = ab.tile([64, 1], FP, tag="zs")
                  nc.vector.tensor_add(zs[:], zacc[:, 0:1], zacc[:, 1:2])
                  rz = ab.tile([64, 1], FP, tag="rz1")
                  nc.vector.reciprocal(rz[:], zs[:])
                  wm = ab.tile([64, 2], BF, tag="wm")
                  nc.vector.tensor_scalar(wm[:], pmask[:], rz[:, 0:1], None, OP.mult)
                  pa_sb = ab.tile([2, NK], BF, tag="pa_sb")
                  for ci, (n0, nw) in enumerate(CH_NK):
                      p = psA.tile([2, 512], FP, tag=f"wh{2 + ci}")
                      nc.tensor.matmul(p[:, :nw], wm[:], att[:, n0:n0 + nw],
                                       start=True, stop=True)
                      nc.vector.tensor_copy(pa_sb[:, n0:n0 + nw], p[:, :nw])
                  paT = ab.tile([128, len(KC), 2], BF, tag="paT")
                  nc.vector.memset(paT[:].rearrange("p a b -> p (a b)"), 0.0)
                  ptp = psC.tile([128, 512], BF, tag="pd")
                  for c, (k0, kw) in enumerate(KC):
                      nc.tensor.matmul(ptp[:kw, 2 * c:2 * c + 2], pa_sb[:, k0:k0 + kw],
                                       identb[:2, :2], is_transpose=True, skip_group_check=True)
                      nc.vector.tensor_copy(paT[:kw, c, :], ptp[:kw, 2 * c:2 * c + 2])
                  for ci, (n0, nw) in enumerate(CH_D):
                      p = psA.tile([2, 512], FP, tag=f"wh{4 - ci}")
                      for c in range(len(KC)):
                          nc.tensor.matmul(p[:, :nw], paT[:, c, :],
                                           Xn[:, c, n0:n0 + nw],
                                           start=(c == 0), stop=(c == len(KC) - 1))
                      nc.vector.tensor_copy(pcxall[:, b * D + n0:b * D + n0 + nw], p[:, :nw])

        # ================= phase E: projections + MLP =====================
        if PHASES >= 4:
            with tc.tile_pool(name="tail", bufs=1) as tail:
              f2 = tail.tile([128, 8, 512], BF, tag="f2")
              for c in range(8):
                  gd(1 + c % 3, f2[:, c, :], f2_w[128 * c:128 * (c + 1), :])
              f3 = f3b
              W3 = W3b
              b2laTb = tail.tile([128, ET], BF, tag="b2laTb")
              nc.vector.tensor_copy(b2laTb[:], b2laT[:])
              vconT = tail.tile([128, ET], FP, tag="vconT")
              for mt in range(ET):
                  p = psC.tile([128, 1], FP, tag="pd")
                  for kt in range(ET):
                      nc.tensor.matmul(p[:], W3[:, kt, 128 * mt:128 * (mt + 1)],
                                       b2laTb[:, kt:kt + 1], start=(kt == 0), stop=(kt == ET - 1))
                  nc.vector.tensor_scalar(vconT[:, mt:mt + 1], p[:], b3laT[:, mt:mt + 1],
                                          Sla[:, 0:1], OP.add, OP.mult)
              pcxT = tail.tile([128, ET, 2 * BL], BF, tag="pcxT")
              ptc = psC.tile([128, 512], BF, tag="pd")
              for b2 in range(BL):
                  for kt in range(ET):
                      nc.tensor.matmul(ptc[:, 2 * (b2 * ET + kt):2 * (b2 * ET + kt) + 2],
                                       pcxall[:, b2 * D + 128 * kt:b2 * D + 128 * (kt + 1)],
                                       identb[:2, :2], is_transpose=True, skip_group_check=True)
              src_v = ptc[:, :96].rearrange("p (b a h) -> p a b h", b=BL, a=ET)
              dst_v = pcxT[:].rearrange("p a (b h) -> p a b h", h=NH)
              nc.vector.tensor_copy(dst_v, src_v)
              W2 = W2b
              pctxT = tail.tile([128, ET, BL], BF, tag="pctxT")
              pcv = pcxT[:].rearrange("p a (b h) -> p a b h", h=NH)
              for h in range(NH):
                  for mi in range(3):
                      mt = h * 3 + mi
                      p = psC.tile([128, BL], FP, tag="pd")
                      for kt in range(ET):
                          nc.tensor.matmul(p[:], W2[:, kt, 128 * mt:128 * (mt + 1)],
                                           pcv[:, kt, :, h], start=(kt == 0), stop=(kt == ET - 1))
                      nc.vector.tensor_copy(pctxT[:, mt, :], p[:])
              loT = tail.tile([128, ET, BL], BF, tag="loT")
              for mt in range(ET):
                  p = psC.tile([128, BL], FP, tag="pd")
                  for kt in range(ET):
                      nc.tensor.matmul(p[:], W3[:, kt, 128 * mt:128 * (mt + 1)],
                                       pctxT[:, kt, :], start=(kt == 0), stop=(kt == ET - 1))
                  nc.vector.tensor_scalar(loT[:, mt, :], p[:], vconT[:, mt:mt + 1], None, OP.add)

              y1T = tail.tile([128, 8, BL], BF, tag="y1T")
              for mt in range(8):
                  p = psC.tile([128, BL], FP, tag="pd")
                  for kt in range(12):
                      r_ = loT[:, kt, :] if kt < ET else goutT[:, kt - ET, :]
                      nc.tensor.matmul(p[:], f1[:, kt, 128 * mt:128 * (mt + 1)], r_,
                                       start=(kt == 0), stop=(kt == 11))
                  nc.vector.tensor_scalar(y1T[:, mt, :], p[:], b1fT[:, mt:mt + 1], None, OP.add)
              y2T = tail.tile([128, 4, BL], BF, tag="y2T")
              for mt in range(4):
                  p = psC.tile([128, BL], FP, tag="pd")
                  for kt in range(8):
                      nc.tensor.matmul(p[:], f2[:, kt, 128 * mt:128 * (mt + 1)],
                                       y1T[:, kt, :], start=(kt == 0), stop=(kt == 7))
                  nc.scalar.activation(y2T[:, mt, :], p[:], AF.Relu, bias=b2fT[:, mt:mt + 1])
              yT = tail.tile([128, 8, BL], FP, tag="yT")
              for mt in range(8):
                  p = psC.tile([128, BL], FP, tag="pd")
                  for kt in range(4):
                      nc.tensor.matmul(p[:], f3[:, kt, 128 * mt:128 * (mt + 1)],
                                       y2T[:, kt, :], start=(kt == 0), stop=(kt == 3))
                  nc.vector.tensor_scalar(yT[:, mt, :], p[:], b3fT[:, mt:mt + 1], None, OP.add)
              ynat = tail.tile([BL, 1024], FP, tag="ynat")
              for g in range(2):
                  po = psB.tile([128, 512], FP, tag="ptw")
                  for i in range(4):
                      mt = g * 4 + i
                      nc.tensor.matmul(po[:BL, 128 * i:128 * (i + 1)], yT[:, mt, :],
                                       ident[:128, :128], is_transpose=True,
                                       skip_group_check=True)
                  nc.vector.tensor_copy(ynat[:, 512 * g:512 * (g + 1)], po[:BL, :])
              nc.sync.dma_start(out_d[:, :], ynat[:])

    nc.compile()
    return nc


_NC = None


def kernel(**inputs):
    global _NC
    if _NC is None:
        _NC = build()
    B = inputs["image_local_embeds"].shape[0]
    per = B // NCORES
    in_maps = []
    for c in range(NCORES):
        sl = slice(c * per, (c + 1) * per)
        m = {
            "img": np.ascontiguousarray(np.asarray(inputs["image_local_embeds"])[sl], dtype=np.float32),
            "h0": np.ascontiguousarray(np.asarray(inputs["h0"])[sl], dtype=np.float32),
        }
        for k in ["gru_w_ih", "gru_w_hh", "gru_b_ih", "gru_b_hh", "ga_w", "ga_b",
                  "ga_pool", "la_w", "la_b", "la_pool", "go_w", "go_b", "go_pool",
                  "f1_w", "f1_b", "f2_w", "f2_b", "f3_w", "f3_b"]:
            m[k] = np.ascontiguousarray(np.asarray(inputs[k], dtype=np.float32))
        in_maps.append(m)
    res = run_bass_kernel_spmd(_NC, in_maps, core_ids=list(range(NCORES)))
    return np.concatenate([res.results[c]["out"] for c in range(NCORES)], axis=0)



# revision 84
# speedup vs baseline: 1.1069x; 1.0119x over previous
"""Trainium2 Bass kernel for nn_BiVision_VQA2 (B=64,T=32,D=768,N=901).

Data-parallel over batch: 8 batch elems per core x 8 cores.
Key math simplifications (validated vs reference in numpy, rel err ~1e-6):
  - ga/go attention use a single key token -> softmax==1 -> those paths are
    linear in cls; question_embeds is mathematically unused.
  - GRU input `a` is constant over time; wx computed once.
  - local attention: scores = (qemb @ W0_h) @ W1_h^T / sqrt(dk) @ X^T ;
    row-constant score terms (K bias, Q.b1) drop out of softmax; query
    pooling applied to the attention matrix before the @X contraction;
    constant bias terms folded into one vector.
"""

import numpy as np
from contextlib import ExitStack

import concourse.bass as bass
import concourse.tile as tile
from concourse import bacc, mybir
from concourse.bass_utils import run_bass_kernel_spmd
from concourse.masks import make_identity

FP = mybir.dt.float32
FPR = mybir.dt.float32r
OP = mybir.AluOpType
AF = mybir.ActivationFunctionType
BF = mybir.dt.bfloat16
F8 = mybir.dt.float8e4
DR = mybir.MatmulPerfMode.DoubleRow

NCORES = 8
BL = 8
D = 768
T = 32
G = 3 * D
NK = 900
NH = 2
DK = 384
ET = D // 128
RQ = BL * T
USE_FPR = True


def chunks(total):
    out, o = [], 0
    while o < total:
        w = min(512, total - o)
        out.append((o, w))
        o += w
    return out


CH_G = chunks(G)
CH_NK = [(0, 512), (512, 388)]
CH_D = [(0, 512), (512, 256)]


def _r(ap):
    return ap.bitcast(FPR) if USE_FPR else ap


def kchunks(n):
    out, o = [], 0
    while o < n:
        out.append((o, min(128, n - o)))
        o += 128
    return out


import os
PHASES = int(os.environ.get("KPHASES", "4"))


def build():
    nc = bacc.Bacc("TRN2", target_bir_lowering=False, debug=False,
                   enable_asserts=False, num_swdge_queues=4)

    def gd(q, out, in_, **kw):
        inst = nc.gpsimd.dma_start(out, in_, **kw)
        if q:
            inst.ins.queue = f"qPoolDynamic{q}"
        return inst

    img = nc.dram_tensor("img", [BL, 901, D], FP, kind="ExternalInput").ap()
    h0 = nc.dram_tensor("h0", [BL, D], FP, kind="ExternalInput").ap()
    w_ih = nc.dram_tensor("gru_w_ih", [G, D], FP, kind="ExternalInput").ap()
    w_hh = nc.dram_tensor("gru_w_hh", [G, D], FP, kind="ExternalInput").ap()
    b_ih = nc.dram_tensor("gru_b_ih", [G], FP, kind="ExternalInput").ap()
    b_hh = nc.dram_tensor("gru_b_hh", [G], FP, kind="ExternalInput").ap()
    ga_w = nc.dram_tensor("ga_w", [4, D, D], FP, kind="ExternalInput").ap()
    ga_b = nc.dram_tensor("ga_b", [4, D], FP, kind="ExternalInput").ap()
    ga_pool = nc.dram_tensor("ga_pool", [1], FP, kind="ExternalInput").ap()
    la_w = nc.dram_tensor("la_w", [4, D, D], FP, kind="ExternalInput").ap()
    la_b = nc.dram_tensor("la_b", [4, D], FP, kind="ExternalInput").ap()
    la_pool = nc.dram_tensor("la_pool", [T], FP, kind="ExternalInput").ap()
    go_w = nc.dram_tensor("go_w", [4, D, D], FP, kind="ExternalInput").ap()
    go_b = nc.dram_tensor("go_b", [4, D], FP, kind="ExternalInput").ap()
    go_pool = nc.dram_tensor("go_pool", [T], FP, kind="ExternalInput").ap()
    f1_w = nc.dram_tensor("f1_w", [2 * D, 1024], FP, kind="ExternalInput").ap()
    f1_b = nc.dram_tensor("f1_b", [1024], FP, kind="ExternalInput").ap()
    f2_w = nc.dram_tensor("f2_w", [1024, 512], FP, kind="ExternalInput").ap()
    f2_b = nc.dram_tensor("f2_b", [512], FP, kind="ExternalInput").ap()
    f3_w = nc.dram_tensor("f3_w", [512, 1024], FP, kind="ExternalInput").ap()
    f3_b = nc.dram_tensor("f3_b", [1024], FP, kind="ExternalInput").ap()
    out_d = nc.dram_tensor("out", [BL, 1024], FP, kind="ExternalOutput").ap()

    with tile.TileContext(nc) as tc, ExitStack() as ctx:
        cpool = ctx.enter_context(tc.tile_pool(name="const", bufs=1))
        gstate = ctx.enter_context(tc.tile_pool(name="gstate", bufs=2))
        xall = ctx.enter_context(tc.tile_pool(name="xall", bufs=1))
        tailw = ctx.enter_context(tc.tile_pool(name="tailw", bufs=1))
        psB = ctx.enter_context(tc.tile_pool(name="psB", bufs=2, space="PSUM"))
        psC = ctx.enter_context(tc.tile_pool(name="psC", bufs=1, space="PSUM"))

        ident = cpool.tile([128, 128], FP, tag="ident")
        make_identity(nc, ident[:])
        ones1 = cpool.tile([1, 128], FP, tag="ones1")
        nc.vector.memset(ones1[:], 1.0)
        onesT = cpool.tile([T, 128], FP, tag="onesT")
        nc.vector.memset(onesT[:], 1.0)
        identr = cpool.tile([128, 128], FP, tag="identr")
        nc.vector.tensor_copy(_r(identr[:]), ident[:])
        identb = cpool.tile([128, 128], BF, tag="identb")
        nc.vector.tensor_copy(identb[:], ident[:])
        ones1r = cpool.tile([1, 128], FP, tag="ones1r")
        nc.vector.tensor_copy(_r(ones1r[:]), ones1[:])

        def colvec(dram_1d, n, tag):
            nt = n // 128
            t_ = cpool.tile([128, nt], FP, tag=tag)
            for j in range(nt):
                nc.sync.dma_start(t_[:, j:j + 1], dram_1d[j * 128:(j + 1) * 128][:, None])
            return t_

        b2gaT = colvec(ga_b[2], D, "b2gaT")
        b3gaT = colvec(ga_b[3], D, "b3gaT")

        gapool_c = cpool.tile([1, 1], FP, tag="gapool_c")
        nc.sync.dma_start(gapool_c[:], ga_pool[:][:, None])

        def sum_bcast(vcol, k, tag):
            p = psC.tile([128, 1], FP, tag="pd")
            lhs = onesT if k == T else ones1
            nc.tensor.matmul(p[:], lhs[:k, :], vcol[:k, :], start=True, stop=True)
            s = cpool.tile([128, 1], FP, tag=tag)
            nc.vector.tensor_copy(s[:], p[:])
            return s

        Sga = sum_bcast(gapool_c, 1, "Sga")

        qemb8 = cpool.tile([128, ET, T, BL], F8, tag="qemb8")
        goutT = cpool.tile([128, ET, BL], BF, tag="goutT")
        aT = cpool.tile([128, ET, BL], FP, tag="aT")
        bhhN_r = cpool.tile([1, D], FP, tag="bhhN_r")

        # img patch tokens, all 8 batch elems, prefetched early (bf16)
        XnA = xall.tile([128, BL, 8, D], BF, tag="XnA")
        # early-persisted tail weights (DMAs issued pre-loop, overlap GRU)
        W2b = tailw.tile([128, ET, D], BF, tag="W2b")
        W3b = tailw.tile([128, ET, D], BF, tag="W3b")
        f3b = tailw.tile([128, 4, 1024], BF, tag="f3b")

        # ================= phase A: cls -> a (ga path only) ===============
        clsTb = cpool.tile([128, ET, BL], BF, tag="clsTb")
        clsT8 = cpool.tile([128, ET, BL], F8, tag="clsT8")

        def dense_T(pool, wdt, w_nat_dram, rhsT, biasT, scaleT, otile, wtag, dmaq):
            wsb = pool.tile([128, ET, D], wdt, tag=wtag)
            for c in range(ET):
                dmaq.dma_start(wsb[:, c, :], w_nat_dram[128 * c:128 * (c + 1), :])
            for mt in range(ET):
                p = psC.tile([128, BL], FP, tag="pd")
                for kt in range(ET):
                    nc.tensor.matmul(p[:], wsb[:, kt, 128 * mt:128 * (mt + 1)],
                                     rhsT[:, kt, :], start=(kt == 0), stop=(kt == ET - 1))
                if scaleT is None:
                    nc.vector.tensor_scalar(otile[:, mt, :], p[:], biasT[:, mt:mt + 1],
                                            None, OP.add)
                else:
                    nc.vector.tensor_scalar(otile[:, mt, :], p[:], biasT[:, mt:mt + 1],
                                            scaleT[:, 0:1], OP.add, OP.mult)

        with tc.tile_pool(name="ph0", bufs=1) as ph0:
            clsn = ph0.tile([BL, D], FP, tag="clsn")
            nc.sync.dma_start(clsn[:], img[0:BL, 0, :])
            ptr = psC.tile([128, 512], FP, tag="pd")
            for kt in range(ET):
                nc.tensor.matmul(ptr[:, 8 * kt:8 * kt + 8], clsn[:, 128 * kt:128 * (kt + 1)],
                                 ident[:BL, :BL], is_transpose=True, skip_group_check=True)
            clsT = ph0.tile([128, ET, BL], FP, tag="clsT")
            nc.vector.tensor_copy(clsT[:].rearrange("p a b -> p (a b)"), ptr[:, :8 * ET])
            nc.scalar.copy(clsTb[:], clsT[:])
            nc.scalar.copy(clsT8[:], clsT[:])

        # ================= phase B: GRU (fp8 DoubleRow) ===================
        cde = ctx.enter_context(tc.tile_pool(name="cde", bufs=1))
        QtT = cde.tile([128, ET, NH * RQ], BF, tag="QtT")
        prep_cm = tc.tile_pool(name="prep", bufs=1)
        prep = prep_cm.__enter__()
        wb8_cm = tc.tile_pool(name="wb8", bufs=1)
        wb8 = wb8_cm.__enter__()
        WhhT8 = wb8.tile([128, ET, G], F8, tag="WhhT8")

        with tc.tile_pool(name="wpro", bufs=1) as wpro, \
             tc.tile_pool(name="wnat", bufs=4) as wnat:
            combr = wpro.tile([1, 2 * D], FP, tag="combr")
            nc.sync.dma_start(combr[:], b_ih[0:2 * D][None, :])
            nc.gpsimd.dma_start(combr[:], b_hh[0:2 * D][None, :], accum_op=OP.add)
            bhhN_t = wpro.tile([1, D], FP, tag="bhhN_t")
            nc.sync.dma_start(bhhN_t[:], b_hh[2 * D:3 * D][None, :])
            nc.vector.tensor_copy(_r(bhhN_r[:]), bhhN_t[:])
            bihN = wpro.tile([1, D], FP, tag="bhhN_t", name="bihN")
            nc.sync.dma_start(bihN[:], b_ih[2 * D:3 * D][None, :])


            WihT8 = prep.tile([128, ET, G], F8, tag="gow3", name="WihT8")

            def build_W8(w_dram, dst):
                jts = kchunks(G)
                for g0 in range(0, len(jts), 4):
                    grp = jts[g0:g0 + 4]
                    nats = []
                    for qi, (j0, jw) in enumerate(grp):
                        wn = wnat.tile([128, D], BF, tag="wn")
                        gd(qi % 4, wn[:jw, :], w_dram[j0:j0 + jw, :])
                        nats.append((wn, j0, jw))
                    for et in range(ET):
                        pt = psB.tile([128, 512], BF, tag="ptw")
                        for i, (wn, j0, jw) in enumerate(nats):
                            nc.tensor.matmul(pt[:, 128 * i:128 * i + jw],
                                             wn[:jw, 128 * et:128 * (et + 1)],
                                             identb[:jw, :jw], is_transpose=True,
                                             skip_group_check=True)
                        w0 = grp[0][0]
                        wlen = sum(jw for (_, _, jw) in nats)
                        if et % 2 == 0:
                            nc.vector.tensor_copy(dst[:, et, w0:w0 + wlen], pt[:, :wlen])
                        else:
                            nc.scalar.copy(dst[:, et, w0:w0 + wlen], pt[:, :wlen])

            gaw2 = prep.tile([128, ET, D], F8, tag="W08", name="gaw2")
            for c in range(ET):
                gd(1 + c % 3, gaw2[:, c, :], ga_w[2][128 * c:128 * (c + 1), :])
            gaw3 = prep.tile([128, ET, D], F8, tag="gow2", name="gaw3")
            for c in range(ET):
                gd(1 + c % 3, gaw3[:, c, :], ga_w[3][128 * c:128 * (c + 1), :])

            build_W8(w_ih, WihT8)

            A2T = wpro.tile([128, ET, BL], F8, tag="A2T")
            for mt in range(ET):
                p = psC.tile([128, BL], FP, tag="pd")
                for kt in range(ET):
                    nc.tensor.matmul(p[:], gaw2[:, kt, 128 * mt:128 * (mt + 1)],
                                     clsT8[:, kt, :], start=(kt == 0), stop=(kt == ET - 1))
                nc.vector.tensor_scalar(A2T[:, mt, :], p[:], b2gaT[:, mt:mt + 1],
                                        None, OP.add)
            for mt in range(ET):
                p = psC.tile([128, BL], FP, tag="pd")
                for kt in range(ET):
                    nc.tensor.matmul(p[:], gaw3[:, kt, 128 * mt:128 * (mt + 1)],
                                     A2T[:, kt, :], start=(kt == 0), stop=(kt == ET - 1))
                nc.vector.tensor_scalar(aT[:, mt, :], p[:], b3gaT[:, mt:mt + 1],
                                        Sga[:, 0:1], OP.add, OP.mult)
            aT8 = cpool.tile([128, ET, 128], F8, tag="aT8")
            nc.vector.memset(aT8[:].rearrange("p a b -> p (a b)"), 0.0)
            nc.vector.tensor_copy(aT8[:, :, 0:BL], aT[:])

            hnat = prep.tile([BL, D], FP, tag="W08", name="hnat")
            nc.sync.dma_start(hnat[:], h0[:, :])
            ptr0 = psC.tile([128, 512], FP, tag="pd")
            for kt in range(ET):
                nc.tensor.matmul(ptr0[:, 8 * kt:8 * kt + 8], hnat[:, 128 * kt:128 * (kt + 1)],
                                 ident[:BL, :BL], is_transpose=True, skip_group_check=True)
            hT32 = gstate.tile([128, ET, BL], FP, tag="hT32")
            nc.vector.tensor_copy(hT32[:].rearrange("p a b -> p (a b)"), ptr0[:, :8 * ET])
            h8_a = cpool.tile([128, ET, 128], F8, tag="h8_a")
            h8_b = cpool.tile([128, ET, 128], F8, tag="h8_b")
            nc.vector.memset(h8_a[:].rearrange("p a b -> p (a b)"), 0.0)
            nc.vector.memset(h8_b[:].rearrange("p a b -> p (a b)"), 0.0)
            nc.scalar.copy(h8_a[:, :, 0:BL], hT32[:])
            h8 = h8_a

            # wx (+ all biases folded): rz sections get bih+bhh, n gets bih
            wxbRZ = prep.tile([BL, 2 * D], FP, tag="wxbRZ")
            wxbN = prep.tile([BL, D], FP, tag="wxbN")
            for (j0, jw) in CH_G:
                p = psC.tile([128, 512], FP, tag="pd")
                for kp in range(3):
                    nc.tensor.matmul(p[:, :jw], aT8[:, 2 * kp:2 * kp + 2, :],
                                     WihT8[:, 2 * kp:2 * kp + 2, j0:j0 + jw],
                                     start=(kp == 0), stop=False, perf_mode=DR)
                src = combr[:, j0:j0 + jw] if j0 < 2 * D else bihN[:, j0 - 2 * D:j0 - 2 * D + jw]
                nc.tensor.matmul(p[:, :jw], ones1[:1, :], src,
                                 start=False, stop=True)
                if j0 < 2 * D:
                    nc.vector.tensor_copy(_r(wxbRZ[:, j0:j0 + jw]), p[:BL, :jw])
                else:
                    nc.vector.tensor_copy(wxbN[:, j0 - 2 * D:j0 - 2 * D + jw], p[:BL, :jw])

            build_W8(w_hh, WhhT8)


        # ---- deferred small constants (off the build critical path) ------
        b2goT = colvec(go_b[2], D, "b2goT")
        b3goT = colvec(go_b[3], D, "b3goT")
        b0laT = colvec(la_b[0], D, "b0laT")
        b2laT = colvec(la_b[2], D, "b2laT")
        b3laT = colvec(la_b[3], D, "b3laT")
        b1fT = colvec(f1_b, 1024, "b1fT")
        b2fT = colvec(f2_b, 512, "b2fT")
        b3fT = colvec(f3_b, 1024, "b3fT")
        lapool_c = cpool.tile([T, 1], FP, tag="lapool_c")
        nc.sync.dma_start(lapool_c[:], la_pool[:][:, None])
        gopool_c = cpool.tile([T, 1], FP, tag="gopool_c")
        nc.sync.dma_start(gopool_c[:], go_pool[:][:, None])
        Sla = sum_bcast(lapool_c, T, "Sla")
        Sgo = sum_bcast(gopool_c, T, "Sgo")
        pmask = cpool.tile([64, 2], FP, tag="pmask")
        nc.vector.memset(pmask[:], 0.0)
        nc.sync.dma_start(pmask[0:T, 0:1], la_pool[:][:, None])
        nc.sync.dma_start(pmask[T:2 * T, 1:2], la_pool[:][:, None])

        # ---- pre-loop early DMA emission (overlaps the GRU steps) --------
        for b in range(BL):
            nc.vector.memset(XnA[:, b, 7, :], 0.0)
            for c, (k0, kw) in enumerate(kchunks(NK)):
                gd(1 + (b * 8 + c) % 3, XnA[:kw, b, c, :], img[b, 1 + k0:1 + k0 + kw, :])
        W08 = prep.tile([128, ET, D], F8, tag="W08")
        for c in range(ET):
            gd(1 + c % 3, W08[:, c, :], la_w[0][128 * c:128 * (c + 1), :])
        gow2 = prep.tile([128, ET, D], BF, tag="gow2")
        for c in range(ET):
            gd(1 + c % 3, gow2[:, c, :], go_w[2][128 * c:128 * (c + 1), :])
        for c in range(ET):
            gd(1 + c % 3, W2b[:, c, :], la_w[2][128 * c:128 * (c + 1), :])
        for c in range(ET):
            gd(1 + c % 3, W3b[:, c, :], la_w[3][128 * c:128 * (c + 1), :])
        for c in range(4):
            gd(1 + c % 3, f3b[:, c, :], f3_w[128 * c:128 * (c + 1), :])
        # stall-prone loads (wait on in-loop readers) go last on queue 0
        gow3 = prep.tile([128, ET, D], BF, tag="gow3")
        for c in range(ET):
            gd(0, gow3[:, c, :], go_w[3][128 * c:128 * (c + 1), :])
        # W1 lands in gow2's buffer once the go stage-1 matmuls are done
        W1n = prep.tile([128, ET, D], BF, tag="gow2", name="W1n")
        for c in range(ET):
            gd(0, W1n[:, c, :], la_w[1][128 * c:128 * (c + 1), :])

        with tc.tile_pool(name="g1", bufs=1) as g1, \
             tc.tile_pool(name="psG", bufs=1, space="PSUM") as psG:

            # section psums: A = j[0:1024] (r + z1), Z = j[1024:1536] (z2),
            # N = j[1536:2304] (n); emission order A0 A1 N0 N1 Z
            STEP_CHUNKS = [("a", 0, 0, 512), ("a", 512, 512, 512),
                           ("n", 0, 1536, 512), ("n", 512, 2048, 256),
                           ("z", 0, 1024, 512)]
            KSTEPS = int(os.environ.get("KSTEPS", str(T)))
            KFILL = int(os.environ.get("KFILL", "2"))
            fill_i = 0
            for t in range(KSTEPS):
                hp8, hp32 = h8, hT32
                pA = psG.tile([128, 1024], FP, tag="a")
                pN = psG.tile([128, D], FP, tag="n")
                pZ = psG.tile([128, 512], FP, tag="z")
                tiles = {"a": pA, "n": pN, "z": pZ}
                for (sec, c0, j0, jw) in STEP_CHUNKS:
                    p = tiles[sec]
                    for kp in range(3):
                        nc.tensor.matmul(p[:, c0:c0 + jw], hp8[:, 2 * kp:2 * kp + 2, :],
                                         WhhT8[:, 2 * kp:2 * kp + 2, j0:j0 + jw],
                                         start=(kp == 0), stop=False, perf_mode=DR)
                    if j0 >= 2 * D:
                        nc.tensor.matmul(p[:, c0:c0 + jw], _r(ones1r[:1, :]),
                                         _r(bhhN_r[:, j0 - 2 * D:j0 - 2 * D + jw]),
                                         start=False, stop=True)
                    else:
                        nc.tensor.matmul(p[:, c0:c0 + jw], _r(identr[:BL, :]),
                                         _r(wxbRZ[:, j0:j0 + jw]), start=False, stop=True)
                r_sig = g1.tile([BL, D], FP, tag="rsig")
                nc.scalar.activation(r_sig[:], pA[:BL, 0:768], AF.Sigmoid)
                z_nat = g1.tile([BL, D], FP, tag="znat")
                nc.scalar.activation(z_nat[:, 0:256], pA[:BL, 768:1024], AF.Sigmoid)
                rwn = g1.tile([BL, D], FP, tag="rwn")
                nc.vector.tensor_mul(rwn[:], r_sig[:], pN[:BL, :])
                npre = rwn
                nc.vector.tensor_add(npre[:], rwn[:], wxbN[:])
                nc.scalar.activation(z_nat[:, 256:768], pZ[:BL, :], AF.Sigmoid)
                nt_ = g1.tile([BL, D], FP, tag="nt")
                nc.scalar.activation(nt_[:], npre[:], AF.Tanh)
                # fills between mm block and transposes keep PE p-state hot
                for _ in range(KFILL // 2):
                    pf = psB.tile([128, 512], FP, tag="ptw")
                    for kp in range(3):
                        nc.tensor.matmul(pf[:, 0:512], hp8[:, 2 * kp:2 * kp + 2, :],
                                         WhhT8[:, 2 * kp:2 * kp + 2, 512 * (fill_i % 4):512 * (fill_i % 4) + 512],
                                         start=(kp == 0), stop=(kp == 2), perf_mode=DR)
                    fill_i += 1
                ptt = psC.tile([128, 512], FP, tag="pd")
                for kt in range(ET):
                    nc.tensor.matmul(ptt[:, 8 * kt:8 * kt + 8], z_nat[:, 128 * kt:128 * (kt + 1)],
                                     ident[:BL, :BL], is_transpose=True, skip_group_check=True)
                for kt in range(ET):
                    nc.tensor.matmul(ptt[:, 64 + 8 * kt:64 + 8 * kt + 8],
                                     nt_[:, 128 * kt:128 * (kt + 1)],
                                     ident[:BL, :BL], is_transpose=True, skip_group_check=True)
                zT = ptt[:, 0:48]
                ntT = ptt[:, 64:112]
                if t < KSTEPS - 1:
                    for _ in range(KFILL - KFILL // 2):
                        pf = psB.tile([128, 512], FP, tag="ptw")
                        for kp in range(3):
                            nc.tensor.matmul(pf[:, 0:512], hp8[:, 2 * kp:2 * kp + 2, :],
                                             WhhT8[:, 2 * kp:2 * kp + 2, 512 * (fill_i % 4):512 * (fill_i % 4) + 512],
                                             start=(kp == 0), stop=(kp == 2), perf_mode=DR)
                        fill_i += 1
                dT = g1.tile([128, 8 * ET], FP, tag="dT")
                nc.vector.tensor_sub(dT[:], hp32[:].rearrange("p a b -> p (a b)"), ntT)
                zdT = g1.tile([128, 8 * ET], FP, tag="zdT")
                nc.vector.tensor_mul(zdT[:], zT, dT[:])
                hT32 = gstate.tile([128, ET, BL], FP, tag="hT32")
                nc.vector.tensor_add(hT32[:].rearrange("p a b -> p (a b)"), ntT, zdT[:])
                h8 = h8_b if t % 2 == 0 else h8_a
                nc.vector.tensor_add(h8[:, :, 0:BL],
                                     ntT.rearrange("p (a b) -> p a b", a=ET),
                                     zdT[:].rearrange("p (a b) -> p a b", a=ET))
                nc.scalar.copy(qemb8[:, :, t, :], hT32[:])
                if t == 17:
                    QT8 = prep.tile([128, ET, RQ], F8, tag="QT8")
                    for mt in range(ET):
                        p2 = psB.tile([128, 512], FP, tag="ptw")
                        for kp in range(3):
                            nc.tensor.matmul(p2[:, 0:128],
                                             W08[:, 2 * kp:2 * kp + 2, 128 * mt:128 * (mt + 1)],
                                             qemb8[:, 2 * kp:2 * kp + 2, 0:16, :],
                                             start=(kp == 0), stop=(kp == 2), perf_mode=DR)
                        nc.vector.tensor_scalar(QT8[:, mt, 0:128], p2[:, 0:128],
                                                b0laT[:, mt:mt + 1], None, OP.add)
                if t == 24:
                    G2Tb = prep.tile([128, ET, BL], BF, tag="G2Tb")
                    for mt in range(ET):
                        p2 = psB.tile([128, 512], FP, tag="ptw")
                        for kt in range(ET):
                            nc.tensor.matmul(p2[:, 0:BL], gow2[:, kt, 128 * mt:128 * (mt + 1)],
                                             clsTb[:, kt, :], start=(kt == 0),
                                             stop=(kt == ET - 1))
                        nc.vector.tensor_scalar(G2Tb[:, mt, :], p2[:, 0:BL],
                                                b2goT[:, mt:mt + 1], None, OP.add)
                if t == 26:
                    for mt in range(ET):
                        p2 = psB.tile([128, 512], FP, tag="ptw")
                        for kt in range(ET):
                            nc.tensor.matmul(p2[:, 0:BL], gow3[:, kt, 128 * mt:128 * (mt + 1)],
                                             G2Tb[:, kt, :], start=(kt == 0),
                                             stop=(kt == ET - 1))
                        nc.vector.tensor_scalar(goutT[:, mt, :], p2[:, 0:BL],
                                                b3goT[:, mt:mt + 1], Sgo[:, 0:1],
                                                OP.add, OP.mult)

        wb8_cm.__exit__(None, None, None)

        # ================= phase C: Q^T, W1^T, Qt^T =======================
        if PHASES >= 2:
          if True:
              for mt in range(ET):
                  p = psC.tile([128, RQ], FP, tag="pd")
                  for kp in range(3):
                      nc.tensor.matmul(p[:, 0:128],
                                       W08[:, 2 * kp:2 * kp + 2, 128 * mt:128 * (mt + 1)],
                                       qemb8[:, 2 * kp:2 * kp + 2, 16:32, :],
                                       start=(kp == 0), stop=(kp == 2), perf_mode=DR)
                  nc.vector.tensor_scalar(QT8[:, mt, 128:256], p[:, 0:128],
                                          b0laT[:, mt:mt + 1], None, OP.add)
              W1T8 = prep.tile([128, ET, D], F8, tag="W08", name="W1T8")
              for hd in range(ET):
                  for grp in range(2):
                      pt2 = psB.tile([128, 512], BF, tag="ptw")
                      for i in range(3):
                          e2 = grp * 3 + i
                          nc.tensor.matmul(pt2[:, 128 * i:128 * (i + 1)],
                                           W1n[:, e2, 128 * hd:128 * (hd + 1)],
                                           identb[:], is_transpose=True, skip_group_check=True)
                      if grp == 0:
                          nc.vector.tensor_copy(W1T8[:, hd, 0:384], pt2[:, 0:384])
                      else:
                          nc.scalar.copy(W1T8[:, hd, 384:768], pt2[:, 0:384])
              scl = 1.0 / float(np.sqrt(DK))
              for h in range(NH):
                  for mt in range(ET):
                      p = psC.tile([128, RQ], FP, tag="pd")
                      nc.tensor.matmul(p[:], W1T8[:, 3 * h:3 * h + 2, 128 * mt:128 * (mt + 1)],
                                       QT8[:, 3 * h:3 * h + 2, :],
                                       start=True, stop=False, perf_mode=DR)
                      nc.tensor.matmul(p[:], W1T8[:, 3 * h + 2, 128 * mt:128 * (mt + 1)],
                                       QT8[:, 3 * h + 2, :], start=False, stop=True)
                      dst = QtT[:, mt, :].rearrange("p (b h2 t) -> p b h2 t",
                                                    h2=NH, t=T)[:, :, h, :]
                      nc.scalar.activation(dst.rearrange("p b t -> p t b"),
                                           p[:].rearrange("p (t b) -> p t b", b=BL),
                                           AF.Copy, scale=scl)
        prep_cm.__exit__(None, None, None)

        # ================= phase D: per-b attention =======================
        de = ctx.enter_context(tc.tile_pool(name="de", bufs=1))
        pcxall = de.tile([2, BL * D], BF, tag="pcxall")
        f1 = de.tile([128, 12, 1024], BF, tag="f1")
        for c in range(12):
            gd(1 + c % 3, f1[:, c, :], f1_w[128 * c:128 * (c + 1), :])

        if PHASES >= 3:
            with tc.tile_pool(name="xb", bufs=2) as xb, \
                 tc.tile_pool(name="ab", bufs=1) as ab, \
                 tc.tile_pool(name="psD", bufs=1, space="PSUM") as psA:
              KC = kchunks(NK)
              for b in range(BL):
                  Xn = XnA[:, b, :, :]
                  XT = xb.tile([128, ET, NK], BF, tag="XT")
                  cpeng = [nc.vector.tensor_copy, nc.scalar.copy, nc.gpsimd.tensor_copy]
                  for et in range(ET):
                      for g in range(2):
                          pt = psB.tile([128, 512], BF, tag="ptw")
                          for i in range(4):
                              c = g * 4 + i
                              nc.tensor.matmul(pt[:, 128 * i:128 * (i + 1)],
                                               Xn[:, c, 128 * et:128 * (et + 1)],
                                               identb[:], is_transpose=True,
                                               skip_group_check=True)
                          w = 512 if g == 0 else NK - 512
                          cpeng[(et * 2 + g) % 2](XT[:, et, 512 * g:512 * g + w], pt[:, :w])
                  att = ab.tile([64, NK], BF, tag="att")
                  zacc = ab.tile([64, 2], FP, tag="zacc")
                  for ci, (n0, nw) in enumerate(CH_NK):
                      p = psA.tile([64, 512], FP, tag=f"wh{ci}")
                      for kt in range(ET):
                          nc.tensor.matmul(p[:, :nw],
                                           QtT[:, kt, b * 2 * T:(b + 1) * 2 * T],
                                           XT[:, kt, n0:n0 + nw],
                                           start=(kt == 0), stop=(kt == ET - 1))
                      nc.scalar.activation(att[:, n0:n0 + nw], p[:, :nw], AF.Exp,
                                           accum_out=zacc[:, ci:ci + 1])
                  zs = ab.tile([64, 1], FP, tag="zs")
                  nc.vector.tensor_add(zs[:], zacc[:, 0:1], zacc[:, 1:2])
                  rz = ab.tile([64, 1], FP, tag="rz1")
                  nc.vector.reciprocal(rz[:], zs[:])
                  wm = ab.tile([64, 2], BF, tag="wm")
                  nc.vector.tensor_scalar(wm[:], pmask[:], rz[:, 0:1], None, OP.mult)
                  pa_sb = ab.tile([2, NK], BF, tag="pa_sb")
                  for ci, (n0, nw) in enumerate(CH_NK):
                      p = psA.tile([2, 512], FP, tag=f"wh{2 + ci}")
                      nc.tensor.matmul(p[:, :nw], wm[:], att[:, n0:n0 + nw],
                                       start=True, stop=True)
                      nc.vector.tensor_copy(pa_sb[:, n0:n0 + nw], p[:, :nw])
                  paT = ab.tile([128, len(KC), 2], BF, tag="paT")
                  nc.vector.memset(paT[:].rearrange("p a b -> p (a b)"), 0.0)
                  ptp = psC.tile([128, 512], BF, tag="pd")
                  for c, (k0, kw) in enumerate(KC):
                      nc.tensor.matmul(ptp[:kw, 2 * c:2 * c + 2], pa_sb[:, k0:k0 + kw],
                                       identb[:2, :2], is_transpose=True, skip_group_check=True)
                      nc.vector.tensor_copy(paT[:kw, c, :], ptp[:kw, 2 * c:2 * c + 2])
                  for ci, (n0, nw) in enumerate(CH_D):
                      p = psA.tile([2, 512], FP, tag=f"wh{4 - ci}")
                      for c in range(len(KC)):
                          nc.tensor.matmul(p[:, :nw], paT[:, c, :],
                                           Xn[:, c, n0:n0 + nw],
                                           start=(c == 0), stop=(c == len(KC) - 1))
                      nc.vector.tensor_copy(pcxall[:, b * D + n0:b * D + n0 + nw], p[:, :nw])

        # ================= phase E: projections + MLP =====================
        if PHASES >= 4:
            with tc.tile_pool(name="tail", bufs=1) as tail:
              f2 = tail.tile([128, 8, 512], BF, tag="f2")
              for c in range(8):
                  gd(1 + c % 3, f2[:, c, :], f2_w[128 * c:128 * (c + 1), :])
              f3 = f3b
              W3 = W3b
              b2laTb = tail.tile([128, ET], BF, tag="b2laTb")
              nc.vector.tensor_copy(b2laTb[:], b2laT[:])
              vconT = tail.tile([128, ET], FP, tag="vconT")
              for mt in range(ET):
                  p = psC.tile([128, 1], FP, tag="pd")
                  for kt in range(ET):
                      nc.tensor.matmul(p[:], W3[:, kt, 128 * mt:128 * (mt + 1)],
                                       b2laTb[:, kt:kt + 1], start=(kt == 0), stop=(kt == ET - 1))
                  nc.vector.tensor_scalar(vconT[:, mt:mt + 1], p[:], b3laT[:, mt:mt + 1],
                                          Sla[:, 0:1], OP.add, OP.mult)
              pcxT = tail.tile([128, ET, 2 * BL], BF, tag="pcxT")
              ptc = psC.tile([128, 512], BF, tag="pd")
              for b2 in range(BL):
                  for kt in range(ET):
                      nc.tensor.matmul(ptc[:, 2 * (b2 * ET + kt):2 * (b2 * ET + kt) + 2],
                                       pcxall[:, b2 * D + 128 * kt:b2 * D + 128 * (kt + 1)],
                                       identb[:2, :2], is_transpose=True, skip_group_check=True)
              src_v = ptc[:, :96].rearrange("p (b a h) -> p a b h", b=BL, a=ET)
              dst_v = pcxT[:].rearrange("p a (b h) -> p a b h", h=NH)
              nc.vector.tensor_copy(dst_v, src_v)
              W2 = W2b
              pctxT = tail.tile([128, ET, BL], BF, tag="pctxT")
              pcv = pcxT[:].rearrange("p a (b h) -> p a b h", h=NH)
              for h in range(NH):
                  for mi in range(3):
                      mt = h * 3 + mi
                      p = psC.tile([128, BL], FP, tag="pd")
                      for kt in range(ET):
                          nc.tensor.matmul(p[:], W2[:, kt, 128 * mt:128 * (mt + 1)],
                                           pcv[:, kt, :, h], start=(kt == 0), stop=(kt == ET - 1))
                      nc.vector.tensor_copy(pctxT[:, mt, :], p[:])
              loT = tail.tile([128, ET, BL], BF, tag="loT")
              for mt in range(ET):
                  p = psC.tile([128, BL], FP, tag="pd")
                  for kt in range(ET):
                      nc.tensor.matmul(p[:], W3[:, kt, 128 * mt:128 * (mt + 1)],
                                       pctxT[:, kt, :], start=(kt == 0), stop=(kt == ET - 1))
                  nc.vector.tensor_scalar(loT[:, mt, :], p[:], vconT[:, mt:mt + 1], None, OP.add)

              y1T = tail.tile([128, 8, BL], BF, tag="y1T")
              for mt in range(8):
                  p = psC.tile([128, BL], FP, tag="pd")
                  for kt in range(12):
                      r_ = loT[:, kt, :] if kt < ET else goutT[:, kt - ET, :]
                      nc.tensor.matmul(p[:], f1[:, kt, 128 * mt:128 * (mt + 1)], r_,
                                       start=(kt == 0), stop=(kt == 11))
                  nc.vector.tensor_scalar(y1T[:, mt, :], p[:], b1fT[:, mt:mt + 1], None, OP.add)
              y2T = tail.tile([128, 4, BL], BF, tag="y2T")
              for mt in range(4):
                  p = psC.tile([128, BL], FP, tag="pd")
                  for kt in range(8):
                      nc.tensor.matmul(p[:], f2[:, kt, 128 * mt:128 * (mt + 1)],
                                       y1T[:, kt, :], start=(kt == 0), stop=(kt == 7))
                  nc.scalar.activation(y2T[:, mt, :], p[:], AF.Relu, bias=b2fT[:, mt:mt + 1])
              yT = tail.tile([128, 8, BL], FP, tag="yT")
              for mt in range(8):
                  p = psC.tile([128, BL], FP, tag="pd")
                  for kt in range(4):
                      nc.tensor.matmul(p[:], f3[:, kt, 128 * mt:128 * (mt + 1)],
                                       y2T[:, kt, :], start=(kt == 0), stop=(kt == 3))
                  nc.vector.tensor_scalar(yT[:, mt, :], p[:], b3fT[:, mt:mt + 1], None, OP.add)
              ynat = tail.tile([BL, 1024], FP, tag="ynat")
              for g in range(2):
                  po = psB.tile([128, 512], FP, tag="ptw")
                  for i in range(4):
                      mt = g * 4 + i
                      nc.tensor.matmul(po[:BL, 128 * i:128 * (i + 1)], yT[:, mt, :],
                                       ident[:128, :128], is_transpose=True,
                                       skip_group_check=True)
                  nc.vector.tensor_copy(ynat[:, 512 * g:512 * (g + 1)], po[:BL, :])
              nc.sync.dma_start(out_d[:, :], ynat[:])

    nc.compile()
    return nc


_NC = None


def kernel(**inputs):
    global _NC
    if _NC is None:
        _NC = build()
    B = inputs["image_local_embeds"].shape[0]
    per = B // NCORES
    in_maps = []
    for c in range(NCORES):
        sl = slice(c * per, (c + 1) * per)
        m = {
            "img": np.ascontiguousarray(np.asarray(inputs["image_local_embeds"])[sl], dtype=np.float32),
            "h0": np.ascontiguousarray(np.asarray(inputs["h0"])[sl], dtype=np.float32),
        }
        for k in ["gru_w_ih", "gru_w_hh", "gru_b_ih", "gru_b_hh", "ga_w", "ga_b",
                  "ga_pool", "la_w", "la_b", "la_pool", "go_w", "go_b", "go_pool",
                  "f1_w", "f1_b", "f2_w", "f2_b", "f3_w", "f3_b"]:
            m[k] = np.ascontiguousarray(np.asarray(inputs[k], dtype=np.float32))
        in_maps.append(m)
    res = run_bass_kernel_spmd(_NC, in_maps, core_ids=list(range(NCORES)))
    return np.concatenate([res.results[c]["out"] for c in range(NCORES)], axis=0)



# revision 88
# speedup vs baseline: 1.1078x; 1.0008x over previous
"""Trainium2 Bass kernel for nn_BiVision_VQA2 (B=64,T=32,D=768,N=901).

Data-parallel over batch: 8 batch elems per core x 8 cores.
Key math simplifications (validated vs reference in numpy, rel err ~1e-6):
  - ga/go attention use a single key token -> softmax==1 -> those paths are
    linear in cls; question_embeds is mathematically unused.
  - GRU input `a` is constant over time; wx computed once.
  - local attention: scores = (qemb @ W0_h) @ W1_h^T / sqrt(dk) @ X^T ;
    row-constant score terms (K bias, Q.b1) drop out of softmax; query
    pooling applied to the attention matrix before the @X contraction;
    constant bias terms folded into one vector.
"""

import numpy as np
from contextlib import ExitStack

import concourse.bass as bass
import concourse.tile as tile
from concourse import bacc, mybir
from concourse.bass_utils import run_bass_kernel_spmd
from concourse.masks import make_identity

FP = mybir.dt.float32
FPR = mybir.dt.float32r
OP = mybir.AluOpType
AF = mybir.ActivationFunctionType
BF = mybir.dt.bfloat16
F8 = mybir.dt.float8e4
DR = mybir.MatmulPerfMode.DoubleRow

NCORES = 8
BL = 8
D = 768
T = 32
G = 3 * D
NK = 900
NH = 2
DK = 384
ET = D // 128
RQ = BL * T
USE_FPR = True


def chunks(total):
    out, o = [], 0
    while o < total:
        w = min(512, total - o)
        out.append((o, w))
        o += w
    return out


CH_G = chunks(G)
CH_NK = [(0, 512), (512, 388)]
CH_D = [(0, 512), (512, 256)]


def _r(ap):
    return ap.bitcast(FPR) if USE_FPR else ap


def kchunks(n):
    out, o = [], 0
    while o < n:
        out.append((o, min(128, n - o)))
        o += 128
    return out


import os
PHASES = int(os.environ.get("KPHASES", "4"))


def build():
    nc = bacc.Bacc("TRN2", target_bir_lowering=False, debug=False,
                   enable_asserts=False, num_swdge_queues=4)

    def gd(q, out, in_, **kw):
        inst = nc.gpsimd.dma_start(out, in_, **kw)
        if q:
            inst.ins.queue = f"qPoolDynamic{q}"
        return inst

    img = nc.dram_tensor("img", [BL, 901, D], FP, kind="ExternalInput").ap()
    h0 = nc.dram_tensor("h0", [BL, D], FP, kind="ExternalInput").ap()
    w_ih = nc.dram_tensor("gru_w_ih", [G, D], FP, kind="ExternalInput").ap()
    w_hh = nc.dram_tensor("gru_w_hh", [G, D], FP, kind="ExternalInput").ap()
    b_ih = nc.dram_tensor("gru_b_ih", [G], FP, kind="ExternalInput").ap()
    b_hh = nc.dram_tensor("gru_b_hh", [G], FP, kind="ExternalInput").ap()
    ga_w = nc.dram_tensor("ga_w", [4, D, D], FP, kind="ExternalInput").ap()
    ga_b = nc.dram_tensor("ga_b", [4, D], FP, kind="ExternalInput").ap()
    ga_pool = nc.dram_tensor("ga_pool", [1], FP, kind="ExternalInput").ap()
    la_w = nc.dram_tensor("la_w", [4, D, D], FP, kind="ExternalInput").ap()
    la_b = nc.dram_tensor("la_b", [4, D], FP, kind="ExternalInput").ap()
    la_pool = nc.dram_tensor("la_pool", [T], FP, kind="ExternalInput").ap()
    go_w = nc.dram_tensor("go_w", [4, D, D], FP, kind="ExternalInput").ap()
    go_b = nc.dram_tensor("go_b", [4, D], FP, kind="ExternalInput").ap()
    go_pool = nc.dram_tensor("go_pool", [T], FP, kind="ExternalInput").ap()
    f1_w = nc.dram_tensor("f1_w", [2 * D, 1024], FP, kind="ExternalInput").ap()
    f1_b = nc.dram_tensor("f1_b", [1024], FP, kind="ExternalInput").ap()
    f2_w = nc.dram_tensor("f2_w", [1024, 512], FP, kind="ExternalInput").ap()
    f2_b = nc.dram_tensor("f2_b", [512], FP, kind="ExternalInput").ap()
    f3_w = nc.dram_tensor("f3_w", [512, 1024], FP, kind="ExternalInput").ap()
    f3_b = nc.dram_tensor("f3_b", [1024], FP, kind="ExternalInput").ap()
    out_d = nc.dram_tensor("out", [BL, 1024], FP, kind="ExternalOutput").ap()

    with tile.TileContext(nc) as tc, ExitStack() as ctx:
        cpool = ctx.enter_context(tc.tile_pool(name="const", bufs=1))
        gstate = ctx.enter_context(tc.tile_pool(name="gstate", bufs=2))
        xall = ctx.enter_context(tc.tile_pool(name="xall", bufs=1))
        tailw = ctx.enter_context(tc.tile_pool(name="tailw", bufs=1))
        psB = ctx.enter_context(tc.tile_pool(name="psB", bufs=2, space="PSUM"))
        psC = ctx.enter_context(tc.tile_pool(name="psC", bufs=1, space="PSUM"))

        ident = cpool.tile([128, 128], FP, tag="ident")
        make_identity(nc, ident[:])
        ones1 = cpool.tile([1, 128], FP, tag="ones1")
        nc.vector.memset(ones1[:], 1.0)
        onesT = cpool.tile([T, 128], FP, tag="onesT")
        nc.vector.memset(onesT[:], 1.0)
        identr = cpool.tile([128, 128], FP, tag="identr")
        nc.vector.tensor_copy(_r(identr[:]), ident[:])
        identb = cpool.tile([128, 128], BF, tag="identb")
        nc.vector.tensor_copy(identb[:], ident[:])
        ones1r = cpool.tile([1, 128], FP, tag="ones1r")
        nc.vector.tensor_copy(_r(ones1r[:]), ones1[:])

        def colvec(dram_1d, n, tag):
            nt = n // 128
            t_ = cpool.tile([128, nt], FP, tag=tag)
            for j in range(nt):
                nc.sync.dma_start(t_[:, j:j + 1], dram_1d[j * 128:(j + 1) * 128][:, None])
            return t_

        b2gaT = colvec(ga_b[2], D, "b2gaT")
        b3gaT = colvec(ga_b[3], D, "b3gaT")

        gapool_c = cpool.tile([1, 1], FP, tag="gapool_c")
        nc.sync.dma_start(gapool_c[:], ga_pool[:][:, None])

        def sum_bcast(vcol, k, tag):
            p = psC.tile([128, 1], FP, tag="pd")
            lhs = onesT if k == T else ones1
            nc.tensor.matmul(p[:], lhs[:k, :], vcol[:k, :], start=True, stop=True)
            s = cpool.tile([128, 1], FP, tag=tag)
            nc.vector.tensor_copy(s[:], p[:])
            return s

        Sga = sum_bcast(gapool_c, 1, "Sga")

        qemb8 = cpool.tile([128, ET, T, BL], F8, tag="qemb8")
        goutT = cpool.tile([128, ET, BL], BF, tag="goutT")
        aT = cpool.tile([128, ET, BL], FP, tag="aT")
        bhhN_r = cpool.tile([1, D], FP, tag="bhhN_r")

        # img patch tokens, all 8 batch elems, prefetched early (bf16)
        XnA = xall.tile([128, BL, 8, D], BF, tag="XnA")
        # early-persisted tail weights (DMAs issued pre-loop, overlap GRU)
        W2b = tailw.tile([128, ET, D], BF, tag="W2b")
        W3b = tailw.tile([128, ET, D], BF, tag="W3b")
        f3b = tailw.tile([128, 4, 1024], BF, tag="f3b")

        # ================= phase A: cls -> a (ga path only) ===============
        clsTb = cpool.tile([128, ET, BL], BF, tag="clsTb")
        clsT8 = cpool.tile([128, ET, BL], F8, tag="clsT8")

        def dense_T(pool, wdt, w_nat_dram, rhsT, biasT, scaleT, otile, wtag, dmaq):
            wsb = pool.tile([128, ET, D], wdt, tag=wtag)
            for c in range(ET):
                dmaq.dma_start(wsb[:, c, :], w_nat_dram[128 * c:128 * (c + 1), :])
            for mt in range(ET):
                p = psC.tile([128, BL], FP, tag="pd")
                for kt in range(ET):
                    nc.tensor.matmul(p[:], wsb[:, kt, 128 * mt:128 * (mt + 1)],
                                     rhsT[:, kt, :], start=(kt == 0), stop=(kt == ET - 1))
                if scaleT is None:
                    nc.vector.tensor_scalar(otile[:, mt, :], p[:], biasT[:, mt:mt + 1],
                                            None, OP.add)
                else:
                    nc.vector.tensor_scalar(otile[:, mt, :], p[:], biasT[:, mt:mt + 1],
                                            scaleT[:, 0:1], OP.add, OP.mult)

        with tc.tile_pool(name="ph0", bufs=1) as ph0:
            clsn = ph0.tile([BL, D], FP, tag="clsn")
            nc.sync.dma_start(clsn[:], img[0:BL, 0, :])
            ptr = psC.tile([128, 512], FP, tag="pd")
            for kt in range(ET):
                nc.tensor.matmul(ptr[:, 8 * kt:8 * kt + 8], clsn[:, 128 * kt:128 * (kt + 1)],
                                 ident[:BL, :BL], is_transpose=True, skip_group_check=True)
            clsT = ph0.tile([128, ET, BL], FP, tag="clsT")
            nc.vector.tensor_copy(clsT[:].rearrange("p a b -> p (a b)"), ptr[:, :8 * ET])
            nc.scalar.copy(clsTb[:], clsT[:])
            nc.scalar.copy(clsT8[:], clsT[:])

        # ================= phase B: GRU (fp8 DoubleRow) ===================
        cde = ctx.enter_context(tc.tile_pool(name="cde", bufs=1))
        QtT = cde.tile([128, ET, NH * RQ], BF, tag="QtT")
        prep_cm = tc.tile_pool(name="prep", bufs=1)
        prep = prep_cm.__enter__()
        wb8_cm = tc.tile_pool(name="wb8", bufs=1)
        wb8 = wb8_cm.__enter__()
        WhhT8 = wb8.tile([128, ET, G], F8, tag="WhhT8")

        with tc.tile_pool(name="wpro", bufs=1) as wpro, \
             tc.tile_pool(name="wnat", bufs=4) as wnat:
            combr = wpro.tile([1, 2 * D], FP, tag="combr")
            nc.sync.dma_start(combr[:], b_ih[0:2 * D][None, :])
            nc.gpsimd.dma_start(combr[:], b_hh[0:2 * D][None, :], accum_op=OP.add)
            bhhN_t = wpro.tile([1, D], FP, tag="bhhN_t")
            nc.sync.dma_start(bhhN_t[:], b_hh[2 * D:3 * D][None, :])
            nc.vector.tensor_copy(_r(bhhN_r[:]), bhhN_t[:])
            bihN = wpro.tile([1, D], FP, tag="bhhN_t", name="bihN")
            nc.sync.dma_start(bihN[:], b_ih[2 * D:3 * D][None, :])


            WihT8 = prep.tile([128, ET, G], F8, tag="gow3", name="WihT8")

            def build_W8(w_dram, dst):
                jts = kchunks(G)
                for g0 in range(0, len(jts), 4):
                    grp = jts[g0:g0 + 4]
                    nats = []
                    for qi, (j0, jw) in enumerate(grp):
                        wn = wnat.tile([128, D], BF, tag="wn")
                        gd(qi % 4, wn[:jw, :], w_dram[j0:j0 + jw, :])
                        nats.append((wn, j0, jw))
                    for et in range(ET):
                        pt = psB.tile([128, 512], BF, tag="ptw")
                        for i, (wn, j0, jw) in enumerate(nats):
                            nc.tensor.matmul(pt[:, 128 * i:128 * i + jw],
                                             wn[:jw, 128 * et:128 * (et + 1)],
                                             identb[:jw, :jw], is_transpose=True,
                                             skip_group_check=True)
                        w0 = grp[0][0]
                        wlen = sum(jw for (_, _, jw) in nats)
                        if et % 2 == 0:
                            nc.vector.tensor_copy(dst[:, et, w0:w0 + wlen], pt[:, :wlen])
                        else:
                            nc.scalar.copy(dst[:, et, w0:w0 + wlen], pt[:, :wlen])

            gaw2 = prep.tile([128, ET, D], F8, tag="W08", name="gaw2")
            for c in range(ET):
                gd(1 + c % 3, gaw2[:, c, :], ga_w[2][128 * c:128 * (c + 1), :])
            gaw3 = prep.tile([128, ET, D], F8, tag="gow2", name="gaw3")
            for c in range(ET):
                gd(1 + c % 3, gaw3[:, c, :], ga_w[3][128 * c:128 * (c + 1), :])

            build_W8(w_ih, WihT8)

            A2T = wpro.tile([128, ET, BL], F8, tag="A2T")
            for mt in range(ET):
                p = psC.tile([128, BL], FP, tag="pd")
                for kt in range(ET):
                    nc.tensor.matmul(p[:], gaw2[:, kt, 128 * mt:128 * (mt + 1)],
                                     clsT8[:, kt, :], start=(kt == 0), stop=(kt == ET - 1))
                nc.vector.tensor_scalar(A2T[:, mt, :], p[:], b2gaT[:, mt:mt + 1],
                                        None, OP.add)
            for mt in range(ET):
                p = psC.tile([128, BL], FP, tag="pd")
                for kt in range(ET):
                    nc.tensor.matmul(p[:], gaw3[:, kt, 128 * mt:128 * (mt + 1)],
                                     A2T[:, kt, :], start=(kt == 0), stop=(kt == ET - 1))
                nc.vector.tensor_scalar(aT[:, mt, :], p[:], b3gaT[:, mt:mt + 1],
                                        Sga[:, 0:1], OP.add, OP.mult)
            aT8 = cpool.tile([128, ET, 128], F8, tag="aT8")
            nc.vector.memset(aT8[:].rearrange("p a b -> p (a b)"), 0.0)
            nc.vector.tensor_copy(aT8[:, :, 0:BL], aT[:])

            hnat = prep.tile([BL, D], FP, tag="W08", name="hnat")
            nc.sync.dma_start(hnat[:], h0[:, :])
            ptr0 = psC.tile([128, 512], FP, tag="pd")
            for kt in range(ET):
                nc.tensor.matmul(ptr0[:, 8 * kt:8 * kt + 8], hnat[:, 128 * kt:128 * (kt + 1)],
                                 ident[:BL, :BL], is_transpose=True, skip_group_check=True)
            hT32 = gstate.tile([128, ET, BL], FP, tag="hT32")
            nc.vector.tensor_copy(hT32[:].rearrange("p a b -> p (a b)"), ptr0[:, :8 * ET])
            h8_a = cpool.tile([128, ET, 128], F8, tag="h8_a")
            h8_b = cpool.tile([128, ET, 128], F8, tag="h8_b")
            nc.vector.memset(h8_a[:].rearrange("p a b -> p (a b)"), 0.0)
            nc.vector.memset(h8_b[:].rearrange("p a b -> p (a b)"), 0.0)
            nc.scalar.copy(h8_a[:, :, 0:BL], hT32[:])
            h8 = h8_a

            # wx (+ all biases folded): rz sections get bih+bhh, n gets bih
            wxbRZ = prep.tile([BL, 2 * D], FP, tag="wxbRZ")
            wxbN = prep.tile([BL, D], FP, tag="wxbN")
            for (j0, jw) in CH_G:
                p = psC.tile([128, 512], FP, tag="pd")
                for kp in range(3):
                    nc.tensor.matmul(p[:, :jw], aT8[:, 2 * kp:2 * kp + 2, :],
                                     WihT8[:, 2 * kp:2 * kp + 2, j0:j0 + jw],
                                     start=(kp == 0), stop=False, perf_mode=DR)
                src = combr[:, j0:j0 + jw] if j0 < 2 * D else bihN[:, j0 - 2 * D:j0 - 2 * D + jw]
                nc.tensor.matmul(p[:, :jw], ones1[:1, :], src,
                                 start=False, stop=True)
                if j0 < 2 * D:
                    nc.vector.tensor_copy(_r(wxbRZ[:, j0:j0 + jw]), p[:BL, :jw])
                else:
                    nc.vector.tensor_copy(wxbN[:, j0 - 2 * D:j0 - 2 * D + jw], p[:BL, :jw])

            build_W8(w_hh, WhhT8)


        # ---- deferred small constants (off the build critical path) ------
        b2goT = colvec(go_b[2], D, "b2goT")
        b3goT = colvec(go_b[3], D, "b3goT")
        b0laT = colvec(la_b[0], D, "b0laT")
        b2laT = colvec(la_b[2], D, "b2laT")
        b3laT = colvec(la_b[3], D, "b3laT")
        b1fT = colvec(f1_b, 1024, "b1fT")
        b2fT = colvec(f2_b, 512, "b2fT")
        b3fT = colvec(f3_b, 1024, "b3fT")
        lapool_c = cpool.tile([T, 1], FP, tag="lapool_c")
        nc.sync.dma_start(lapool_c[:], la_pool[:][:, None])
        gopool_c = cpool.tile([T, 1], FP, tag="gopool_c")
        nc.sync.dma_start(gopool_c[:], go_pool[:][:, None])
        Sla = sum_bcast(lapool_c, T, "Sla")
        Sgo = sum_bcast(gopool_c, T, "Sgo")
        pmask = cpool.tile([64, 2], FP, tag="pmask")
        nc.vector.memset(pmask[:], 0.0)
        nc.sync.dma_start(pmask[0:T, 0:1], la_pool[:][:, None])
        nc.sync.dma_start(pmask[T:2 * T, 1:2], la_pool[:][:, None])

        # ---- pre-loop early DMA emission (overlaps the GRU steps) --------
        for b in range(BL):
            nc.vector.memset(XnA[:, b, 7, :], 0.0)
            for c, (k0, kw) in enumerate(kchunks(NK)):
                gd(1 + (b * 8 + c) % 3, XnA[:kw, b, c, :], img[b, 1 + k0:1 + k0 + kw, :])
        W08 = prep.tile([128, ET, D], F8, tag="W08")
        for c in range(ET):
            gd(1 + c % 3, W08[:, c, :], la_w[0][128 * c:128 * (c + 1), :])
        gow2 = prep.tile([128, ET, D], BF, tag="gow2")
        for c in range(ET):
            gd(1 + c % 3, gow2[:, c, :], go_w[2][128 * c:128 * (c + 1), :])
        for c in range(ET):
            gd(1 + c % 3, W2b[:, c, :], la_w[2][128 * c:128 * (c + 1), :])
        for c in range(ET):
            gd(1 + c % 3, W3b[:, c, :], la_w[3][128 * c:128 * (c + 1), :])
        for c in range(4):
            gd(1 + c % 3, f3b[:, c, :], f3_w[128 * c:128 * (c + 1), :])
        # stall-prone loads (wait on in-loop readers) go last on queue 0
        gow3 = prep.tile([128, ET, D], BF, tag="gow3")
        for c in range(ET):
            gd(0, gow3[:, c, :], go_w[3][128 * c:128 * (c + 1), :])
        # W1 lands in gow2's buffer once the go stage-1 matmuls are done
        W1n = prep.tile([128, ET, D], BF, tag="gow2", name="W1n")
        for c in range(ET):
            gd(0, W1n[:, c, :], la_w[1][128 * c:128 * (c + 1), :])

        with tc.tile_pool(name="g1", bufs=1) as g1, \
             tc.tile_pool(name="psG", bufs=1, space="PSUM") as psG:

            # section psums: A = j[0:1024] (r + z1), Z = j[1024:1536] (z2),
            # N = j[1536:2304] (n); emission order A0 A1 N0 N1 Z
            STEP_CHUNKS = [("a", 0, 0, 512), ("a", 512, 512, 512),
                           ("n", 0, 1536, 512), ("n", 512, 2048, 256),
                           ("z", 0, 1024, 512)]
            KSTEPS = int(os.environ.get("KSTEPS", str(T)))
            KFILL = int(os.environ.get("KFILL", "2"))
            fill_i = 0
            for t in range(KSTEPS):
                hp8, hp32 = h8, hT32
                pA = psG.tile([128, 1024], FP, tag="a")
                pN = psG.tile([128, D], FP, tag="n")
                pZ = psG.tile([128, 512], FP, tag="z")
                tiles = {"a": pA, "n": pN, "z": pZ}
                for (sec, c0, j0, jw) in STEP_CHUNKS:
                    p = tiles[sec]
                    for kp in range(3):
                        nc.tensor.matmul(p[:, c0:c0 + jw], hp8[:, 2 * kp:2 * kp + 2, :],
                                         WhhT8[:, 2 * kp:2 * kp + 2, j0:j0 + jw],
                                         start=(kp == 0), stop=False, perf_mode=DR)
                    if j0 >= 2 * D:
                        nc.tensor.matmul(p[:, c0:c0 + jw], _r(ones1r[:1, :]),
                                         _r(bhhN_r[:, j0 - 2 * D:j0 - 2 * D + jw]),
                                         start=False, stop=True)
                    else:
                        nc.tensor.matmul(p[:, c0:c0 + jw], _r(identr[:BL, :]),
                                         _r(wxbRZ[:, j0:j0 + jw]), start=False, stop=True)
                r_sig = g1.tile([BL, D], FP, tag="rsig")
                nc.scalar.activation(r_sig[:], pA[:BL, 0:768], AF.Sigmoid)
                z_nat = g1.tile([BL, D], FP, tag="znat")
                nc.scalar.activation(z_nat[:, 0:256], pA[:BL, 768:1024], AF.Sigmoid)
                rwn = g1.tile([BL, D], FP, tag="rwn")
                nc.vector.tensor_mul(rwn[:], r_sig[:], pN[:BL, :])
                npre = rwn
                nc.vector.tensor_add(npre[:], rwn[:], wxbN[:])
                nc.scalar.activation(z_nat[:, 256:768], pZ[:BL, :], AF.Sigmoid)
                nt_ = g1.tile([BL, D], FP, tag="nt")
                nc.scalar.activation(nt_[:], npre[:], AF.Tanh)
                # fills between mm block and transposes keep PE p-state hot
                for _ in range(KFILL // 2):
                    pf = psB.tile([128, 512], FP, tag="ptw")
                    for kp in range(3):
                        nc.tensor.matmul(pf[:, 0:512], hp8[:, 2 * kp:2 * kp + 2, :],
                                         WhhT8[:, 2 * kp:2 * kp + 2, 512 * (fill_i % 4):512 * (fill_i % 4) + 512],
                                         start=(kp == 0), stop=(kp == 2), perf_mode=DR)
                    fill_i += 1
                ptt = psC.tile([128, 512], FP, tag="pd")
                for kt in range(ET):
                    nc.tensor.matmul(ptt[:, 8 * kt:8 * kt + 8], z_nat[:, 128 * kt:128 * (kt + 1)],
                                     ident[:BL, :BL], is_transpose=True, skip_group_check=True)
                for kt in range(ET):
                    nc.tensor.matmul(ptt[:, 64 + 8 * kt:64 + 8 * kt + 8],
                                     nt_[:, 128 * kt:128 * (kt + 1)],
                                     ident[:BL, :BL], is_transpose=True, skip_group_check=True)
                zT = ptt[:, 0:48]
                ntT = ptt[:, 64:112]
                if t < KSTEPS - 1:
                    for _ in range(KFILL - KFILL // 2):
                        pf = psB.tile([128, 512], FP, tag="ptw")
                        for kp in range(3):
                            nc.tensor.matmul(pf[:, 0:512], hp8[:, 2 * kp:2 * kp + 2, :],
                                             WhhT8[:, 2 * kp:2 * kp + 2, 512 * (fill_i % 4):512 * (fill_i % 4) + 512],
                                             start=(kp == 0), stop=(kp == 2), perf_mode=DR)
                        fill_i += 1
                dT = g1.tile([128, 8 * ET], FP, tag="dT")
                nc.vector.tensor_sub(dT[:], hp32[:].rearrange("p a b -> p (a b)"), ntT)
                zdT = g1.tile([128, 8 * ET], FP, tag="zdT")
                nc.vector.tensor_mul(zdT[:], zT, dT[:])
                hT32 = gstate.tile([128, ET, BL], FP, tag="hT32")
                nc.vector.tensor_add(hT32[:].rearrange("p a b -> p (a b)"), ntT, zdT[:])
                h8 = h8_b if t % 2 == 0 else h8_a
                nc.vector.tensor_add(h8[:, :, 0:BL],
                                     ntT.rearrange("p (a b) -> p a b", a=ET),
                                     zdT[:].rearrange("p (a b) -> p a b", a=ET))
                nc.scalar.copy(qemb8[:, :, t, :], hT32[:])
                if t == 17:
                    QT8 = prep.tile([128, ET, RQ], F8, tag="QT8")
                if 17 <= t <= 19:
                    for mt in range(2 * (t - 17), 2 * (t - 17) + 2):
                        p2 = psB.tile([128, 512], FP, tag="ptw")
                        for kp in range(3):
                            nc.tensor.matmul(p2[:, 0:128],
                                             W08[:, 2 * kp:2 * kp + 2, 128 * mt:128 * (mt + 1)],
                                             qemb8[:, 2 * kp:2 * kp + 2, 0:16, :],
                                             start=(kp == 0), stop=(kp == 2), perf_mode=DR)
                        nc.vector.tensor_scalar(QT8[:, mt, 0:128], p2[:, 0:128],
                                                b0laT[:, mt:mt + 1], None, OP.add)
                if t == 21:
                    G2Tb = prep.tile([128, ET, BL], BF, tag="G2Tb")
                if 21 <= t <= 23:
                    for mt in range(2 * (t - 21), 2 * (t - 21) + 2):
                        p2 = psB.tile([128, 512], FP, tag="ptw")
                        for kt in range(ET):
                            nc.tensor.matmul(p2[:, 0:BL], gow2[:, kt, 128 * mt:128 * (mt + 1)],
                                             clsTb[:, kt, :], start=(kt == 0),
                                             stop=(kt == ET - 1))
                        nc.vector.tensor_scalar(G2Tb[:, mt, :], p2[:, 0:BL],
                                                b2goT[:, mt:mt + 1], None, OP.add)
                if 25 <= t <= 27:
                    for mt in range(2 * (t - 25), 2 * (t - 25) + 2):
                        p2 = psB.tile([128, 512], FP, tag="ptw")
                        for kt in range(ET):
                            nc.tensor.matmul(p2[:, 0:BL], gow3[:, kt, 128 * mt:128 * (mt + 1)],
                                             G2Tb[:, kt, :], start=(kt == 0),
                                             stop=(kt == ET - 1))
                        nc.vector.tensor_scalar(goutT[:, mt, :], p2[:, 0:BL],
                                                b3goT[:, mt:mt + 1], Sgo[:, 0:1],
                                                OP.add, OP.mult)

        wb8_cm.__exit__(None, None, None)

        # ================= phase C: Q^T, W1^T, Qt^T =======================
        if PHASES >= 2:
          if True:
              for mt in range(ET):
                  p = psC.tile([128, RQ], FP, tag="pd")
                  for kp in range(3):
                      nc.tensor.matmul(p[:, 0:128],
                                       W08[:, 2 * kp:2 * kp + 2, 128 * mt:128 * (mt + 1)],
                                       qemb8[:, 2 * kp:2 * kp + 2, 16:32, :],
                                       start=(kp == 0), stop=(kp == 2), perf_mode=DR)
                  nc.vector.tensor_scalar(QT8[:, mt, 128:256], p[:, 0:128],
                                          b0laT[:, mt:mt + 1], None, OP.add)
              W1T8 = prep.tile([128, ET, D], F8, tag="W08", name="W1T8")
              for hd in range(ET):
                  for grp in range(2):
                      pt2 = psB.tile([128, 512], BF, tag="ptw")
                      for i in range(3):
                          e2 = grp * 3 + i
                          nc.tensor.matmul(pt2[:, 128 * i:128 * (i + 1)],
                                           W1n[:, e2, 128 * hd:128 * (hd + 1)],
                                           identb[:], is_transpose=True, skip_group_check=True)
                      if grp == 0:
                          nc.vector.tensor_copy(W1T8[:, hd, 0:384], pt2[:, 0:384])
                      else:
                          nc.scalar.copy(W1T8[:, hd, 384:768], pt2[:, 0:384])
              scl = 1.0 / float(np.sqrt(DK))
              for h in range(NH):
                  for mt in range(ET):
                      p = psC.tile([128, RQ], FP, tag="pd")
                      nc.tensor.matmul(p[:], W1T8[:, 3 * h:3 * h + 2, 128 * mt:128 * (mt + 1)],
                                       QT8[:, 3 * h:3 * h + 2, :],
                                       start=True, stop=False, perf_mode=DR)
                      nc.tensor.matmul(p[:], W1T8[:, 3 * h + 2, 128 * mt:128 * (mt + 1)],
                                       QT8[:, 3 * h + 2, :], start=False, stop=True)
                      dst = QtT[:, mt, :].rearrange("p (b h2 t) -> p b h2 t",
                                                    h2=NH, t=T)[:, :, h, :]
                      nc.scalar.activation(dst.rearrange("p b t -> p t b"),
                                           p[:].rearrange("p (t b) -> p t b", b=BL),
                                           AF.Copy, scale=scl)
        prep_cm.__exit__(None, None, None)

        # ================= phase D: per-b attention =======================
        de = ctx.enter_context(tc.tile_pool(name="de", bufs=1))
        pcxall = de.tile([2, BL * D], BF, tag="pcxall")
        f1 = de.tile([128, 12, 1024], BF, tag="f1")
        for c in range(12):
            gd(1 + c % 3, f1[:, c, :], f1_w[128 * c:128 * (c + 1), :])

        if PHASES >= 3:
            with tc.tile_pool(name="xb", bufs=2) as xb, \
                 tc.tile_pool(name="ab", bufs=1) as ab, \
                 tc.tile_pool(name="psD", bufs=1, space="PSUM") as psA:
              KC = kchunks(NK)
              for b in range(BL):
                  Xn = XnA[:, b, :, :]
                  XT = xb.tile([128, ET, NK], BF, tag="XT")
                  cpeng = [nc.vector.tensor_copy, nc.scalar.copy, nc.gpsimd.tensor_copy]
                  for et in range(ET):
                      for g in range(2):
                          pt = psB.tile([128, 512], BF, tag="ptw")
                          for i in range(4):
                              c = g * 4 + i
                              nc.tensor.matmul(pt[:, 128 * i:128 * (i + 1)],
                                               Xn[:, c, 128 * et:128 * (et + 1)],
                                               identb[:], is_transpose=True,
                                               skip_group_check=True)
                          w = 512 if g == 0 else NK - 512
                          cpeng[(et * 2 + g) % 2](XT[:, et, 512 * g:512 * g + w], pt[:, :w])
                  att = ab.tile([64, NK], BF, tag="att")
                  zacc = ab.tile([64, 2], FP, tag="zacc")
                  for ci, (n0, nw) in enumerate(CH_NK):
                      p = psA.tile([64, 512], FP, tag=f"wh{ci}")
                      for kt in range(ET):
                          nc.tensor.matmul(p[:, :nw],
                                           QtT[:, kt, b * 2 * T:(b + 1) * 2 * T],
                                           XT[:, kt, n0:n0 + nw],
                                           start=(kt == 0), stop=(kt == ET - 1))
                      nc.scalar.activation(att[:, n0:n0 + nw], p[:, :nw], AF.Exp,
                                           accum_out=zacc[:, ci:ci + 1])
                  zs = ab.tile([64, 1], FP, tag="zs")
                  nc.vector.tensor_add(zs[:], zacc[:, 0:1], zacc[:, 1:2])
                  rz = ab.tile([64, 1], FP, tag="rz1")
                  nc.vector.reciprocal(rz[:], zs[:])
                  wm = ab.tile([64, 2], BF, tag="wm")
                  nc.vector.tensor_scalar(wm[:], pmask[:], rz[:, 0:1], None, OP.mult)
                  pa_sb = ab.tile([2, NK], BF, tag="pa_sb")
                  for ci, (n0, nw) in enumerate(CH_NK):
                      p = psA.tile([2, 512], FP, tag=f"wh{2 + ci}")
                      nc.tensor.matmul(p[:, :nw], wm[:], att[:, n0:n0 + nw],
                                       start=True, stop=True)
                      nc.vector.tensor_copy(pa_sb[:, n0:n0 + nw], p[:, :nw])
                  paT = ab.tile([128, len(KC), 2], BF, tag="paT")
                  nc.vector.memset(paT[:].rearrange("p a b -> p (a b)"), 0.0)
                  ptp = psC.tile([128, 512], BF, tag="pd")
                  for c, (k0, kw) in enumerate(KC):
                      nc.tensor.matmul(ptp[:kw, 2 * c:2 * c + 2], pa_sb[:, k0:k0 + kw],
                                       identb[:2, :2], is_transpose=True, skip_group_check=True)
                      nc.vector.tensor_copy(paT[:kw, c, :], ptp[:kw, 2 * c:2 * c + 2])
                  for ci, (n0, nw) in enumerate(CH_D):
                      p = psA.tile([2, 512], FP, tag=f"wh{4 - ci}")
                      for c in range(len(KC)):
                          nc.tensor.matmul(p[:, :nw], paT[:, c, :],
                                           Xn[:, c, n0:n0 + nw],
                                           start=(c == 0), stop=(c == len(KC) - 1))
                      nc.vector.tensor_copy(pcxall[:, b * D + n0:b * D + n0 + nw], p[:, :nw])

        # ================= phase E: projections + MLP =====================
        if PHASES >= 4:
            with tc.tile_pool(name="tail", bufs=1) as tail:
              f2 = tail.tile([128, 8, 512], BF, tag="f2")
              for c in range(8):
                  gd(1 + c % 3, f2[:, c, :], f2_w[128 * c:128 * (c + 1), :])
              f3 = f3b
              W3 = W3b
              b2laTb = tail.tile([128, ET], BF, tag="b2laTb")
              nc.vector.tensor_copy(b2laTb[:], b2laT[:])
              vconT = tail.tile([128, ET], FP, tag="vconT")
              for mt in range(ET):
                  p = psC.tile([128, 1], FP, tag="pd")
                  for kt in range(ET):
                      nc.tensor.matmul(p[:], W3[:, kt, 128 * mt:128 * (mt + 1)],
                                       b2laTb[:, kt:kt + 1], start=(kt == 0), stop=(kt == ET - 1))
                  nc.vector.tensor_scalar(vconT[:, mt:mt + 1], p[:], b3laT[:, mt:mt + 1],
                                          Sla[:, 0:1], OP.add, OP.mult)
              pcxT = tail.tile([128, ET, 2 * BL], BF, tag="pcxT")
              ptc = psC.tile([128, 512], BF, tag="pd")
              for b2 in range(BL):
                  for kt in range(ET):
                      nc.tensor.matmul(ptc[:, 2 * (b2 * ET + kt):2 * (b2 * ET + kt) + 2],
                                       pcxall[:, b2 * D + 128 * kt:b2 * D + 128 * (kt + 1)],
                                       identb[:2, :2], is_transpose=True, skip_group_check=True)
              src_v = ptc[:, :96].rearrange("p (b a h) -> p a b h", b=BL, a=ET)
              dst_v = pcxT[:].rearrange("p a (b h) -> p a b h", h=NH)
              nc.vector.tensor_copy(dst_v, src_v)
              W2 = W2b
              pctxT = tail.tile([128, ET, BL], BF, tag="pctxT")
              pcv = pcxT[:].rearrange("p a (b h) -> p a b h", h=NH)
              for h in range(NH):
                  for mi in range(3):
                      mt = h * 3 + mi
                      p = psC.tile([128, BL], FP, tag="pd")
                      for kt in range(ET):
                          nc.tensor.matmul(p[:], W2[:, kt, 128 * mt:128 * (mt + 1)],
                                           pcv[:, kt, :, h], start=(kt == 0), stop=(kt == ET - 1))
                      nc.vector.tensor_copy(pctxT[:, mt, :], p[:])
              loT = tail.tile([128, ET, BL], BF, tag="loT")
              for mt in range(ET):
                  p = psC.tile([128, BL], FP, tag="pd")
                  for kt in range(ET):
                      nc.tensor.matmul(p[:], W3[:, kt, 128 * mt:128 * (mt + 1)],
                                       pctxT[:, kt, :], start=(kt == 0), stop=(kt == ET - 1))
                  nc.vector.tensor_scalar(loT[:, mt, :], p[:], vconT[:, mt:mt + 1], None, OP.add)

              y1T = tail.tile([128, 8, BL], BF, tag="y1T")
              for mt in range(8):
                  p = psC.tile([128, BL], FP, tag="pd")
                  for kt in range(12):
                      r_ = loT[:, kt, :] if kt < ET else goutT[:, kt - ET, :]
                      nc.tensor.matmul(p[:], f1[:, kt, 128 * mt:128 * (mt + 1)], r_,
                                       start=(kt == 0), stop=(kt == 11))
                  nc.vector.tensor_scalar(y1T[:, mt, :], p[:], b1fT[:, mt:mt + 1], None, OP.add)
              y2T = tail.tile([128, 4, BL], BF, tag="y2T")
              for mt in range(4):
                  p = psC.tile([128, BL], FP, tag="pd")
                  for kt in range(8):
                      nc.tensor.matmul(p[:], f2[:, kt, 128 * mt:128 * (mt + 1)],
                                       y1T[:, kt, :], start=(kt == 0), stop=(kt == 7))
                  nc.scalar.activation(y2T[:, mt, :], p[:], AF.Relu, bias=b2fT[:, mt:mt + 1])
              yT = tail.tile([128, 8, BL], FP, tag="yT")
              for mt in range(8):
                  p = psC.tile([128, BL], FP, tag="pd")
                  for kt in range(4):
                      nc.tensor.matmul(p[:], f3[:, kt, 128 * mt:128 * (mt + 1)],
                                       y2T[:, kt, :], start=(kt == 0), stop=(kt == 3))
                  nc.vector.tensor_scalar(yT[:, mt, :], p[:], b3fT[:, mt:mt + 1], None, OP.add)
              ynat = tail.tile([BL, 1024], FP, tag="ynat")
              for g in range(2):
                  po = psB.tile([128, 512], FP, tag="ptw")
                  for i in range(4):
                      mt = g * 4 + i
                      nc.tensor.matmul(po[:BL, 128 * i:128 * (i + 1)], yT[:, mt, :],
                                       ident[:128, :128], is_transpose=True,
                                       skip_group_check=True)
                  nc.vector.tensor_copy(ynat[:, 512 * g:512 * (g + 1)], po[:BL, :])
              nc.sync.dma_start(out_d[:, :], ynat[:])

    nc.compile()
    return nc


_NC = None


def kernel(**inputs):
    global _NC
    if _NC is None:
        _NC = build()
    B = inputs["image_local_embeds"].shape[0]
    per = B // NCORES
    in_maps = []
    for c in range(NCORES):
        sl = slice(c * per, (c + 1) * per)
        m = {
            "img": np.ascontiguousarray(np.asarray(inputs["image_local_embeds"])[sl], dtype=np.float32),
            "h0": np.ascontiguousarray(np.asarray(inputs["h0"])[sl], dtype=np.float32),
        }
        for k in ["gru_w_ih", "gru_w_hh", "gru_b_ih", "gru_b_hh", "ga_w", "ga_b",
                  "ga_pool", "la_w", "la_b", "la_pool", "go_w", "go_b", "go_pool",
                  "f1_w", "f1_b", "f2_w", "f2_b", "f3_w", "f3_b"]:
            m[k] = np.ascontiguousarray(np.asarray(inputs[k], dtype=np.float32))
        in_maps.append(m)
    res = run_bass_kernel_spmd(_NC, in_maps, core_ids=list(range(NCORES)))
    return np.concatenate([res.results[c]["out"] for c in range(NCORES)], axis=0)



# revision 89
# speedup vs baseline: 1.1178x; 1.0091x over previous
"""Trainium2 Bass kernel for nn_BiVision_VQA2 (B=64,T=32,D=768,N=901).

Data-parallel over batch: 8 batch elems per core x 8 cores.
Key math simplifications (validated vs reference in numpy, rel err ~1e-6):
  - ga/go attention use a single key token -> softmax==1 -> those paths are
    linear in cls; question_embeds is mathematically unused.
  - GRU input `a` is constant over time; wx computed once.
  - local attention: scores = (qemb @ W0_h) @ W1_h^T / sqrt(dk) @ X^T ;
    row-constant score terms (K bias, Q.b1) drop out of softmax; query
    pooling applied to the attention matrix before the @X contraction;
    constant bias terms folded into one vector.
"""

import numpy as np
from contextlib import ExitStack

import concourse.bass as bass
import concourse.tile as tile
from concourse import bacc, mybir
from concourse.bass_utils import run_bass_kernel_spmd
from concourse.masks import make_identity

FP = mybir.dt.float32
FPR = mybir.dt.float32r
OP = mybir.AluOpType
AF = mybir.ActivationFunctionType
BF = mybir.dt.bfloat16
F8 = mybir.dt.float8e4
DR = mybir.MatmulPerfMode.DoubleRow

NCORES = 8
BL = 8
D = 768
T = 32
G = 3 * D
NK = 900
NH = 2
DK = 384
ET = D // 128
RQ = BL * T
USE_FPR = True


def chunks(total):
    out, o = [], 0
    while o < total:
        w = min(512, total - o)
        out.append((o, w))
        o += w
    return out


CH_G = chunks(G)
CH_NK = [(0, 512), (512, 388)]
CH_D = [(0, 512), (512, 256)]


def _r(ap):
    return ap.bitcast(FPR) if USE_FPR else ap


def kchunks(n):
    out, o = [], 0
    while o < n:
        out.append((o, min(128, n - o)))
        o += 128
    return out


import os
PHASES = int(os.environ.get("KPHASES", "4"))


def build():
    nc = bacc.Bacc("TRN2", target_bir_lowering=False, debug=False,
                   enable_asserts=False, num_swdge_queues=4)

    def gd(q, out, in_, **kw):
        inst = nc.gpsimd.dma_start(out, in_, **kw)
        if q:
            inst.ins.queue = f"qPoolDynamic{q}"
        return inst

    img = nc.dram_tensor("img", [BL, 901, D], FP, kind="ExternalInput").ap()
    h0 = nc.dram_tensor("h0", [BL, D], FP, kind="ExternalInput").ap()
    w_ih = nc.dram_tensor("gru_w_ih", [G, D], FP, kind="ExternalInput").ap()
    w_hh = nc.dram_tensor("gru_w_hh", [G, D], FP, kind="ExternalInput").ap()
    b_ih = nc.dram_tensor("gru_b_ih", [G], FP, kind="ExternalInput").ap()
    b_hh = nc.dram_tensor("gru_b_hh", [G], FP, kind="ExternalInput").ap()
    ga_w = nc.dram_tensor("ga_w", [4, D, D], FP, kind="ExternalInput").ap()
    ga_b = nc.dram_tensor("ga_b", [4, D], FP, kind="ExternalInput").ap()
    ga_pool = nc.dram_tensor("ga_pool", [1], FP, kind="ExternalInput").ap()
    la_w = nc.dram_tensor("la_w", [4, D, D], FP, kind="ExternalInput").ap()
    la_b = nc.dram_tensor("la_b", [4, D], FP, kind="ExternalInput").ap()
    la_pool = nc.dram_tensor("la_pool", [T], FP, kind="ExternalInput").ap()
    go_w = nc.dram_tensor("go_w", [4, D, D], FP, kind="ExternalInput").ap()
    go_b = nc.dram_tensor("go_b", [4, D], FP, kind="ExternalInput").ap()
    go_pool = nc.dram_tensor("go_pool", [T], FP, kind="ExternalInput").ap()
    f1_w = nc.dram_tensor("f1_w", [2 * D, 1024], FP, kind="ExternalInput").ap()
    f1_b = nc.dram_tensor("f1_b", [1024], FP, kind="ExternalInput").ap()
    f2_w = nc.dram_tensor("f2_w", [1024, 512], FP, kind="ExternalInput").ap()
    f2_b = nc.dram_tensor("f2_b", [512], FP, kind="ExternalInput").ap()
    f3_w = nc.dram_tensor("f3_w", [512, 1024], FP, kind="ExternalInput").ap()
    f3_b = nc.dram_tensor("f3_b", [1024], FP, kind="ExternalInput").ap()
    out_d = nc.dram_tensor("out", [BL, 1024], FP, kind="ExternalOutput").ap()

    with tile.TileContext(nc) as tc, ExitStack() as ctx:
        cpool = ctx.enter_context(tc.tile_pool(name="const", bufs=1))
        gstate = ctx.enter_context(tc.tile_pool(name="gstate", bufs=2))
        xall = ctx.enter_context(tc.tile_pool(name="xall", bufs=1))
        tailw = ctx.enter_context(tc.tile_pool(name="tailw", bufs=1))
        psB = ctx.enter_context(tc.tile_pool(name="psB", bufs=2, space="PSUM"))
        psC = ctx.enter_context(tc.tile_pool(name="psC", bufs=1, space="PSUM"))

        ident = cpool.tile([128, 128], FP, tag="ident")
        make_identity(nc, ident[:])
        ones1 = cpool.tile([1, 128], FP, tag="ones1")
        nc.vector.memset(ones1[:], 1.0)
        onesT = cpool.tile([T, 128], FP, tag="onesT")
        nc.vector.memset(onesT[:], 1.0)
        identr = cpool.tile([128, 128], FP, tag="identr")
        nc.vector.tensor_copy(_r(identr[:]), ident[:])
        identb = cpool.tile([128, 128], BF, tag="identb")
        nc.vector.tensor_copy(identb[:], ident[:])
        ones1r = cpool.tile([1, 128], FP, tag="ones1r")
        nc.vector.tensor_copy(_r(ones1r[:]), ones1[:])

        def colvec(dram_1d, n, tag):
            nt = n // 128
            t_ = cpool.tile([128, nt], FP, tag=tag)
            for j in range(nt):
                nc.sync.dma_start(t_[:, j:j + 1], dram_1d[j * 128:(j + 1) * 128][:, None])
            return t_

        b2gaT = colvec(ga_b[2], D, "b2gaT")
        b3gaT = colvec(ga_b[3], D, "b3gaT")

        gapool_c = cpool.tile([1, 1], FP, tag="gapool_c")
        nc.sync.dma_start(gapool_c[:], ga_pool[:][:, None])

        def sum_bcast(vcol, k, tag):
            p = psC.tile([128, 1], FP, tag="pd")
            lhs = onesT if k == T else ones1
            nc.tensor.matmul(p[:], lhs[:k, :], vcol[:k, :], start=True, stop=True)
            s = cpool.tile([128, 1], FP, tag=tag)
            nc.vector.tensor_copy(s[:], p[:])
            return s

        Sga = sum_bcast(gapool_c, 1, "Sga")

        qemb8 = cpool.tile([128, ET, T, BL], F8, tag="qemb8")
        goutT = cpool.tile([128, ET, BL], BF, tag="goutT")
        aT = cpool.tile([128, ET, BL], FP, tag="aT")
        bhhN_r = cpool.tile([1, D], FP, tag="bhhN_r")

        # img patch tokens, all 8 batch elems, prefetched early (bf16)
        XnA = xall.tile([128, BL, 8, D], BF, tag="XnA")
        # early-persisted tail weights (DMAs issued pre-loop, overlap GRU)
        W2b = tailw.tile([128, ET, D], BF, tag="W2b")
        W3b = tailw.tile([128, ET, D], BF, tag="W3b")
        f3b = tailw.tile([128, 4, 1024], BF, tag="f3b")

        # ================= phase A: cls -> a (ga path only) ===============
        clsTb = cpool.tile([128, ET, BL], BF, tag="clsTb")
        clsT8 = cpool.tile([128, ET, BL], F8, tag="clsT8")

        def dense_T(pool, wdt, w_nat_dram, rhsT, biasT, scaleT, otile, wtag, dmaq):
            wsb = pool.tile([128, ET, D], wdt, tag=wtag)
            for c in range(ET):
                dmaq.dma_start(wsb[:, c, :], w_nat_dram[128 * c:128 * (c + 1), :])
            for mt in range(ET):
                p = psC.tile([128, BL], FP, tag="pd")
                for kt in range(ET):
                    nc.tensor.matmul(p[:], wsb[:, kt, 128 * mt:128 * (mt + 1)],
                                     rhsT[:, kt, :], start=(kt == 0), stop=(kt == ET - 1))
                if scaleT is None:
                    nc.vector.tensor_scalar(otile[:, mt, :], p[:], biasT[:, mt:mt + 1],
                                            None, OP.add)
                else:
                    nc.vector.tensor_scalar(otile[:, mt, :], p[:], biasT[:, mt:mt + 1],
                                            scaleT[:, 0:1], OP.add, OP.mult)

        with tc.tile_pool(name="ph0", bufs=1) as ph0:
            clsn = ph0.tile([BL, D], FP, tag="clsn")
            nc.sync.dma_start(clsn[:], img[0:BL, 0, :])
            ptr = psC.tile([128, 512], FP, tag="pd")
            for kt in range(ET):
                nc.tensor.matmul(ptr[:, 8 * kt:8 * kt + 8], clsn[:, 128 * kt:128 * (kt + 1)],
                                 ident[:BL, :BL], is_transpose=True, skip_group_check=True)
            clsT = ph0.tile([128, ET, BL], FP, tag="clsT")
            nc.vector.tensor_copy(clsT[:].rearrange("p a b -> p (a b)"), ptr[:, :8 * ET])
            nc.scalar.copy(clsTb[:], clsT[:])
            nc.scalar.copy(clsT8[:], clsT[:])

        # ================= phase B: GRU (fp8 DoubleRow) ===================
        cde = ctx.enter_context(tc.tile_pool(name="cde", bufs=1))
        QtT = cde.tile([128, ET, NH * RQ], BF, tag="QtT")
        prep_cm = tc.tile_pool(name="prep", bufs=1)
        prep = prep_cm.__enter__()
        wb8_cm = tc.tile_pool(name="wb8", bufs=1)
        wb8 = wb8_cm.__enter__()
        WhhT8 = wb8.tile([128, ET, G], F8, tag="WhhT8")

        with tc.tile_pool(name="wpro", bufs=1) as wpro, \
             tc.tile_pool(name="wnat", bufs=4) as wnat:
            combr = wpro.tile([1, 2 * D], FP, tag="combr")
            nc.sync.dma_start(combr[:], b_ih[0:2 * D][None, :])
            nc.gpsimd.dma_start(combr[:], b_hh[0:2 * D][None, :], accum_op=OP.add)
            bhhN_t = wpro.tile([1, D], FP, tag="bhhN_t")
            nc.sync.dma_start(bhhN_t[:], b_hh[2 * D:3 * D][None, :])
            nc.vector.tensor_copy(_r(bhhN_r[:]), bhhN_t[:])
            bihN = wpro.tile([1, D], FP, tag="bhhN_t", name="bihN")
            nc.sync.dma_start(bihN[:], b_ih[2 * D:3 * D][None, :])


            WihT8 = prep.tile([128, ET, G], F8, tag="gow3", name="WihT8")

            def build_W8(w_dram, dst):
                jts = kchunks(G)
                for g0 in range(0, len(jts), 4):
                    grp = jts[g0:g0 + 4]
                    nats = []
                    for qi, (j0, jw) in enumerate(grp):
                        wn = wnat.tile([128, D], BF, tag="wn")
                        gd(qi % 4, wn[:jw, :], w_dram[j0:j0 + jw, :])
                        nats.append((wn, j0, jw))
                    for et in range(ET):
                        pt = psB.tile([128, 512], BF, tag="ptw")
                        for i, (wn, j0, jw) in enumerate(nats):
                            nc.tensor.matmul(pt[:, 128 * i:128 * i + jw],
                                             wn[:jw, 128 * et:128 * (et + 1)],
                                             identb[:jw, :jw], is_transpose=True,
                                             skip_group_check=True)
                        w0 = grp[0][0]
                        wlen = sum(jw for (_, _, jw) in nats)
                        if et % 2 == 0:
                            nc.vector.tensor_copy(dst[:, et, w0:w0 + wlen], pt[:, :wlen])
                        else:
                            nc.scalar.copy(dst[:, et, w0:w0 + wlen], pt[:, :wlen])

            gaw2 = prep.tile([128, ET, D], F8, tag="W08", name="gaw2")
            for c in range(ET):
                gd(1 + c % 3, gaw2[:, c, :], ga_w[2][128 * c:128 * (c + 1), :])
            gaw3 = prep.tile([128, ET, D], F8, tag="gow2", name="gaw3")
            for c in range(ET):
                gd(1 + c % 3, gaw3[:, c, :], ga_w[3][128 * c:128 * (c + 1), :])

            build_W8(w_ih, WihT8)

            A2T = wpro.tile([128, ET, BL], F8, tag="A2T")
            for mt in range(ET):
                p = psC.tile([128, BL], FP, tag="pd")
                for kt in range(ET):
                    nc.tensor.matmul(p[:], gaw2[:, kt, 128 * mt:128 * (mt + 1)],
                                     clsT8[:, kt, :], start=(kt == 0), stop=(kt == ET - 1))
                nc.vector.tensor_scalar(A2T[:, mt, :], p[:], b2gaT[:, mt:mt + 1],
                                        None, OP.add)
            for mt in range(ET):
                p = psC.tile([128, BL], FP, tag="pd")
                for kt in range(ET):
                    nc.tensor.matmul(p[:], gaw3[:, kt, 128 * mt:128 * (mt + 1)],
                                     A2T[:, kt, :], start=(kt == 0), stop=(kt == ET - 1))
                nc.vector.tensor_scalar(aT[:, mt, :], p[:], b3gaT[:, mt:mt + 1],
                                        Sga[:, 0:1], OP.add, OP.mult)
            aT8 = cpool.tile([128, ET, 128], F8, tag="aT8")
            nc.vector.memset(aT8[:].rearrange("p a b -> p (a b)"), 0.0)
            nc.vector.tensor_copy(aT8[:, :, 0:BL], aT[:])

            hnat = prep.tile([BL, D], FP, tag="W08", name="hnat")
            nc.sync.dma_start(hnat[:], h0[:, :])
            ptr0 = psC.tile([128, 512], FP, tag="pd")
            for kt in range(ET):
                nc.tensor.matmul(ptr0[:, 8 * kt:8 * kt + 8], hnat[:, 128 * kt:128 * (kt + 1)],
                                 ident[:BL, :BL], is_transpose=True, skip_group_check=True)
            hT32 = gstate.tile([128, ET, BL], FP, tag="hT32")
            nc.vector.tensor_copy(hT32[:].rearrange("p a b -> p (a b)"), ptr0[:, :8 * ET])
            h8_a = cpool.tile([128, ET, 128], F8, tag="h8_a")
            h8_b = cpool.tile([128, ET, 128], F8, tag="h8_b")
            nc.vector.memset(h8_a[:].rearrange("p a b -> p (a b)"), 0.0)
            nc.vector.memset(h8_b[:].rearrange("p a b -> p (a b)"), 0.0)
            nc.scalar.copy(h8_a[:, :, 0:BL], hT32[:])
            h8 = h8_a

            # wx (+ all biases folded): rz sections get bih+bhh, n gets bih
            wxbRZ = prep.tile([BL, 2 * D], FP, tag="wxbRZ")
            wxbN = prep.tile([BL, D], FP, tag="wxbN")
            for (j0, jw) in CH_G:
                p = psC.tile([128, 512], FP, tag="pd")
                for kp in range(3):
                    nc.tensor.matmul(p[:, :jw], aT8[:, 2 * kp:2 * kp + 2, :],
                                     WihT8[:, 2 * kp:2 * kp + 2, j0:j0 + jw],
                                     start=(kp == 0), stop=False, perf_mode=DR)
                src = combr[:, j0:j0 + jw] if j0 < 2 * D else bihN[:, j0 - 2 * D:j0 - 2 * D + jw]
                nc.tensor.matmul(p[:, :jw], ones1[:1, :], src,
                                 start=False, stop=True)
                if j0 < 2 * D:
                    nc.vector.tensor_copy(_r(wxbRZ[:, j0:j0 + jw]), p[:BL, :jw])
                else:
                    nc.vector.tensor_copy(wxbN[:, j0 - 2 * D:j0 - 2 * D + jw], p[:BL, :jw])

            build_W8(w_hh, WhhT8)


        # ---- deferred small constants (off the build critical path) ------
        b2goT = colvec(go_b[2], D, "b2goT")
        b3goT = colvec(go_b[3], D, "b3goT")
        b0laT = colvec(la_b[0], D, "b0laT")
        b2laT = colvec(la_b[2], D, "b2laT")
        b3laT = colvec(la_b[3], D, "b3laT")
        b1fT = colvec(f1_b, 1024, "b1fT")
        b2fT = colvec(f2_b, 512, "b2fT")
        b3fT = colvec(f3_b, 1024, "b3fT")
        lapool_c = cpool.tile([T, 1], FP, tag="lapool_c")
        nc.sync.dma_start(lapool_c[:], la_pool[:][:, None])
        gopool_c = cpool.tile([T, 1], FP, tag="gopool_c")
        nc.sync.dma_start(gopool_c[:], go_pool[:][:, None])
        Sla = sum_bcast(lapool_c, T, "Sla")
        Sgo = sum_bcast(gopool_c, T, "Sgo")
        pmask = cpool.tile([64, 2], FP, tag="pmask")
        nc.vector.memset(pmask[:], 0.0)
        nc.sync.dma_start(pmask[0:T, 0:1], la_pool[:][:, None])
        nc.sync.dma_start(pmask[T:2 * T, 1:2], la_pool[:][:, None])

        # ---- pre-loop early DMA emission (overlaps the GRU steps) --------
        for b in range(BL):
            nc.vector.memset(XnA[:, b, 7, :], 0.0)
            for c, (k0, kw) in enumerate(kchunks(NK)):
                gd(1 + (b * 8 + c) % 3, XnA[:kw, b, c, :], img[b, 1 + k0:1 + k0 + kw, :])
        W08 = prep.tile([128, ET, D], F8, tag="W08")
        for c in range(ET):
            gd(1 + c % 3, W08[:, c, :], la_w[0][128 * c:128 * (c + 1), :])
        gow2 = prep.tile([128, ET, D], BF, tag="gow2")
        for c in range(ET):
            gd(1 + c % 3, gow2[:, c, :], go_w[2][128 * c:128 * (c + 1), :])
        for c in range(ET):
            gd(1 + c % 3, W2b[:, c, :], la_w[2][128 * c:128 * (c + 1), :])
        for c in range(ET):
            gd(1 + c % 3, W3b[:, c, :], la_w[3][128 * c:128 * (c + 1), :])
        for c in range(4):
            gd(1 + c % 3, f3b[:, c, :], f3_w[128 * c:128 * (c + 1), :])
        # stall-prone loads (wait on in-loop readers) go last on queue 0
        gow3 = prep.tile([128, ET, D], BF, tag="gow3")
        for c in range(ET):
            gd(0, gow3[:, c, :], go_w[3][128 * c:128 * (c + 1), :])
        # W1 lands in gow2's buffer once the go stage-1 matmuls are done
        W1n = prep.tile([128, ET, D], BF, tag="gow2", name="W1n")
        for c in range(ET):
            gd(0, W1n[:, c, :], la_w[1][128 * c:128 * (c + 1), :])

        with tc.tile_pool(name="g1", bufs=1) as g1, \
             tc.tile_pool(name="psG", bufs=1, space="PSUM") as psG:

            # section psums: A = j[0:1024] (r + z1), Z = j[1024:1536] (z2),
            # N = j[1536:2304] (n); emission order A0 A1 N0 N1 Z
            STEP_CHUNKS = [("a", 0, 0, 512), ("a", 512, 512, 512),
                           ("n", 0, 1536, 512), ("n", 512, 2048, 256),
                           ("z", 0, 1024, 512)]
            KSTEPS = int(os.environ.get("KSTEPS", str(T)))
            KFILL = int(os.environ.get("KFILL", "2"))
            fill_i = 0
            for t in range(KSTEPS):
                hp8, hp32 = h8, hT32
                pA = psG.tile([128, 1024], FP, tag="a")
                pN = psG.tile([128, D], FP, tag="n")
                pZ = psG.tile([128, 512], FP, tag="z")
                tiles = {"a": pA, "n": pN, "z": pZ}
                for (sec, c0, j0, jw) in STEP_CHUNKS:
                    p = tiles[sec]
                    # bias/wx add first so the last (critical) matmul is a cheap DR
                    if j0 >= 2 * D:
                        nc.tensor.matmul(p[:, c0:c0 + jw], _r(ones1r[:1, :]),
                                         _r(bhhN_r[:, j0 - 2 * D:j0 - 2 * D + jw]),
                                         start=True, stop=False)
                    else:
                        nc.tensor.matmul(p[:, c0:c0 + jw], _r(identr[:BL, :]),
                                         _r(wxbRZ[:, j0:j0 + jw]), start=True, stop=False)
                    for kp in range(3):
                        nc.tensor.matmul(p[:, c0:c0 + jw], hp8[:, 2 * kp:2 * kp + 2, :],
                                         WhhT8[:, 2 * kp:2 * kp + 2, j0:j0 + jw],
                                         start=False, stop=(kp == 2), perf_mode=DR)
                r_sig = g1.tile([BL, D], FP, tag="rsig")
                nc.scalar.activation(r_sig[:], pA[:BL, 0:768], AF.Sigmoid)
                z_nat = g1.tile([BL, D], FP, tag="znat")
                nc.scalar.activation(z_nat[:, 0:256], pA[:BL, 768:1024], AF.Sigmoid)
                rwn = g1.tile([BL, D], FP, tag="rwn")
                nc.vector.tensor_mul(rwn[:], r_sig[:], pN[:BL, :])
                npre = rwn
                nc.vector.tensor_add(npre[:], rwn[:], wxbN[:])
                nc.scalar.activation(z_nat[:, 256:768], pZ[:BL, :], AF.Sigmoid)
                nt_ = g1.tile([BL, D], FP, tag="nt")
                nc.scalar.activation(nt_[:], npre[:], AF.Tanh)
                # fills between mm block and transposes keep PE p-state hot
                for _ in range(KFILL // 2):
                    pf = psB.tile([128, 512], FP, tag="ptw")
                    for kp in range(3):
                        nc.tensor.matmul(pf[:, 0:512], hp8[:, 2 * kp:2 * kp + 2, :],
                                         WhhT8[:, 2 * kp:2 * kp + 2, 512 * (fill_i % 4):512 * (fill_i % 4) + 512],
                                         start=(kp == 0), stop=(kp == 2), perf_mode=DR)
                    fill_i += 1
                ptt = psC.tile([128, 512], FP, tag="pd")
                for kt in range(ET):
                    nc.tensor.matmul(ptt[:, 8 * kt:8 * kt + 8], z_nat[:, 128 * kt:128 * (kt + 1)],
                                     ident[:BL, :BL], is_transpose=True, skip_group_check=True)
                for kt in range(ET):
                    nc.tensor.matmul(ptt[:, 64 + 8 * kt:64 + 8 * kt + 8],
                                     nt_[:, 128 * kt:128 * (kt + 1)],
                                     ident[:BL, :BL], is_transpose=True, skip_group_check=True)
                zT = ptt[:, 0:48]
                ntT = ptt[:, 64:112]
                if t < KSTEPS - 1:
                    for _ in range(KFILL - KFILL // 2):
                        pf = psB.tile([128, 512], FP, tag="ptw")
                        for kp in range(3):
                            nc.tensor.matmul(pf[:, 0:512], hp8[:, 2 * kp:2 * kp + 2, :],
                                             WhhT8[:, 2 * kp:2 * kp + 2, 512 * (fill_i % 4):512 * (fill_i % 4) + 512],
                                             start=(kp == 0), stop=(kp == 2), perf_mode=DR)
                        fill_i += 1
                dT = g1.tile([128, 8 * ET], FP, tag="dT")
                nc.vector.tensor_sub(dT[:], hp32[:].rearrange("p a b -> p (a b)"), ntT)
                zdT = g1.tile([128, 8 * ET], FP, tag="zdT")
                nc.vector.tensor_mul(zdT[:], zT, dT[:])
                hT32 = gstate.tile([128, ET, BL], FP, tag="hT32")
                nc.vector.tensor_add(hT32[:].rearrange("p a b -> p (a b)"), ntT, zdT[:])
                h8 = h8_b if t % 2 == 0 else h8_a
                nc.vector.tensor_add(h8[:, :, 0:BL],
                                     ntT.rearrange("p (a b) -> p a b", a=ET),
                                     zdT[:].rearrange("p (a b) -> p a b", a=ET))
                nc.scalar.copy(qemb8[:, :, t, :], hT32[:])
                if t == 17:
                    QT8 = prep.tile([128, ET, RQ], F8, tag="QT8")
                if 17 <= t <= 19:
                    for mt in range(2 * (t - 17), 2 * (t - 17) + 2):
                        p2 = psB.tile([128, 512], FP, tag="ptw")
                        for kp in range(3):
                            nc.tensor.matmul(p2[:, 0:128],
                                             W08[:, 2 * kp:2 * kp + 2, 128 * mt:128 * (mt + 1)],
                                             qemb8[:, 2 * kp:2 * kp + 2, 0:16, :],
                                             start=(kp == 0), stop=(kp == 2), perf_mode=DR)
                        nc.vector.tensor_scalar(QT8[:, mt, 0:128], p2[:, 0:128],
                                                b0laT[:, mt:mt + 1], None, OP.add)
                if t == 21:
                    G2Tb = prep.tile([128, ET, BL], BF, tag="G2Tb")
                if 21 <= t <= 23:
                    for mt in range(2 * (t - 21), 2 * (t - 21) + 2):
                        p2 = psB.tile([128, 512], FP, tag="ptw")
                        for kt in range(ET):
                            nc.tensor.matmul(p2[:, 0:BL], gow2[:, kt, 128 * mt:128 * (mt + 1)],
                                             clsTb[:, kt, :], start=(kt == 0),
                                             stop=(kt == ET - 1))
                        nc.vector.tensor_scalar(G2Tb[:, mt, :], p2[:, 0:BL],
                                                b2goT[:, mt:mt + 1], None, OP.add)
                if 25 <= t <= 27:
                    for mt in range(2 * (t - 25), 2 * (t - 25) + 2):
                        p2 = psB.tile([128, 512], FP, tag="ptw")
                        for kt in range(ET):
                            nc.tensor.matmul(p2[:, 0:BL], gow3[:, kt, 128 * mt:128 * (mt + 1)],
                                             G2Tb[:, kt, :], start=(kt == 0),
                                             stop=(kt == ET - 1))
                        nc.vector.tensor_scalar(goutT[:, mt, :], p2[:, 0:BL],
                                                b3goT[:, mt:mt + 1], Sgo[:, 0:1],
                                                OP.add, OP.mult)

        wb8_cm.__exit__(None, None, None)

        # ================= phase C: Q^T, W1^T, Qt^T =======================
        if PHASES >= 2:
          if True:
              for mt in range(ET):
                  p = psC.tile([128, RQ], FP, tag="pd")
                  for kp in range(3):
                      nc.tensor.matmul(p[:, 0:128],
                                       W08[:, 2 * kp:2 * kp + 2, 128 * mt:128 * (mt + 1)],
                                       qemb8[:, 2 * kp:2 * kp + 2, 16:32, :],
                                       start=(kp == 0), stop=(kp == 2), perf_mode=DR)
                  nc.vector.tensor_scalar(QT8[:, mt, 128:256], p[:, 0:128],
                                          b0laT[:, mt:mt + 1], None, OP.add)
              W1T8 = prep.tile([128, ET, D], F8, tag="W08", name="W1T8")
              for hd in range(ET):
                  for grp in range(2):
                      pt2 = psB.tile([128, 512], BF, tag="ptw")
                      for i in range(3):
                          e2 = grp * 3 + i
                          nc.tensor.matmul(pt2[:, 128 * i:128 * (i + 1)],
                                           W1n[:, e2, 128 * hd:128 * (hd + 1)],
                                           identb[:], is_transpose=True, skip_group_check=True)
                      if grp == 0:
                          nc.vector.tensor_copy(W1T8[:, hd, 0:384], pt2[:, 0:384])
                      else:
                          nc.scalar.copy(W1T8[:, hd, 384:768], pt2[:, 0:384])
              scl = 1.0 / float(np.sqrt(DK))
              for h in range(NH):
                  for mt in range(ET):
                      p = psC.tile([128, RQ], FP, tag="pd")
                      nc.tensor.matmul(p[:], W1T8[:, 3 * h:3 * h + 2, 128 * mt:128 * (mt + 1)],
                                       QT8[:, 3 * h:3 * h + 2, :],
                                       start=True, stop=False, perf_mode=DR)
                      nc.tensor.matmul(p[:], W1T8[:, 3 * h + 2, 128 * mt:128 * (mt + 1)],
                                       QT8[:, 3 * h + 2, :], start=False, stop=True)
                      dst = QtT[:, mt, :].rearrange("p (b h2 t) -> p b h2 t",
                                                    h2=NH, t=T)[:, :, h, :]
                      nc.scalar.activation(dst.rearrange("p b t -> p t b"),
                                           p[:].rearrange("p (t b) -> p t b", b=BL),
                                           AF.Copy, scale=scl)
        prep_cm.__exit__(None, None, None)

        # ================= phase D: per-b attention =======================
        de = ctx.enter_context(tc.tile_pool(name="de", bufs=1))
        pcxall = de.tile([2, BL * D], BF, tag="pcxall")
        f1 = de.tile([128, 12, 1024], BF, tag="f1")
        for c in range(12):
            gd(1 + c % 3, f1[:, c, :], f1_w[128 * c:128 * (c + 1), :])

        if PHASES >= 3:
            with tc.tile_pool(name="xb", bufs=2) as xb, \
                 tc.tile_pool(name="ab", bufs=1) as ab, \
                 tc.tile_pool(name="psD", bufs=1, space="PSUM") as psA:
              KC = kchunks(NK)
              for b in range(BL):
                  Xn = XnA[:, b, :, :]
                  XT = xb.tile([128, ET, NK], BF, tag="XT")
                  cpeng = [nc.vector.tensor_copy, nc.scalar.copy, nc.gpsimd.tensor_copy]
                  for et in range(ET):
                      for g in range(2):
                          pt = psB.tile([128, 512], BF, tag="ptw")
                          for i in range(4):
                              c = g * 4 + i
                              nc.tensor.matmul(pt[:, 128 * i:128 * (i + 1)],
                                               Xn[:, c, 128 * et:128 * (et + 1)],
                                               identb[:], is_transpose=True,
                                               skip_group_check=True)
                          w = 512 if g == 0 else NK - 512
                          cpeng[(et * 2 + g) % 2](XT[:, et, 512 * g:512 * g + w], pt[:, :w])
                  att = ab.tile([64, NK], BF, tag="att")
                  zacc = ab.tile([64, 2], FP, tag="zacc")
                  for ci, (n0, nw) in enumerate(CH_NK):
                      p = psA.tile([64, 512], FP, tag=f"wh{ci}")
                      for kt in range(ET):
                          nc.tensor.matmul(p[:, :nw],
                                           QtT[:, kt, b * 2 * T:(b + 1) * 2 * T],
                                           XT[:, kt, n0:n0 + nw],
                                           start=(kt == 0), stop=(kt == ET - 1))
                      nc.scalar.activation(att[:, n0:n0 + nw], p[:, :nw], AF.Exp,
                                           accum_out=zacc[:, ci:ci + 1])
                  zs = ab.tile([64, 1], FP, tag="zs")
                  nc.vector.tensor_add(zs[:], zacc[:, 0:1], zacc[:, 1:2])
                  rz = ab.tile([64, 1], FP, tag="rz1")
                  nc.vector.reciprocal(rz[:], zs[:])
                  wm = ab.tile([64, 2], BF, tag="wm")
                  nc.vector.tensor_scalar(wm[:], pmask[:], rz[:, 0:1], None, OP.mult)
                  pa_sb = ab.tile([2, NK], BF, tag="pa_sb")
                  for ci, (n0, nw) in enumerate(CH_NK):
                      p = psA.tile([2, 512], FP, tag=f"wh{2 + ci}")
                      nc.tensor.matmul(p[:, :nw], wm[:], att[:, n0:n0 + nw],
                                       start=True, stop=True)
                      nc.vector.tensor_copy(pa_sb[:, n0:n0 + nw], p[:, :nw])
                  paT = ab.tile([128, len(KC), 2], BF, tag="paT")
                  nc.vector.memset(paT[:].rearrange("p a b -> p (a b)"), 0.0)
                  ptp = psC.tile([128, 512], BF, tag="pd")
                  for c, (k0, kw) in enumerate(KC):
                      nc.tensor.matmul(ptp[:kw, 2 * c:2 * c + 2], pa_sb[:, k0:k0 + kw],
                                       identb[:2, :2], is_transpose=True, skip_group_check=True)
                      nc.vector.tensor_copy(paT[:kw, c, :], ptp[:kw, 2 * c:2 * c + 2])
                  for ci, (n0, nw) in enumerate(CH_D):
                      p = psA.tile([2, 512], FP, tag=f"wh{4 - ci}")
                      for c in range(len(KC)):
                          nc.tensor.matmul(p[:, :nw], paT[:, c, :],
                                           Xn[:, c, n0:n0 + nw],
                                           start=(c == 0), stop=(c == len(KC) - 1))
                      nc.vector.tensor_copy(pcxall[:, b * D + n0:b * D + n0 + nw], p[:, :nw])

        # ================= phase E: projections + MLP =====================
        if PHASES >= 4:
            with tc.tile_pool(name="tail", bufs=1) as tail:
              f2 = tail.tile([128, 8, 512], BF, tag="f2")
              for c in range(8):
                  gd(1 + c % 3, f2[:, c, :], f2_w[128 * c:128 * (c + 1), :])
              f3 = f3b
              W3 = W3b
              b2laTb = tail.tile([128, ET], BF, tag="b2laTb")
              nc.vector.tensor_copy(b2laTb[:], b2laT[:])
              vconT = tail.tile([128, ET], FP, tag="vconT")
              for mt in range(ET):
                  p = psC.tile([128, 1], FP, tag="pd")
                  for kt in range(ET):
                      nc.tensor.matmul(p[:], W3[:, kt, 128 * mt:128 * (mt + 1)],
                                       b2laTb[:, kt:kt + 1], start=(kt == 0), stop=(kt == ET - 1))
                  nc.vector.tensor_scalar(vconT[:, mt:mt + 1], p[:], b3laT[:, mt:mt + 1],
                                          Sla[:, 0:1], OP.add, OP.mult)
              pcxT = tail.tile([128, ET, 2 * BL], BF, tag="pcxT")
              ptc = psC.tile([128, 512], BF, tag="pd")
              for b2 in range(BL):
                  for kt in range(ET):
                      nc.tensor.matmul(ptc[:, 2 * (b2 * ET + kt):2 * (b2 * ET + kt) + 2],
                                       pcxall[:, b2 * D + 128 * kt:b2 * D + 128 * (kt + 1)],
                                       identb[:2, :2], is_transpose=True, skip_group_check=True)
              src_v = ptc[:, :96].rearrange("p (b a h) -> p a b h", b=BL, a=ET)
              dst_v = pcxT[:].rearrange("p a (b h) -> p a b h", h=NH)
              nc.vector.tensor_copy(dst_v, src_v)
              W2 = W2b
              pctxT = tail.tile([128, ET, BL], BF, tag="pctxT")
              pcv = pcxT[:].rearrange("p a (b h) -> p a b h", h=NH)
              for h in range(NH):
                  for mi in range(3):
                      mt = h * 3 + mi
                      p = psC.tile([128, BL], FP, tag="pd")
                      for kt in range(ET):
                          nc.tensor.matmul(p[:], W2[:, kt, 128 * mt:128 * (mt + 1)],
                                           pcv[:, kt, :, h], start=(kt == 0), stop=(kt == ET - 1))
                      nc.vector.tensor_copy(pctxT[:, mt, :], p[:])
              loT = tail.tile([128, ET, BL], BF, tag="loT")
              for mt in range(ET):
                  p = psC.tile([128, BL], FP, tag="pd")
                  for kt in range(ET):
                      nc.tensor.matmul(p[:], W3[:, kt, 128 * mt:128 * (mt + 1)],
                                       pctxT[:, kt, :], start=(kt == 0), stop=(kt == ET - 1))
                  nc.vector.tensor_scalar(loT[:, mt, :], p[:], vconT[:, mt:mt + 1], None, OP.add)

              y1T = tail.tile([128, 8, BL], BF, tag="y1T")
              for mt in range(8):
                  p = psC.tile([128, BL], FP, tag="pd")
                  for kt in range(12):
                      r_ = loT[:, kt, :] if kt < ET else goutT[:, kt - ET, :]
                      nc.tensor.matmul(p[:], f1[:, kt, 128 * mt:128 * (mt + 1)], r_,
                                       start=(kt == 0), stop=(kt == 11))
                  nc.vector.tensor_scalar(y1T[:, mt, :], p[:], b1fT[:, mt:mt + 1], None, OP.add)
              y2T = tail.tile([128, 4, BL], BF, tag="y2T")
              for mt in range(4):
                  p = psC.tile([128, BL], FP, tag="pd")
                  for kt in range(8):
                      nc.tensor.matmul(p[:], f2[:, kt, 128 * mt:128 * (mt + 1)],
                                       y1T[:, kt, :], start=(kt == 0), stop=(kt == 7))
                  nc.scalar.activation(y2T[:, mt, :], p[:], AF.Relu, bias=b2fT[:, mt:mt + 1])
              yT = tail.tile([128, 8, BL], FP, tag="yT")
              for mt in range(8):
                  p = psC.tile([128, BL], FP, tag="pd")
                  for kt in range(4):
                      nc.tensor.matmul(p[:], f3[:, kt, 128 * mt:128 * (mt + 1)],
                                       y2T[:, kt, :], start=(kt == 0), stop=(kt == 3))
                  nc.vector.tensor_scalar(yT[:, mt, :], p[:], b3fT[:, mt:mt + 1], None, OP.add)
              ynat = tail.tile([BL, 1024], FP, tag="ynat")
              for g in range(2):
                  po = psB.tile([128, 512], FP, tag="ptw")
                  for i in range(4):
                      mt = g * 4 + i
                      nc.tensor.matmul(po[:BL, 128 * i:128 * (i + 1)], yT[:, mt, :],
                                       ident[:128, :128], is_transpose=True,
                                       skip_group_check=True)
                  nc.vector.tensor_copy(ynat[:, 512 * g:512 * (g + 1)], po[:BL, :])
              nc.sync.dma_start(out_d[:, :], ynat[:])

    nc.compile()
    return nc


_NC = None


def kernel(**inputs):
    global _NC
    if _NC is None:
        _NC = build()
    B = inputs["image_local_embeds"].shape[0]
    per = B // NCORES
    in_maps = []
    for c in range(NCORES):
        sl = slice(c * per, (c + 1) * per)
        m = {
            "img": np.ascontiguousarray(np.asarray(inputs["image_local_embeds"])[sl], dtype=np.float32),
            "h0": np.ascontiguousarray(np.asarray(inputs["h0"])[sl], dtype=np.float32),
        }
        for k in ["gru_w_ih", "gru_w_hh", "gru_b_ih", "gru_b_hh", "ga_w", "ga_b",
                  "ga_pool", "la_w", "la_b", "la_pool", "go_w", "go_b", "go_pool",
                  "f1_w", "f1_b", "f2_w", "f2_b", "f3_w", "f3_b"]:
            m[k] = np.ascontiguousarray(np.asarray(inputs[k], dtype=np.float32))
        in_maps.append(m)
    res = run_bass_kernel_spmd(_NC, in_maps, core_ids=list(range(NCORES)))
    return np.concatenate([res.results[c]["out"] for c in range(NCORES)], axis=0)

